# revision 1
# baseline (speedup 1.0000x reference)
"""DNC MemoryAccess kernel for Trainium2 (Bass/Tile), data-parallel over batch.

Shapes (hardcoded): B=8, T=16, C=1024, IFACE=471, N=512, WORD=64, R=4, NW=1.
Each of the 8 cores processes one batch element; all recurrent state
(memory [64,512]T, link [512,512], usage/prec [1,512], read_w [4,512])
stays SBUF-resident across the T=16 sequential steps.

Precision notes: ACT-table exp has ~1e-5 max rel err, enough to flip the
DNC allocation sort on near-tied usage values. So: strengths/key-norms are
precomputed in the prologue with a full-precision polynomial exp, per-step
norms use Newton-refined rsqrt, and the three per-step exps are refined
with one ln-based correction: z = y*(1 + x - ln(y)).
"""
import sys

sys.path.insert(0, "/opt/trn_rl_repo")

import numpy as np

import concourse.bacc as bacc
import concourse.bass as bass
import concourse.mybir as mybir
import concourse.tile as tile

F32 = mybir.dt.float32
F16 = mybir.dt.float16
I32 = mybir.dt.int32
AF = mybir.ActivationFunctionType
OP = mybir.AluOpType

B, T, C, IF = 8, 16, 1024, 471
N, W, R = 512, 64, 4
NT = N // 128  # 4 N-tiles
LOG2E = 1.4426950408889634
MAGIC2 = 12582912.0 + 127.0  # round-to-int magic + exponent bias for 2^k bits
_LN2 = 0.6931471805599453
# 2^f = 1 + sum_{i>=1} EXPC[i-1] f^i  (Taylor of exp(f ln2); deg-6 err ~2e-9)
EXPC = [_LN2, _LN2**2 / 2, _LN2**3 / 6, _LN2**4 / 24, _LN2**5 / 120,
        _LN2**6 / 720]

# iface field offsets
O_RK, O_RS, O_WK, O_WS = 0, 256, 260, 324
O_ER, O_WV, O_FG, O_AG, O_WG, O_MD = 325, 389, 453, 457, 458, 459


def build_nc():
    nc = bacc.Bacc("TRN2", target_bir_lowering=False, debug=False, num_devices=8)

    co_d = nc.declare_dram_parameter("co", [T, C], F32, isOutput=False)
    w_d = nc.declare_dram_parameter("wif", [C, IF], F32, isOutput=False)
    b_d = nc.declare_dram_parameter("bif", [1, IF], F32, isOutput=False)
    m0_d = nc.declare_dram_parameter("mem0", [N, W], F32, isOutput=False)
    ident_d = nc.declare_dram_parameter("ident", [128, 128], F32, isOutput=False)
    ones_d = nc.declare_dram_parameter("ones", [128, 128], F32, isOutput=False)
    offd_d = nc.declare_dram_parameter("offdiag", [N, N], F16, isOutput=False)
    out_d = nc.declare_dram_parameter("out", [T, R, W], F32, isOutput=True)

    with tile.TileContext(nc) as tc:
        with (
            tc.tile_pool(name="const", bufs=1) as cp,
            tc.tile_pool(name="state", bufs=2) as sp,
            tc.tile_pool(name="work", bufs=2) as wp,
            tc.tile_pool(name="psP", bufs=3, space="PSUM") as psA,
            tc.tile_pool(name="psM", bufs=3, space="PSUM") as psM,
            tc.tile_pool(name="psS", bufs=1, space="PSUM") as psS,
        ):
            _build_body(nc, tc, cp, sp, wp, psA, psM, psS,
                        co_d, w_d, b_d, m0_d, ident_d, ones_d, offd_d, out_d)
    nc.compile()
    return nc


def _helpers(nc):
    V, S = nc.vector, nc.scalar

    def pexp(pool, x_ap, shape, tg, nb=1):
        """exp(x) to ~1e-7 via 2^(x*log2e): magic rounding + deg-6 poly +
        exponent-bit assembly. ~13 DVE ops; prologue/small-tensor use."""
        t_ = pool.tile(shape, F32, tag=tg + "_t", name=tg + "_t")
        V.tensor_scalar(t_[:], x_ap, LOG2E, None, op0=OP.mult)
        a_ = pool.tile(shape, F32, tag=tg + "_a", name=tg + "_a")
        V.tensor_scalar(a_[:], t_[:], MAGIC2, None, op0=OP.add)
        k_ = pool.tile(shape, F32, tag=tg + "_k", name=tg + "_k")
        V.tensor_scalar(k_[:], a_[:], MAGIC2, None, op0=OP.subtract)
        f_ = pool.tile(shape, F32, tag=tg + "_f", name=tg + "_f")
        V.tensor_tensor(f_[:], t_[:], k_[:], op=OP.subtract)
        p2 = pool.tile(shape, I32, tag=tg + "_p2", name=tg + "_p2")
        V.tensor_scalar(p2[:], a_[:].bitcast(I32), 23, None,
                        op0=OP.arith_shift_left)
        ac = [pool.tile(shape, F32, tag=tg + "_ac0", name=tg + "_ac0"),
              pool.tile(shape, F32, tag=tg + "_ac1", name=tg + "_ac1")]
        V.tensor_scalar(ac[0][:], f_[:], EXPC[5], None, op0=OP.mult)
        cur = 0
        for c_ in (EXPC[4], EXPC[3], EXPC[2], EXPC[1], EXPC[0]):
            V.scalar_tensor_tensor(ac[1 - cur][:], ac[cur][:], c_, f_[:],
                                   op0=OP.add, op1=OP.mult)
            cur = 1 - cur
        y_ = pool.tile(shape, F32, tag=tg + "_y", name=tg + "_y")
        V.scalar_tensor_tensor(y_[:], ac[cur][:], 1.0, p2[:].bitcast(F32),
                               op0=OP.add, op1=OP.mult)
        return y_

    def pexp2(pool, x_ap, shape, tg, accum_out=None, out=None, clamp=None):
        """2^x for prescaled x (log2 units); magic-round + deg-6 poly +
        exponent bits. All DVE; ~1e-7. x_ap should be SBUF."""
        if clamp is not None:
            t_ = pool.tile(shape, F32, tag=tg + "_t", name=tg + "_t", bufs=1)
            V.tensor_scalar(t_[:], x_ap, clamp, None, op0=OP.max)
            x_ap = t_[:]
        a_ = pool.tile(shape, F32, tag=tg + "_a", name=tg + "_a", bufs=1)
        V.tensor_scalar(a_[:], x_ap, MAGIC2, None, op0=OP.add)
        fn = pool.tile(shape, F32, tag=tg + "_fn", name=tg + "_fn", bufs=1)
        V.scalar_tensor_tensor(fn[:], a_[:], MAGIC2, x_ap,
                               op0=OP.subtract, op1=OP.subtract)  # -f
        p2 = pool.tile(shape, I32, tag=tg + "_p2", name=tg + "_p2", bufs=1)
        V.tensor_scalar(p2[:], a_[:].bitcast(I32), 23, None,
                        op0=OP.arith_shift_left)
        ac = [pool.tile(shape, F32, tag=tg + "_a0", name=tg + "_a0", bufs=1),
              pool.tile(shape, F32, tag=tg + "_a1", name=tg + "_a1", bufs=1)]
        V.tensor_scalar(ac[0][:], fn[:], EXPC[5], None, op0=OP.mult)
        cur = 0
        for i, c_ in ((5, EXPC[4]), (4, EXPC[3]), (3, EXPC[2]), (2, EXPC[1]),
                      (1, EXPC[0])):
            b_ = c_ if (i % 2 == 0) else -c_
            V.scalar_tensor_tensor(ac[1 - cur][:], ac[cur][:], b_, fn[:],
                                   op0=OP.add, op1=OP.mult)
            cur = 1 - cur
        if out is None:
            out_t = pool.tile(shape, F32, tag=tg + "_y", name=tg + "_y", bufs=1)
            out = out_t[:]
        V.scalar_tensor_tensor(out, ac[cur][:], 1.0, p2[:].bitcast(F32),
                               op0=OP.add, op1=OP.mult, accum_out=accum_out)
        return out

    def rsqrt_pm(pool, x_psum, shape, tg, out=None, iters=3):
        """rsqrt via quake seed + Newton; all DVE/gpsimd, no ACT.
        Seeds from an SBUF copy (no int ops on PSUM views)."""
        xs = pool.tile(shape, F32, tag=tg + "_xs", name=tg + "_xs", bufs=1)
        V.tensor_copy(xs[:], x_psum)
        sh = pool.tile(shape, I32, tag=tg + "_sh", name=tg + "_sh", bufs=1)
        V.tensor_scalar(sh[:], xs[:].bitcast(I32), 1, None,
                        op0=OP.arith_shift_right)
        nb = pool.tile(shape, I32, tag=tg + "_nb", name=tg + "_nb", bufs=1)
        V.tensor_scalar(nb[:], sh[:], -1, None, op0=OP.bitwise_xor)
        y_ = pool.tile(shape, F32, tag=tg + "_y", name=tg + "_y", bufs=1)
        V.tensor_scalar(y_[:].bitcast(I32), nb[:], 0x5f3759e0, None, op0=OP.add)
        for i in range(iters):
            s_ = pool.tile(shape, F32, tag=tg + f"_s{i}", name=tg + f"_s{i}", bufs=1)
            nc.gpsimd.tensor_tensor(s_[:], y_[:], y_[:], op=OP.mult)
            t_ = pool.tile(shape, F32, tag=tg + f"_t{i}", name=tg + f"_t{i}", bufs=1)
            V.tensor_tensor(t_[:], xs[:], s_[:], op=OP.mult)
            h_ = pool.tile(shape, F32, tag=tg + f"_h{i}", name=tg + f"_h{i}", bufs=1)
            V.tensor_scalar(h_[:], t_[:], -0.5, 1.5, op0=OP.mult, op1=OP.add)
            if i == iters - 1 and out is not None:
                V.tensor_tensor(out, y_[:], h_[:], op=OP.mult)
                return out
            y2 = pool.tile(shape, F32, tag=tg + f"_y{i}", name=tg + f"_y{i}", bufs=1)
            V.tensor_tensor(y2[:], y_[:], h_[:], op=OP.mult)
            y_ = y2
        return y_[:]

    def softplus_precise(pool, x_ap, shape, tg):
        """ln(1+e^x) with table-ln seed + one Newton step (via pexp)."""
        e_ = pexp(pool, x_ap, shape, tg + "e")
        w_ = pool.tile(shape, F32, tag=tg + "_w", name=tg + "_w")
        V.tensor_scalar(w_[:], e_[:], 1.0, None, op0=OP.add)
        z_ = pool.tile(shape, F32, tag=tg + "_z", name=tg + "_z")
        S.activation(z_[:], w_[:], AF.Ln)
        nz = pool.tile(shape, F32, tag=tg + "_nz", name=tg + "_nz")
        S.mul(nz[:], z_[:], -1.0)
        e2 = pexp(pool, nz[:], shape, tg + "e2")
        m_ = pool.tile(shape, F32, tag=tg + "_m", name=tg + "_m")
        V.tensor_tensor(m_[:], w_[:], e2[:], op=OP.mult)
        o_ = pool.tile(shape, F32, tag=tg + "_o", name=tg + "_o")
        V.scalar_tensor_tensor(o_[:], m_[:], -1.0, z_[:], op0=OP.add, op1=OP.add)
        return o_

    def rsqrt_refined(pool, x_ap, shape, tg, iters=1, nb=1):
        """rsqrt(x): ACT-table seed exp(-0.5 ln x) + Newton (no division)."""
        l_ = pool.tile(shape, F32, tag=tg + "_l", name=tg + "_l", bufs=nb)
        S.activation(l_[:], x_ap, AF.Ln)
        y_ = pool.tile(shape, F32, tag=tg + "_y", name=tg + "_y", bufs=nb)
        S.activation(y_[:], l_[:], AF.Exp, scale=-0.5)
        for i in range(iters):
            s_ = pool.tile(shape, F32, tag=tg + f"_s{i}", name=tg + f"_s{i}", bufs=nb)
            nc.gpsimd.tensor_tensor(s_[:], y_[:], y_[:], op=OP.mult)
            t_ = pool.tile(shape, F32, tag=tg + f"_t{i}", name=tg + f"_t{i}", bufs=nb)
            V.tensor_tensor(t_[:], x_ap, s_[:], op=OP.mult)
            h_ = pool.tile(shape, F32, tag=tg + f"_h{i}", name=tg + f"_h{i}", bufs=nb)
            V.tensor_scalar(h_[:], t_[:], -0.5, 1.5, op0=OP.mult, op1=OP.add)
            y2 = pool.tile(shape, F32, tag=tg + f"_y{i}", name=tg + f"_y{i}", bufs=nb)
            V.tensor_tensor(y2[:], y_[:], h_[:], op=OP.mult)
            y_ = y2
        return y_

    def rsqrt_refined_into(pool, x_ap, shape, tg, out, iters=1):
        y_ = rsqrt_refined(pool, x_ap, shape, tg, iters=iters - 1) if iters > 1 \
            else None
        if y_ is None:
            l_ = pool.tile(shape, F32, tag=tg + "_l", name=tg + "_l", bufs=1)
            S.activation(l_[:], x_ap, AF.Ln)
            y0 = pool.tile(shape, F32, tag=tg + "_y", name=tg + "_y", bufs=1)
            S.activation(y0[:], l_[:], AF.Exp, scale=-0.5)
            y_ = y0
        s_ = pool.tile(shape, F32, tag=tg + "_sf", name=tg + "_sf", bufs=1)
        nc.gpsimd.tensor_tensor(s_[:], y_[:], y_[:], op=OP.mult)
        t_ = pool.tile(shape, F32, tag=tg + "_tf", name=tg + "_tf", bufs=1)
        V.tensor_tensor(t_[:], x_ap, s_[:], op=OP.mult)
        h_ = pool.tile(shape, F32, tag=tg + "_hf", name=tg + "_hf", bufs=1)
        V.tensor_scalar(h_[:], t_[:], -0.5, 1.5, op0=OP.mult, op1=OP.add)
        V.tensor_tensor(out, y_[:], h_[:], op=OP.mult)
        return out

    def exp_refined(pool, x_ap, shape, tg, out, accum_out=None):
        """exp(x) = table seed y, then out = y*(1 + x - ln(y))."""
        y_ = pool.tile(shape, F32, tag=tg + "_y", name=tg + "_y", bufs=1)
        S.activation(y_[:], x_ap, AF.Exp)
        ly = pool.tile(shape, F32, tag=tg + "_ly", name=tg + "_ly", bufs=1)
        S.activation(ly[:], y_[:], AF.Ln)
        d_ = pool.tile(shape, F32, tag=tg + "_d", name=tg + "_d", bufs=1)
        deng = nc.gpsimd if x_ap.tensor.space == bass.MemorySpace.SBUF else V
        deng.tensor_tensor(d_[:], x_ap, ly[:], op=OP.subtract)
        V.scalar_tensor_tensor(out, d_[:], 1.0, y_[:], op0=OP.add,
                               op1=OP.mult, accum_out=accum_out)
        return out

    return (pexp, softplus_precise, rsqrt_refined, exp_refined,
            rsqrt_refined_into, pexp2, rsqrt_pm)


def _build_body(nc, tc, cp, sp, wp, psA, psM, psS,
                co_d, w_d, b_d, m0_d, ident_d, ones_d, offd_d, out_d,
                dbg_d=None):
    V, S, P, DMA = nc.vector, nc.scalar, nc.tensor, nc.sync
    (pexp, softplus_precise, rsqrt_refined, exp_refined,
     rsqrt_refined_into, pexp2, rsqrt_pm) = _helpers(nc)

    # ---------------- constants ----------------
    ident = cp.tile([128, 128], F32)
    DMA.dma_start(ident[:], ident_d[:])
    ones = cp.tile([128, 128], F32)
    DMA.dma_start(ones[:], ones_d[:])
    offd = cp.tile([128, NT, N], F16)
    for c in range(NT):
        DMA.dma_start(offd[:, c, :], offd_d[128 * c:128 * (c + 1), :])
    nege0_pm = cp.tile([128, NT], F32)
    nc.gpsimd.memset(nege0_pm[:], 0.0)
    nc.gpsimd.memset(nege0_pm[0:1, 0:1], -1.0)

    # persistent per-t parse tables
    rkT = cp.tile([W, R, T], F32)
    wkT = cp.tile([W, T], F32)
    neg_er = cp.tile([W, T], F32)
    wvT = cp.tile([W, T], F32)
    bkw = cp.tile([1, T], F32)       # softplus(ws)/||wk||
    wkT2 = cp.tile([W, T], F32)      # wk * bkw * log2e
    rkT2 = cp.tile([W, R, T], F32)   # rk * bkr * log2e
    fgF = cp.tile([1, R, T], F32)
    c1n = cp.tile([1, T], F32)
    c2 = cp.tile([1, T], F32)
    modes = cp.tile([R, 3, T], F32)

    # ---------------- prologue: iface (transient pool) ----------------
    with tc.tile_pool(name="prolog", bufs=1) as pp:
        co_sb = pp.tile([T, C], F32)
        DMA.dma_start(co_sb[:], co_d[:])
        bif_sb = pp.tile([1, IF], F32)
        DMA.dma_start(bif_sb[:], b_d[:])

        coT_p = psA.tile([128, 8, T], F32, tag="p")
        for k in range(8):
            P.transpose(coT_p[:, k, :], co_sb[:, 128 * k:128 * (k + 1)],
                        ident[0:T, 0:T])
        coT = pp.tile([128, 8, T], F32)
        S.copy(coT[:], coT_p[:])

        if_p = psS.tile([T, IF], F32, tag="s")
        for h in range(2):
            w_sb = pp.tile([128, 4, IF], F32, tag="w_sb", name=f"w_sb{h}")
            for k in range(4):
                DMA.dma_start(w_sb[:, k, :],
                              w_d[128 * (4 * h + k):128 * (4 * h + k + 1), :])
            for k in range(4):
                P.matmul(if_p[:], coT[:, 4 * h + k, :], w_sb[:, k, :],
                         start=(h == 0 and k == 0), stop=False)
        P.matmul(if_p[:], ones[0:1, 0:T], bif_sb[:], start=False, stop=True)
        iface = pp.tile([T, IF], F32)
        S.copy(iface[:], if_p[:])

        # field transposes -> per-t column layouts
        def tp_field(lo, hi):
            n = hi - lo
            pt = psA.tile([128, T], F32, tag="p")
            P.transpose(pt[0:n, :], iface[:, lo:hi], ident[0:T, 0:T])
            return pt

        for r in range(R):
            pt = tp_field(O_RK + W * r, O_RK + W * (r + 1))
            S.copy(rkT[:, r, :], pt[0:W, :])
        pt = tp_field(O_WK, O_WK + W)
        S.copy(wkT[:], pt[0:W, :])
        pt = tp_field(O_ER, O_ER + W)
        er_in = pp.tile([W, T], F32)
        V.tensor_scalar(er_in[:], pt[0:W, :], -1.0, None, op0=OP.mult)
        er_e = pexp(pp, er_in[:], [W, T], "sge")     # e^{-x}
        er_w = pp.tile([W, T], F32)
        V.tensor_scalar(er_w[:], er_e[:], 1.0, None, op0=OP.add)
        er_r = pp.tile([W, T], F32)
        V.reciprocal(er_r[:], er_w[:])               # sigmoid(x)
        V.tensor_scalar(neg_er[:], er_r[:], -1.0, None, op0=OP.mult)
        pt = tp_field(O_WV, O_WV + W)
        S.copy(wvT[:], pt[0:W, :])

        # strengths / ||k|| folded: bkw, bkr  (free-layout pipeline)
        rsF_p = psA.tile([1, R, T], F32, tag="p")
        for r in range(R):
            P.transpose(rsF_p[0:1, r, :], iface[:, O_RS + r:O_RS + r + 1],
                        ident[0:T, 0:T])
        rsF = pp.tile([1, R, T], F32)
        S.copy(rsF[:], rsF_p[:])
        wsF_p = psA.tile([1, T], F32, tag="p")
        P.transpose(wsF_p[:], iface[:, O_WS:O_WS + 1], ident[0:T, 0:T])
        wsF = pp.tile([1, T], F32)
        S.copy(wsF[:], wsF_p[:])
        rs_pre = softplus_precise(pp, rsF[:].rearrange("o r t -> o (r t)"),
                                  [1, R * T], "rsp")
        ws_pre = softplus_precise(pp, wsF[:], [1, T], "wsp")

        sqw = pp.tile([W, T], F32)
        S.square(sqw[:], wkT[:])
        wk2_p = psM.tile([1, T], F32, tag="m")
        P.matmul(wk2_p[:], ones[0:W, 0:1], sqw[:])
        wkr = rsqrt_refined(pp, wk2_p[:], [1, T], "wkr", iters=2)
        V.tensor_tensor(bkw[:], ws_pre[:], wkr[:], op=OP.mult)

        sqr = pp.tile([W, R, T], F32)
        S.square(sqr[:], rkT[:])
        rk2_p = psM.tile([1, R * T], F32, tag="m")
        P.matmul(rk2_p[:], ones[0:W, 0:1], sqr[:].rearrange("w r t -> w (r t)"))
        rkr = rsqrt_refined(pp, rk2_p[:], [1, R * T], "rkr", iters=2)
        bkrF = cp.tile([1, R, T], F32, name="bkrF")
        V.tensor_tensor(bkrF[:].rearrange("o r t -> o (r t)"), rs_pre[:],
                        rkr[:], op=OP.mult)
        # scaled keys: wkT2 = wkT * bkw * log2e ; rkT2 = rkT * bkr * log2e
        bkwb_p = psA.tile([W, T], F32, tag="p")
        P.matmul(bkwb_p[:], ones[0:1, 0:W], bkw[:])
        V.scalar_tensor_tensor(wkT2[:], wkT[:], LOG2E, bkwb_p[:],
                               op0=OP.mult, op1=OP.mult)
        bkrb_p = psA.tile([W, R * T], F32, tag="p")
        P.matmul(bkrb_p[:], ones[0:1, 0:W], bkrF[:].rearrange("o r t -> o (r t)"))
        V.scalar_tensor_tensor(rkT2[:].rearrange("w r t -> w (r t)"),
                               rkT[:].rearrange("w r t -> w (r t)"), LOG2E,
                               bkrb_p[:], op0=OP.mult, op1=OP.mult)

        # gates
        # fg, ag, wg sigmoids via precise V pipeline, packed in one [1,6,T]
        gats_p = psA.tile([1, 6, T], F32, tag="p")
        for r in range(R):
            P.transpose(gats_p[0:1, r, :], iface[:, O_FG + r:O_FG + r + 1],
                        ident[0:T, 0:T])
        P.transpose(gats_p[0:1, 4, :], iface[:, O_AG:O_AG + 1], ident[0:T, 0:T])
        P.transpose(gats_p[0:1, 5, :], iface[:, O_WG:O_WG + 1], ident[0:T, 0:T])
        g_in = pp.tile([1, 6 * T], F32)
        V.tensor_scalar(g_in[:], gats_p[:].rearrange("o g t -> o (g t)"), -1.0,
                        None, op0=OP.mult)
        g_e = pexp(pp, g_in[:], [1, 6 * T], "sgg")
        g_w = pp.tile([1, 6 * T], F32)
        V.tensor_scalar(g_w[:], g_e[:], 1.0, None, op0=OP.add)
        g_r = pp.tile([1, 6, T], F32)
        V.reciprocal(g_r[:].rearrange("o g t -> o (g t)"), g_w[:])
        V.tensor_copy(fgF[:], g_r[0:1, 0:R, :])
        ag_t = g_r[0:1, 4, :]
        wg_t = g_r[0:1, 5, :]
        c1t = pp.tile([1, T], F32)
        V.tensor_tensor(c1t[:], ag_t, wg_t, op=OP.mult)
        V.tensor_scalar(c1n[:], c1t[:], -1.0, None, op0=OP.mult)
        V.tensor_tensor(c2[:], wg_t, c1t[:], op=OP.subtract)

        # modes softmax (precise exp; normalize in [T,12]; m-major; transpose)
        me = pexp(pp, iface[:, O_MD:O_MD + 12], [T, 12], "me")
        me3 = me[:].rearrange("t (r m) -> t r m", m=3)
        msum = pp.tile([T, R], F32)
        V.tensor_tensor(msum[:], me3[:, :, 0], me3[:, :, 1], op=OP.add)
        V.tensor_tensor(msum[:], msum[:], me3[:, :, 2], op=OP.add)
        mrs = pp.tile([T, R], F32)
        V.reciprocal(mrs[:], msum[:])
        mn = pp.tile([T, 12], F32)
        mn3 = mn[:].rearrange("t (r m) -> t r m", m=3)
        for m in range(3):
            V.tensor_tensor(mn3[:, :, m], me3[:, :, m], mrs[:], op=OP.mult)
        mo = pp.tile([T, 12], F32)
        mo3 = mo[:].rearrange("t (m r) -> t m r", r=R)
        S.copy(mo3[:], mn3[:].rearrange("t r m -> t m r"))
        modes_p = psA.tile([R, 3, T], F32, tag="p")
        for m in range(3):
            P.transpose(modes_p[:, m, :], mo[:, 4 * m:4 * (m + 1)],
                        ident[0:T, 0:T])
        S.copy(modes[:], modes_p[:])

    # ---------------- initial state ----------------
    mem_nrm = sp.tile([128, NT, W], F32, tag="mem_nrm")
    for c in range(NT):
        DMA.dma_start(mem_nrm[:, c, :], m0_d[128 * c:128 * (c + 1), :])
    memT_p = psA.tile([W, N], F32, tag="p")
    for c in range(NT):
        P.transpose(memT_p[:, 128 * c:128 * (c + 1)], mem_nrm[:, c, :], ident[:])
    memT = sp.tile([W, N], F32, tag="memT")
    S.copy(memT[:], memT_p[:])

    sqm = wp.tile([W, N], F32, tag="sqm", bufs=1)
    nc.gpsimd.tensor_tensor(sqm[:], memT[:], memT[:], op=OP.mult)
    msf_p0 = psM.tile([1, N], F32, tag="m")
    P.matmul(msf_p0[:], ones[0:W, 0:1], sqm[:])
    msf0 = wp.tile([1, N], F32, tag="msf")
    V.tensor_copy(msf0[:], msf_p0[:])
    ms_tp0 = psA.tile([128, NT], F32, tag="p")
    for c in range(NT):
        P.transpose(ms_tp0[:, c:c + 1], msf0[0:1, 128 * c:128 * (c + 1)],
                    ident[0:1, 0:1])
    mnorm = sp.tile([128, NT], F32, tag="mnorm")
    rsqrt_pm(wp, ms_tp0[:], [128, NT], "w1", out=mnorm[:], iters=3)

    L = sp.tile([128, NT, N], F32, tag="L")
    nc.gpsimd.memset(L[:], 0.0)
    LT0 = sp.tile([128, NT, N], F32, tag="LT")
    nc.gpsimd.memset(LT0[:], 0.0)
    u0 = sp.tile([1, N], F32, tag="u")
    nc.gpsimd.memset(u0[:], 0.0)
    rw0 = sp.tile([R, N], F32, tag="rw")
    nc.gpsimd.memset(rw0[:], 0.0)
    rwT0 = sp.tile([128, NT * R], F32, tag="rwT")
    nc.gpsimd.memset(rwT0[:], 0.0)

    out_sb = cp.tile([R, T, W], F32)
    dbg_sb = None

    st = dict(memT=memT, mem_nrm=mem_nrm, mnorm=mnorm, L=L, LT=LT0,
              u=u0, prec=None, rw=rw0, rwT=rwT0)

    for t in range(T):
        st = _step(nc, t, st, cp, sp, wp, psA, psM, psS,
                   ident, ones, offd, nege0_pm, wkT2, rkT2, neg_er, wvT,
                   fgF, c1n, c2, modes, out_sb, pexp2, rsqrt_pm)

    DMA.dma_start(out_d[:].rearrange("t r w -> r t w"), out_sb[:])


def _step(nc, t, st, cp, sp, wp, psA, psM, psS,
          ident, ones, offd, nege0_pm, wkT2, rkT2, neg_er, wvT,
          fgF, c1n, c2, modes, out_sb, pexp2, rsqrt_pm):
    V, S, P = nc.vector, nc.scalar, nc.tensor
    memT, mem_nrm, mnorm = st["memT"], st["mem_nrm"], st["mnorm"]
    L, LT, u, prec, rw, rwT = st["L"], st["LT"], st["u"], st["prec"], st["rw"], st["rwT"]
    last = (t == T - 1)

    # ---- write content weights (PM layout; FL matmul + transposes) ----
    wdf_p = psM.tile([1, N], F32, tag="m")
    P.matmul(wdf_p[:], wkT2[:, t:t + 1], memT[:])
    wdf = wp.tile([1, N], F32, tag="wdf")
    V.tensor_copy(wdf[:], wdf_p[:])
    wdots_p = psA.tile([128, NT], F32, tag="p")
    for c in range(NT):
        P.transpose(wdots_p[:, c:c + 1], wdf[0:1, 128 * c:128 * (c + 1)],
                    ident[0:1, 0:1])
    wlog = wp.tile([128, NT], F32, tag="wlog")
    V.tensor_tensor(wlog[:], wdots_p[:], mnorm[:], op=OP.mult)
    wpart = wp.tile([128, 1], F32, tag="wpart")
    wexp = wp.tile([128, NT], F32, tag="wexp")
    pexp2(wp, wlog[:], [128, NT], "wex", accum_out=wpart[:], out=wexp[:])
    wsf_p = psS.tile([1, 128], F32, tag="s2")
    P.transpose(wsf_p[:], wpart[:], ident[:])
    wsum = wp.tile([1, 1], F32, tag="wsum")
    V.tensor_reduce(wsum[:], wsf_p[:], axis=mybir.AxisListType.X, op=OP.add)
    wrs = wp.tile([1, 1], F32, tag="wrs")
    V.reciprocal(wrs[:], wsum[:])

    # ---- allocation weighting ----
    if t == 0:
        negalloc = None
        omu_a = None
    else:
        u_tp = psA.tile([128, NT], F32, tag="p")
        for c in range(NT):
            P.transpose(u_tp[:, c:c + 1], u[0:1, 128 * c:128 * (c + 1)],
                        ident[0:1, 0:1])
        u_pm = wp.tile([128, NT], F32, tag="u_pm")
        V.tensor_copy(u_pm[:], u_tp[:])
        lu_pm = wp.tile([128, NT], F32, tag="lu_pm")
        S.activation(lu_pm[:], u_pm[:], AF.Ln)
        lu2 = wp.tile([128, NT], F32, tag="lu2")
        V.tensor_scalar(lu2[:], lu_pm[:], LOG2E, None, op0=OP.mult)
        ub_p = psA.tile([128, N], F32, tag="p")
        P.matmul(ub_p[:], ones[0:1, :], u[:])
        G = wp.tile([128, NT, N], F32, tag="G", bufs=1)
        for c in range(NT):
            V.tensor_scalar(G[:, c, :], ub_p[:], u_pm[:, c:c + 1], None, op0=OP.is_gt)
        s_p = psM.tile([1, N], F32, tag="m")
        for c in range(NT):
            P.matmul(s_p[:], lu2[:, c:c + 1], G[:, c, :],
                     start=(c == 0), stop=(c == NT - 1))
        s_f = wp.tile([1, N], F32, tag="s_f")
        V.tensor_copy(s_f[:], s_p[:])
        s_tp = psA.tile([128, NT], F32, tag="p")
        for c in range(NT):
            P.transpose(s_tp[:, c:c + 1], s_f[0:1, 128 * c:128 * (c + 1)],
                        ident[0:1, 0:1])
        s_pm = wp.tile([128, NT], F32, tag="s_pm")
        V.tensor_copy(s_pm[:], s_tp[:])
        es = pexp2(wp, s_pm[:], [128, NT], "es", clamp=-115.0)
        negalloc = wp.tile([128, NT], F32, tag="negalloc")
        V.scalar_tensor_tensor(negalloc[:], u_pm[:], 1.0, es,
                               op0=OP.subtract, op1=OP.mult)
        omu_a = wp.tile([1, N], F32, tag="omu")
        V.tensor_scalar(omu_a[:], u[:], -1.0, 1.0, op0=OP.mult, op1=OP.add)

    # ---- write weights ww (PM, then flatten via proven transposes) ----
    cc = wp.tile([1, 2], F32, tag="cc")
    V.tensor_copy(cc[0:1, 0:1], c1n[0:1, t:t + 1])
    V.tensor_tensor(cc[0:1, 1:2], wrs[:], c2[0:1, t:t + 1], op=OP.mult)
    c12_p = psS.tile([128, 2], F32, tag="s2")
    P.matmul(c12_p[:], ones[0:1, :], cc[:])
    c12b = wp.tile([128, 2], F32, tag="c12b")
    V.tensor_copy(c12b[:], c12_p[:])
    t_wc = wp.tile([128, NT], F32, tag="t_wc")
    V.tensor_scalar(t_wc[:], wexp[:], c12b[:, 1:2], None, op0=OP.mult)
    ww_pm = wp.tile([128, NT], F32, tag="ww_pm")
    swp = wp.tile([128, 1], F32, tag="swp")
    na_ap = nege0_pm[:] if t == 0 else negalloc[:]
    V.scalar_tensor_tensor(ww_pm[:], na_ap, c12b[:, 0:1], t_wc[:],
                           op0=OP.mult, op1=OP.add, accum_out=swp[:])
    wwf_p = psA.tile([1, N], F32, tag="p")
    for c in range(NT):
        P.transpose(wwf_p[0:1, 128 * c:128 * (c + 1)], ww_pm[:, c:c + 1],
                    ident[:])
    ww = wp.tile([1, N], F32, tag="ww")
    V.tensor_copy(ww[:], wwf_p[:])
    swf_p = psS.tile([1, 128], F32, tag="s2")
    P.transpose(swf_p[:], swp[:], ident[:])
    sw = wp.tile([1, 1], F32, tag="sw")
    V.tensor_reduce(sw[:], swf_p[:], axis=mybir.AxisListType.X, op=OP.add)

    # ---- prec update (uses prec BEFORE update; link also uses old prec) ----
    if t == 0:
        prec_n = ww  # (1-sw)*0 + ww
    elif last:
        prec_n = None
    else:
        omsw = wp.tile([1, 1], F32, tag="omsw")
        V.tensor_scalar(omsw[:], sw[:], -1.0, 1.0, op0=OP.mult, op1=OP.add)
        prec_n = sp.tile([1, N], F32, tag="prec")
        V.scalar_tensor_tensor(prec_n[:], prec[:], omsw[:], ww[:],
                               op0=OP.mult, op1=OP.add)

    # ---- usage update ----
    if t == 0:
        u_n = ww  # psi=1, u=0 -> u' = ww
    elif last:
        u_n = None
    else:
        fgb_p = psA.tile([128, R], F32, tag="p")
        P.matmul(fgb_p[:], ones[0:1, :], fgF[0:1, :, t])
        yyT = wp.tile([128, NT, R], F32, tag="yyT")
        V.scalar_tensor_tensor(
            yyT[:], fgb_p[:, None, :].broadcast_to([128, NT, R]), -1.0,
            rwT[:].rearrange("p (c r) -> p c r", r=R), op0=OP.mult, op1=OP.mult)
        om = wp.tile([128, NT, R], F32, tag="om")
        V.tensor_scalar(om[:], yyT[:], 1.0, None, op0=OP.add)
        p1 = wp.tile([128, NT], F32, tag="p1")
        V.tensor_tensor(p1[:], om[:, :, 0], om[:, :, 1], op=OP.mult)
        p2 = wp.tile([128, NT], F32, tag="p2")
        V.tensor_tensor(p2[:], om[:, :, 2], om[:, :, 3], op=OP.mult)
        psi_pm = wp.tile([128, NT], F32, tag="psi_pm")
        V.tensor_tensor(psi_pm[:], p1[:], p2[:], op=OP.mult)
        psiT_p = psA.tile([1, N], F32, tag="p")
        for c in range(NT):
            P.transpose(psiT_p[0:1, 128 * c:128 * (c + 1)], psi_pm[:, c:c + 1],
                        ident[:])
        tn = wp.tile([1, N], F32, tag="tn")
        V.scalar_tensor_tensor(tn[:], ww[:], 1.0, omu_a[:],
                               op0=OP.subtract, op1=OP.mult)
        u_n = sp.tile([1, N], F32, tag="u")
        V.scalar_tensor_tensor(u_n[:], tn[:], 1.0, psiT_p[:],
                               op0=OP.add, op1=OP.mult)

    # ---- memory update ----
    wwb_p = psM.tile([W, N], F32, tag="m")
    P.matmul(wwb_p[:], ones[0:1, 0:W], ww[:])
    keep = wp.tile([W, N], F32, tag="keep", bufs=1)
    V.tensor_scalar(keep[:], wwb_p[:], neg_er[:, t:t + 1], 1.0,
                    op0=OP.mult, op1=OP.add)
    m1 = wp.tile([W, N], F32, tag="m1", bufs=1)
    nc.gpsimd.tensor_tensor(m1[:], memT[:], keep[:], op=OP.mult)
    memT_n = sp.tile([W, N], F32, tag="memT")
    V.scalar_tensor_tensor(memT_n[:], wwb_p[:], wvT[:, t:t + 1], m1[:],
                           op0=OP.mult, op1=OP.add)
    mem_nrm_p = psA.tile([128, NT, W], F32, tag="p")
    for c in range(NT):
        P.transpose(mem_nrm_p[:, c, :], memT_n[:, 128 * c:128 * (c + 1)],
                    ident[0:W, 0:W])
    mem_nrm_n = sp.tile([128, NT, W], F32, tag="mem_nrm")
    V.tensor_copy(mem_nrm_n[:], mem_nrm_p[:])
    sqm = wp.tile([W, N], F32, tag="sqm", bufs=1)
    nc.gpsimd.tensor_tensor(sqm[:], memT_n[:], memT_n[:], op=OP.mult)
    msf_p = psM.tile([1, N], F32, tag="m")
    P.matmul(msf_p[:], ones[0:W, 0:1], sqm[:])
    msf = wp.tile([1, N], F32, tag="msf")
    V.tensor_copy(msf[:], msf_p[:])
    ms_tp = psA.tile([128, NT], F32, tag="p")
    for c in range(NT):
        P.transpose(ms_tp[:, c:c + 1], msf[0:1, 128 * c:128 * (c + 1)],
                    ident[0:1, 0:1])
    mnorm_n = sp.tile([128, NT], F32, tag="mnorm")
    rsqrt_pm(wp, ms_tp[:], [128, NT], "w1", out=mnorm_n[:], iters=3)

    # ---- link update ----
    if t == 0:
        L_n, LT_n = L, LT  # stays zero
    else:
        w_pm = ww_pm
        omw_pm = wp.tile([128, NT], F32, tag="omw_pm")
        V.tensor_scalar(omw_pm[:], w_pm[:], -1.0, 1.0, op0=OP.mult, op1=OP.add)
        wb_p = psA.tile([128, N], F32, tag="p")
        P.matmul(wb_p[:], ones[0:1, :], ww[:])
        pb_p = psA.tile([128, N], F32, tag="p")
        P.matmul(pb_p[:], ones[0:1, :], prec[:])
        pb_sb = wp.tile([128, N], F32, tag="pb_sb")
        V.tensor_copy(pb_sb[:], pb_p[:])
        L_n = sp.tile([128, NT, N], F32, tag="L")
        for c in range(NT):
            pbm = wp.tile([128, N], F32, tag="pbm")
            nc.gpsimd.tensor_tensor(pbm[:], pb_sb[:], offd[:, c, :], op=OP.mult)
            t1 = wp.tile([128, N], F32, tag="t1")
            V.scalar_tensor_tensor(t1[:], wb_p[:], omw_pm[:, c:c + 1], L[:, c, :],
                                   op0=OP.subtract, op1=OP.mult)
            V.scalar_tensor_tensor(L_n[:, c, :], pbm[:], w_pm[:, c:c + 1], t1[:],
                                   op0=OP.mult, op1=OP.subtract)
        LT_n = sp.tile([128, NT, N], F32, tag="LT")
        for j in range(NT):
            lt_p = psA.tile([128, N], F32, tag="p")
            for i in range(NT):
                P.transpose(lt_p[:, 128 * i:128 * (i + 1)],
                            L_n[:, i, 128 * j:128 * (j + 1)], ident[:])
            V.tensor_copy(LT_n[:, j, :], lt_p[:])

    # ---- read content weights (PM via FL matmul + transposes) ----
    rdf_p = psM.tile([R, N], F32, tag="m")
    P.matmul(rdf_p[:], rkT2[:, :, t], memT_n[:])
    rdf = wp.tile([R, N], F32, tag="rdf")
    V.tensor_copy(rdf[:], rdf_p[:])
    rdots_p = psA.tile([128, NT, R], F32, tag="p")
    for c in range(NT):
        P.transpose(rdots_p[:, c, :], rdf[:, 128 * c:128 * (c + 1)],
                    ident[0:R, 0:R])
    rlog = wp.tile([128, NT, R], F32, tag="rlog")
    V.tensor_tensor(rlog[:], rdots_p[:],
                    mnorm_n[:, :, None].broadcast_to([128, NT, R]), op=OP.mult)
    rexp = wp.tile([128, NT, R], F32, tag="rexp")
    pexp2(wp, rlog[:].rearrange("p c r -> p (c r)"), [128, NT * R], "rex",
          out=rexp[:].rearrange("p c r -> p (c r)"))
    rpart = wp.tile([128, R], F32, tag="rpart")
    V.tensor_reduce(rpart[:], rexp[:].rearrange("p c r -> p r c"),
                    axis=mybir.AxisListType.X, op=OP.add)
    rsum_p = psS.tile([R, 128], F32, tag="s2")
    P.transpose(rsum_p[:], rpart[:], ident[:])
    rsum = wp.tile([R, 1], F32, tag="rsum")
    V.tensor_reduce(rsum[:], rsum_p[:], axis=mybir.AxisListType.X, op=OP.add)
    rsr = wp.tile([R, 1], F32, tag="rsr")
    V.reciprocal(rsr[:], rsum[:])
    s1c = wp.tile([R, 1], F32, tag="s1c")
    V.tensor_tensor(s1c[:], rsr[:], modes[:, 1, t:t + 1], op=OP.mult)
    rexf_p = psA.tile([R, N], F32, tag="p")
    for c in range(NT):
        P.transpose(rexf_p[:, 128 * c:128 * (c + 1)], rexp[:, c, :], ident[:])

    # ---- read weights ----
    rw_n = sp.tile([R, N], F32, tag="rw")
    if t == 0:
        V.tensor_scalar(rw_n[:], rexf_p[:], s1c[:], None, op0=OP.mult)
    else:
        bwd_p = psM.tile([R, N], F32, tag="m")
        for c in range(NT):
            P.matmul(bwd_p[:], rwT[:, R * c:R * (c + 1)], L_n[:, c, :],
                     start=(c == 0), stop=(c == NT - 1))
        fwd_p = psM.tile([R, N], F32, tag="m")
        for c in range(NT):
            P.matmul(fwd_p[:], rwT[:, R * c:R * (c + 1)], LT_n[:, c, :],
                     start=(c == 0), stop=(c == NT - 1))
        a1 = wp.tile([R, N], F32, tag="a1")
        V.tensor_scalar(a1[:], rexf_p[:], s1c[:], None, op0=OP.mult)
        b1 = wp.tile([R, N], F32, tag="b1")
        V.scalar_tensor_tensor(b1[:], fwd_p[:], modes[:, 2, t:t + 1], a1[:],
                               op0=OP.mult, op1=OP.add)
        V.scalar_tensor_tensor(rw_n[:], bwd_p[:], modes[:, 0, t:t + 1], b1[:],
                               op0=OP.mult, op1=OP.add)

    rwT_p = psA.tile([128, NT * R], F32, tag="p")
    for c in range(NT):
        P.transpose(rwT_p[:, R * c:R * (c + 1)], rw_n[:, 128 * c:128 * (c + 1)],
                    ident[0:R, 0:R])
    rwT_n = sp.tile([128, NT * R], F32, tag="rwT")
    V.tensor_copy(rwT_n[:], rwT_p[:])

    # ---- read words ----
    rwd_p = psS.tile([R, W], F32, tag="s")
    for c in range(NT):
        P.matmul(rwd_p[:], rwT_n[:, R * c:R * (c + 1)], mem_nrm_n[:, c, :],
                 start=(c == 0), stop=(c == NT - 1))
    V.tensor_copy(out_sb[:, t, :], rwd_p[:])

    return dict(memT=memT_n, mem_nrm=mem_nrm_n, mnorm=mnorm_n, L=L_n, LT=LT_n,
                u=u_n, prec=prec_n, rw=rw_n, rwT=rwT_n)


# ---------------------------------------------------------------------------
_NC_CACHE = {}


def _get_nc():
    if "nc" not in _NC_CACHE:
        _NC_CACHE["nc"] = build_nc()
    return _NC_CACHE["nc"]


def _consts():
    ident = np.eye(128, dtype=np.float32)
    ones = np.ones((128, 128), dtype=np.float32)
    offd = (1.0 - np.eye(N)).astype(np.float16)
    return ident, ones, offd


def make_in_maps(controller_output, W_if, b_if, memory0):
    ident, ones, offd = _consts()
    maps = []
    for b in range(B):
        maps.append({
            "co": np.ascontiguousarray(controller_output[b]),
            "wif": np.ascontiguousarray(W_if),
            "bif": np.ascontiguousarray(b_if.reshape(1, IF)),
            "mem0": np.ascontiguousarray(memory0[b]),
            "ident": ident, "ones": ones, "offdiag": offd,
        })
    return maps


def kernel(controller_output, W_if, b_if, memory0):
    from concourse.bass_utils import run_bass_kernel_spmd
    controller_output = np.asarray(controller_output, dtype=np.float32)
    W_if = np.asarray(W_if, dtype=np.float32)
    b_if = np.asarray(b_if, dtype=np.float32)
    memory0 = np.asarray(memory0, dtype=np.float32)
    nc = _get_nc()
    maps = make_in_maps(controller_output, W_if, b_if, memory0)
    res = run_bass_kernel_spmd(nc, maps, core_ids=list(range(B)))
    return np.stack([res.results[b]["out"] for b in range(B)], axis=0)


if __name__ == "__main__":
    mode = sys.argv[1] if len(sys.argv) > 1 else "sim"
    sys.path.insert(0, "/root/problem")
    import jax
    with jax.default_device(jax.devices("cpu")[0]):
        import reference
        inputs = {k: np.asarray(v) for k, v in reference.setup_inputs().items()}
        expected = np.asarray(reference.reference(**inputs))

    if mode == "sim":
        from concourse.bass_interp import CoreSim
        nc = build_nc()
        maps = make_in_maps(inputs["controller_output"], inputs["W_if"],
                            inputs["b_if"], inputs["memory0"])
        sim = CoreSim(nc)
        for k, v in maps[0].items():
            sim.tensor(k)[:] = v
        sim.simulate()
        got = sim.tensor("out").copy()
        exp = expected[0]
        err = np.abs(got - exp)
        rel = np.linalg.norm(got - exp) / (np.linalg.norm(exp) + 1e-12)
        print("sim modeled time (ns):", sim.time)
        print("max abs err:", err.max(), " rel err:", rel)
    else:
        got = kernel(**inputs)
        rel = np.linalg.norm(got - expected) / (np.linalg.norm(expected) + 1e-12)
        print("max abs err:", np.abs(got - expected).max(), " rel err:", rel)



# revision 9
# speedup vs baseline: 2.0010x; 2.0010x over previous
"""DNC MemoryAccess kernel for Trainium2 (Bass/Tile), data-parallel over batch.

Shapes (hardcoded): B=8, T=16, C=1024, IFACE=471, N=512, WORD=64, R=4, NW=1.
Each of the 8 cores processes one batch element; all recurrent state
(memory [64,512]T + [128,4,64], link/linkT [128,4,512] f16, usage/prec,
read_weights) stays SBUF-resident across the T=16 sequential steps.

Engine balance (vs the DVE-bound ancestor): broadcasts and the link-decay
scalar_tensor_tensor chain run on GpSimd (partition_broadcast / STT at
0.83 ns/el), PSUM->SBUF copies run on ACT (one act-table set covers
Copy/Ln/Square/Exp so no per-step table reloads), usage/prec update in
partition-major [128,4] layout, softmax sums via gpsimd partition_all_reduce,
and the read-weight mode mix is accumulated in one PSUM bank by the PE
(bwd/fwd matmuls in f16 + transposed content term).

Precision notes: ACT-table exp has ~1e-5 max rel err, enough to flip the
DNC allocation sort on near-tied usage values. So: strengths/key-norms are
precomputed in the prologue with a full-precision polynomial exp, per-step
norms use Newton-refined rsqrt, and the three per-step exps use the
magic-bits 2^x polynomial (pexp2). Link/linkT state and the allocation
log-sum matmul run in f16 (validated against the 2e-2 harness gate).
"""
import sys

sys.path.insert(0, "/opt/trn_rl_repo")

import numpy as np

import concourse.bacc as bacc
import concourse.bass as bass
import concourse.bass_isa as bass_isa
import concourse.mybir as mybir
import concourse.tile as tile

F32 = mybir.dt.float32
F16 = mybir.dt.float16
I32 = mybir.dt.int32
AF = mybir.ActivationFunctionType
OP = mybir.AluOpType
RED = bass_isa.ReduceOp

B, T, C, IF = 8, 16, 1024, 471
N, W, R = 512, 64, 4
NT = N // 128  # 4 N-tiles
LOG2E = 1.4426950408889634
MAGIC2 = 12582912.0 + 127.0  # round-to-int magic + exponent bias for 2^k bits
_LN2 = 0.6931471805599453
# 2^f = 1 + sum_{i>=1} EXPC[i-1] f^i  (Taylor of exp(f ln2); deg-6 err ~2e-9)
EXPC = [_LN2, _LN2**2 / 2, _LN2**3 / 6, _LN2**4 / 24, _LN2**5 / 120,
        _LN2**6 / 720]

# iface field offsets
O_RK, O_RS, O_WK, O_WS = 0, 256, 260, 324
O_ER, O_WV, O_FG, O_AG, O_WG, O_MD = 325, 389, 453, 457, 458, 459


def build_nc():
    nc = bacc.Bacc("TRN2", target_bir_lowering=False, debug=False, num_devices=8)

    co_d = nc.declare_dram_parameter("co", [T, C], F32, isOutput=False)
    w_d = nc.declare_dram_parameter("wif", [C, IF], F32, isOutput=False)
    b_d = nc.declare_dram_parameter("bif", [1, IF], F32, isOutput=False)
    m0_d = nc.declare_dram_parameter("mem0", [N, W], F32, isOutput=False)
    ident_d = nc.declare_dram_parameter("ident", [128, 128], F32, isOutput=False)
    identh_d = nc.declare_dram_parameter("identh", [128, 128], F16, isOutput=False)
    ones_d = nc.declare_dram_parameter("ones", [128, 128], F32, isOutput=False)
    offd_d = nc.declare_dram_parameter("offdiag", [N, N], F16, isOutput=False)
    out_d = nc.declare_dram_parameter("out", [T, R, W], F32, isOutput=True)

    with tile.TileContext(nc) as tc:
        with (
            tc.tile_pool(name="const", bufs=1) as cp,
            tc.tile_pool(name="state", bufs=2) as sp,
            tc.tile_pool(name="work", bufs=2) as wp,
            tc.tile_pool(name="psP", bufs=2, space="PSUM") as psA,
            tc.tile_pool(name="psM", bufs=2, space="PSUM") as psM,
            tc.tile_pool(name="psS", bufs=1, space="PSUM") as psS,
        ):
            _build_body(nc, tc, cp, sp, wp, psA, psM, psS,
                        co_d, w_d, b_d, m0_d, ident_d, identh_d, ones_d,
                        offd_d, out_d)
    nc.compile()
    return nc


def _helpers(nc):
    V, S = nc.vector, nc.scalar

    def pexp(pool, x_ap, shape, tg, nb=1):
        """exp(x) to ~1e-7 via 2^(x*log2e): magic rounding + deg-6 poly +
        exponent-bit assembly. ~13 DVE ops; prologue/small-tensor use."""
        t_ = pool.tile(shape, F32, tag=tg + "_t", name=tg + "_t")
        V.tensor_scalar(t_[:], x_ap, LOG2E, None, op0=OP.mult)
        a_ = pool.tile(shape, F32, tag=tg + "_a", name=tg + "_a")
        V.tensor_scalar(a_[:], t_[:], MAGIC2, None, op0=OP.add)
        k_ = pool.tile(shape, F32, tag=tg + "_k", name=tg + "_k")
        V.tensor_scalar(k_[:], a_[:], MAGIC2, None, op0=OP.subtract)
        f_ = pool.tile(shape, F32, tag=tg + "_f", name=tg + "_f")
        V.tensor_tensor(f_[:], t_[:], k_[:], op=OP.subtract)
        p2 = pool.tile(shape, I32, tag=tg + "_p2", name=tg + "_p2")
        V.tensor_scalar(p2[:], a_[:].bitcast(I32), 23, None,
                        op0=OP.arith_shift_left)
        ac = [pool.tile(shape, F32, tag=tg + "_ac0", name=tg + "_ac0"),
              pool.tile(shape, F32, tag=tg + "_ac1", name=tg + "_ac1")]
        V.tensor_scalar(ac[0][:], f_[:], EXPC[5], None, op0=OP.mult)
        cur = 0
        for c_ in (EXPC[4], EXPC[3], EXPC[2], EXPC[1], EXPC[0]):
            V.scalar_tensor_tensor(ac[1 - cur][:], ac[cur][:], c_, f_[:],
                                   op0=OP.add, op1=OP.mult)
            cur = 1 - cur
        y_ = pool.tile(shape, F32, tag=tg + "_y", name=tg + "_y")
        V.scalar_tensor_tensor(y_[:], ac[cur][:], 1.0, p2[:].bitcast(F32),
                               op0=OP.add, op1=OP.mult)
        return y_

    def pexp2(pool, x_ap, shape, tg, accum_out=None, out=None, clamp=None):
        """2^x for prescaled x (log2 units); magic-round + deg-6 poly +
        exponent bits. All DVE; ~1e-7. x_ap should be SBUF."""
        if clamp is not None:
            t_ = pool.tile(shape, F32, tag=tg + "_t", name=tg + "_t", bufs=1)
            V.tensor_scalar(t_[:], x_ap, clamp, None, op0=OP.max)
            x_ap = t_[:]
        a_ = pool.tile(shape, F32, tag=tg + "_a", name=tg + "_a", bufs=1)
        V.tensor_scalar(a_[:], x_ap, MAGIC2, None, op0=OP.add)
        fn = pool.tile(shape, F32, tag=tg + "_fn", name=tg + "_fn", bufs=1)
        V.scalar_tensor_tensor(fn[:], a_[:], MAGIC2, x_ap,
                               op0=OP.subtract, op1=OP.subtract)  # -f
        p2 = pool.tile(shape, I32, tag=tg + "_p2", name=tg + "_p2", bufs=1)
        V.tensor_scalar(p2[:], a_[:].bitcast(I32), 23, None,
                        op0=OP.arith_shift_left)
        ac = [pool.tile(shape, F32, tag=tg + "_a0", name=tg + "_a0", bufs=1),
              pool.tile(shape, F32, tag=tg + "_a1", name=tg + "_a1", bufs=1)]
        V.tensor_scalar(ac[0][:], fn[:], EXPC[5], None, op0=OP.mult)
        cur = 0
        for i, c_ in ((5, EXPC[4]), (4, EXPC[3]), (3, EXPC[2]), (2, EXPC[1]),
                      (1, EXPC[0])):
            b_ = c_ if (i % 2 == 0) else -c_
            V.scalar_tensor_tensor(ac[1 - cur][:], ac[cur][:], b_, fn[:],
                                   op0=OP.add, op1=OP.mult)
            cur = 1 - cur
        if out is None:
            out_t = pool.tile(shape, F32, tag=tg + "_y", name=tg + "_y", bufs=1)
            out = out_t[:]
        V.scalar_tensor_tensor(out, ac[cur][:], 1.0, p2[:].bitcast(F32),
                               op0=OP.add, op1=OP.mult, accum_out=accum_out)
        return out

    def rsqrt_sb(pool, x_sb, shape, tg, out=None, iters=3):
        """rsqrt via quake seed + Newton; all DVE/gpsimd, no ACT.
        x_sb must be an SBUF ap."""
        sh = pool.tile(shape, I32, tag=tg + "_sh", name=tg + "_sh", bufs=1)
        V.tensor_scalar(sh[:], x_sb.bitcast(I32), 1, None,
                        op0=OP.arith_shift_right)
        nb = pool.tile(shape, I32, tag=tg + "_nb", name=tg + "_nb", bufs=1)
        V.tensor_scalar(nb[:], sh[:], -1, None, op0=OP.bitwise_xor)
        y_ = pool.tile(shape, F32, tag=tg + "_y", name=tg + "_y", bufs=1)
        V.tensor_scalar(y_[:].bitcast(I32), nb[:], 0x5f3759e0, None, op0=OP.add)
        for i in range(iters):
            s_ = pool.tile(shape, F32, tag=tg + f"_s{i}", name=tg + f"_s{i}", bufs=1)
            V.tensor_tensor(s_[:], y_[:], y_[:], op=OP.mult)
            t_ = pool.tile(shape, F32, tag=tg + f"_t{i}", name=tg + f"_t{i}", bufs=1)
            V.tensor_tensor(t_[:], x_sb, s_[:], op=OP.mult)
            h_ = pool.tile(shape, F32, tag=tg + f"_h{i}", name=tg + f"_h{i}", bufs=1)
            V.tensor_scalar(h_[:], t_[:], -0.5, 1.5, op0=OP.mult, op1=OP.add)
            if i == iters - 1 and out is not None:
                V.tensor_tensor(out, y_[:], h_[:], op=OP.mult)
                return out
            y2 = pool.tile(shape, F32, tag=tg + f"_y{i}", name=tg + f"_y{i}", bufs=1)
            V.tensor_tensor(y2[:], y_[:], h_[:], op=OP.mult)
            y_ = y2
        return y_[:]

    def softplus_precise(pool, x_ap, shape, tg):
        """ln(1+e^x) with table-ln seed + one Newton step (via pexp)."""
        e_ = pexp(pool, x_ap, shape, tg + "e")
        w_ = pool.tile(shape, F32, tag=tg + "_w", name=tg + "_w")
        V.tensor_scalar(w_[:], e_[:], 1.0, None, op0=OP.add)
        z_ = pool.tile(shape, F32, tag=tg + "_z", name=tg + "_z")
        S.activation(z_[:], w_[:], AF.Ln)
        nz = pool.tile(shape, F32, tag=tg + "_nz", name=tg + "_nz")
        S.mul(nz[:], z_[:], -1.0)
        e2 = pexp(pool, nz[:], shape, tg + "e2")
        m_ = pool.tile(shape, F32, tag=tg + "_m", name=tg + "_m")
        V.tensor_tensor(m_[:], w_[:], e2[:], op=OP.mult)
        o_ = pool.tile(shape, F32, tag=tg + "_o", name=tg + "_o")
        V.scalar_tensor_tensor(o_[:], m_[:], -1.0, z_[:], op0=OP.add, op1=OP.add)
        return o_

    def rsqrt_refined(pool, x_ap, shape, tg, iters=1, nb=1):
        """rsqrt(x): ACT-table seed exp(-0.5 ln x) + Newton (no division)."""
        l_ = pool.tile(shape, F32, tag=tg + "_l", name=tg + "_l", bufs=nb)
        S.activation(l_[:], x_ap, AF.Ln)
        y_ = pool.tile(shape, F32, tag=tg + "_y", name=tg + "_y", bufs=nb)
        S.activation(y_[:], l_[:], AF.Exp, scale=-0.5)
        for i in range(iters):
            s_ = pool.tile(shape, F32, tag=tg + f"_s{i}", name=tg + f"_s{i}", bufs=nb)
            nc.gpsimd.tensor_tensor(s_[:], y_[:], y_[:], op=OP.mult)
            t_ = pool.tile(shape, F32, tag=tg + f"_t{i}", name=tg + f"_t{i}", bufs=nb)
            V.tensor_tensor(t_[:], x_ap, s_[:], op=OP.mult)
            h_ = pool.tile(shape, F32, tag=tg + f"_h{i}", name=tg + f"_h{i}", bufs=nb)
            V.tensor_scalar(h_[:], t_[:], -0.5, 1.5, op0=OP.mult, op1=OP.add)
            y2 = pool.tile(shape, F32, tag=tg + f"_y{i}", name=tg + f"_y{i}", bufs=nb)
            V.tensor_tensor(y2[:], y_[:], h_[:], op=OP.mult)
            y_ = y2
        return y_

    return (pexp, softplus_precise, rsqrt_refined, pexp2, rsqrt_sb)


def _build_body(nc, tc, cp, sp, wp, psA, psM, psS,
                co_d, w_d, b_d, m0_d, ident_d, identh_d, ones_d, offd_d, out_d):
    V, S, P, DMA, G = nc.vector, nc.scalar, nc.tensor, nc.sync, nc.gpsimd
    (pexp, softplus_precise, rsqrt_refined, pexp2, rsqrt_sb) = _helpers(nc)

    # ---------------- constants ----------------
    ident = cp.tile([128, 128], F32)
    DMA.dma_start(ident[:], ident_d[:])
    identh = cp.tile([128, 128], F16)
    DMA.dma_start(identh[:], identh_d[:])
    ones = cp.tile([128, 128], F32)
    DMA.dma_start(ones[:], ones_d[:])
    offd = cp.tile([128, NT, N], F16)
    for c in range(NT):
        DMA.dma_start(offd[:, c, :], offd_d[128 * c:128 * (c + 1), :])
    nege0_pm = cp.tile([128, NT], F32)
    G.memset(nege0_pm[:], 0.0)
    G.memset(nege0_pm[0:1, 0:1], -1.0)

    # persistent per-t parse tables
    rkT = cp.tile([W, R, T], F32)
    wkT = cp.tile([W, T], F32)
    neg_er = cp.tile([W, T], F32)
    wvT = cp.tile([W, T], F32)
    bkw = cp.tile([1, T], F32)       # softplus(ws)/||wk||
    wkT2 = cp.tile([W, T], F32)      # wk * bkw * log2e
    rkT2 = cp.tile([W, R, T], F32)   # rk * bkr * log2e
    fgB = cp.tile([128, R, T], F32)  # sigmoid(fg) bcast to all partitions
    c1nB = cp.tile([128, T], F32)    # -ag*wg bcast
    c2B = cp.tile([128, T], F32)     # wg - ag*wg bcast
    modesB = cp.tile([128, 3, R, T], F32)  # softmax(modes) bcast

    # ---------------- prologue: iface (transient pool) ----------------
    with tc.tile_pool(name="prolog", bufs=1) as pp:
        co_sb = pp.tile([T, C], F32)
        DMA.dma_start(co_sb[:], co_d[:])
        bif_sb = pp.tile([1, IF], F32)
        DMA.dma_start(bif_sb[:], b_d[:])

        coT_p = psA.tile([128, 8, T], F32, tag="p")
        for k in range(8):
            P.transpose(coT_p[:, k, :], co_sb[:, 128 * k:128 * (k + 1)],
                        ident[0:T, 0:T])
        coT = pp.tile([128, 8, T], F32)
        S.copy(coT[:], coT_p[:])

        if_p = psS.tile([T, IF], F32, tag="s")
        for h in range(2):
            w_sb = pp.tile([128, 4, IF], F32, tag="w_sb", name=f"w_sb{h}")
            for k in range(4):
                DMA.dma_start(w_sb[:, k, :],
                              w_d[128 * (4 * h + k):128 * (4 * h + k + 1), :])
            for k in range(4):
                P.matmul(if_p[:], coT[:, 4 * h + k, :], w_sb[:, k, :],
                         start=(h == 0 and k == 0), stop=False)
        P.matmul(if_p[:], ones[0:1, 0:T], bif_sb[:], start=False, stop=True)
        iface = pp.tile([T, IF], F32)
        S.copy(iface[:], if_p[:])

        # field transposes -> per-t column layouts
        def tp_field(lo, hi):
            n = hi - lo
            pt = psA.tile([128, T], F32, tag="p")
            P.transpose(pt[0:n, :], iface[:, lo:hi], ident[0:T, 0:T])
            return pt

        for r in range(R):
            pt = tp_field(O_RK + W * r, O_RK + W * (r + 1))
            S.copy(rkT[:, r, :], pt[0:W, :])
        pt = tp_field(O_WK, O_WK + W)
        S.copy(wkT[:], pt[0:W, :])
        pt = tp_field(O_ER, O_ER + W)
        er_in = pp.tile([W, T], F32)
        V.tensor_scalar(er_in[:], pt[0:W, :], -1.0, None, op0=OP.mult)
        er_e = pexp(pp, er_in[:], [W, T], "sge")     # e^{-x}
        er_w = pp.tile([W, T], F32)
        V.tensor_scalar(er_w[:], er_e[:], 1.0, None, op0=OP.add)
        er_r = pp.tile([W, T], F32)
        V.reciprocal(er_r[:], er_w[:])               # sigmoid(x)
        V.tensor_scalar(neg_er[:], er_r[:], -1.0, None, op0=OP.mult)
        pt = tp_field(O_WV, O_WV + W)
        S.copy(wvT[:], pt[0:W, :])

        # strengths / ||k|| folded: bkw, bkr  (free-layout pipeline)
        rsF_p = psA.tile([1, R, T], F32, tag="p")
        for r in range(R):
            P.transpose(rsF_p[0:1, r, :], iface[:, O_RS + r:O_RS + r + 1],
                        ident[0:T, 0:T])
        rsF = pp.tile([1, R, T], F32)
        S.copy(rsF[:], rsF_p[:])
        wsF_p = psA.tile([1, T], F32, tag="p")
        P.transpose(wsF_p[:], iface[:, O_WS:O_WS + 1], ident[0:T, 0:T])
        wsF = pp.tile([1, T], F32)
        S.copy(wsF[:], wsF_p[:])
        rs_pre = softplus_precise(pp, rsF[:].rearrange("o r t -> o (r t)"),
                                  [1, R * T], "rsp")
        ws_pre = softplus_precise(pp, wsF[:], [1, T], "wsp")

        sqw = pp.tile([W, T], F32)
        S.square(sqw[:], wkT[:])
        wk2_p = psM.tile([1, T], F32, tag="m")
        P.matmul(wk2_p[:], ones[0:W, 0:1], sqw[:])
        wkr = rsqrt_refined(pp, wk2_p[:], [1, T], "wkr", iters=2)
        V.tensor_tensor(bkw[:], ws_pre[:], wkr[:], op=OP.mult)

        sqr = pp.tile([W, R, T], F32)
        S.square(sqr[:], rkT[:])
        rk2_p = psM.tile([1, R * T], F32, tag="m")
        P.matmul(rk2_p[:], ones[0:W, 0:1], sqr[:].rearrange("w r t -> w (r t)"))
        rkr = rsqrt_refined(pp, rk2_p[:], [1, R * T], "rkr", iters=2)
        bkrF = cp.tile([1, R, T], F32, name="bkrF")
        V.tensor_tensor(bkrF[:].rearrange("o r t -> o (r t)"), rs_pre[:],
                        rkr[:], op=OP.mult)
        # scaled keys: wkT2 = wkT * bkw * log2e ; rkT2 = rkT * bkr * log2e
        bkwb_p = psA.tile([W, T], F32, tag="p")
        P.matmul(bkwb_p[:], ones[0:1, 0:W], bkw[:])
        V.scalar_tensor_tensor(wkT2[:], wkT[:], LOG2E, bkwb_p[:],
                               op0=OP.mult, op1=OP.mult)
        bkrb_p = psA.tile([W, R * T], F32, tag="p")
        P.matmul(bkrb_p[:], ones[0:1, 0:W], bkrF[:].rearrange("o r t -> o (r t)"))
        V.scalar_tensor_tensor(rkT2[:].rearrange("w r t -> w (r t)"),
                               rkT[:].rearrange("w r t -> w (r t)"), LOG2E,
                               bkrb_p[:], op0=OP.mult, op1=OP.mult)

        # gates: fg, ag, wg sigmoids via precise V pipeline, packed [1,6,T]
        gats_p = psA.tile([1, 6, T], F32, tag="p")
        for r in range(R):
            P.transpose(gats_p[0:1, r, :], iface[:, O_FG + r:O_FG + r + 1],
                        ident[0:T, 0:T])
        P.transpose(gats_p[0:1, 4, :], iface[:, O_AG:O_AG + 1], ident[0:T, 0:T])
        P.transpose(gats_p[0:1, 5, :], iface[:, O_WG:O_WG + 1], ident[0:T, 0:T])
        g_in = pp.tile([1, 6 * T], F32)
        V.tensor_scalar(g_in[:], gats_p[:].rearrange("o g t -> o (g t)"), -1.0,
                        None, op0=OP.mult)
        g_e = pexp(pp, g_in[:], [1, 6 * T], "sgg")
        g_w = pp.tile([1, 6 * T], F32)
        V.tensor_scalar(g_w[:], g_e[:], 1.0, None, op0=OP.add)
        g_r = pp.tile([1, 6, T], F32)
        V.reciprocal(g_r[:].rearrange("o g t -> o (g t)"), g_w[:])
        G.partition_broadcast(fgB[:].rearrange("p r t -> p (r t)"),
                              g_r[0:1, 0:R, :].rearrange("o r t -> o (r t)"))
        ag_t = g_r[0:1, 4, :]
        wg_t = g_r[0:1, 5, :]
        c1t = pp.tile([1, T], F32)
        V.tensor_tensor(c1t[:], ag_t, wg_t, op=OP.mult)
        c1n = pp.tile([1, T], F32)
        V.tensor_scalar(c1n[:], c1t[:], -1.0, None, op0=OP.mult)
        c2 = pp.tile([1, T], F32)
        V.tensor_tensor(c2[:], wg_t, c1t[:], op=OP.subtract)
        G.partition_broadcast(c1nB[:], c1n[:])
        G.partition_broadcast(c2B[:], c2[:])

        # modes softmax (precise exp; normalize in [T,12]; flatten; bcast)
        me = pexp(pp, iface[:, O_MD:O_MD + 12], [T, 12], "me")
        me3 = me[:].rearrange("t (r m) -> t r m", m=3)
        msum = pp.tile([T, R], F32)
        V.tensor_tensor(msum[:], me3[:, :, 0], me3[:, :, 1], op=OP.add)
        V.tensor_tensor(msum[:], msum[:], me3[:, :, 2], op=OP.add)
        mrs = pp.tile([T, R], F32)
        V.reciprocal(mrs[:], msum[:])
        mn = pp.tile([T, 12], F32)
        mn3 = mn[:].rearrange("t (r m) -> t r m", m=3)
        for m in range(3):
            V.tensor_tensor(mn3[:, :, m], me3[:, :, m], mrs[:], op=OP.mult)
        mo = pp.tile([T, 12], F32)
        mo3 = mo[:].rearrange("t (m r) -> t m r", r=R)
        S.copy(mo3[:], mn3[:].rearrange("t r m -> t m r"))
        # flatten [T,12] -> [1, (m r t)] via 12 single-column transposes
        mF_p = psA.tile([1, 12, T], F32, tag="p")
        for j in range(12):
            P.transpose(mF_p[0:1, j, :], mo[:, j:j + 1], ident[0:T, 0:T])
        mF = pp.tile([1, 12, T], F32)
        S.copy(mF[:], mF_p[:])
        G.partition_broadcast(modesB[:].rearrange("p m r t -> p (m r t)"),
                              mF[:].rearrange("o j t -> o (j t)"))

    # ---------------- initial state ----------------
    mem_nrm = sp.tile([128, NT, W], F32, tag="mem_nrm")
    for c in range(NT):
        DMA.dma_start(mem_nrm[:, c, :], m0_d[128 * c:128 * (c + 1), :])
    memT_p = psA.tile([W, N], F32, tag="p")
    for c in range(NT):
        P.transpose(memT_p[:, 128 * c:128 * (c + 1)], mem_nrm[:, c, :], ident[:])
    memT = sp.tile([W, N], F32, tag="memT")
    S.copy(memT[:], memT_p[:])

    sq0 = wp.tile([128, NT, W], F32, tag="sq0", bufs=1)
    S.square(sq0[:], mem_nrm[:])
    msum0 = wp.tile([128, NT], F32, tag="msum")
    V.tensor_reduce(msum0[:], sq0[:], axis=mybir.AxisListType.X, op=OP.add)
    mnorm = sp.tile([128, NT], F32, tag="mnorm")
    rsqrt_sb(wp, msum0[:], [128, NT], "w1", out=mnorm[:], iters=3)

    L = sp.tile([128, NT, N], F16, tag="L")
    G.memset(L[:], 0.0)
    LT0 = sp.tile([128, NT, N], F16, tag="LT")
    G.memset(LT0[:], 0.0)
    u_pm0 = sp.tile([128, NT], F32, tag="u_pm")
    G.memset(u_pm0[:], 0.0)
    rwT0 = sp.tile([128, NT * R], F32, tag="rwT")
    G.memset(rwT0[:], 0.0)

    out_sb = cp.tile([R, T, W], F32)

    st = dict(memT=memT, mem_nrm=mem_nrm, mnorm=mnorm, L=L, LT=LT0,
              u_pm=u_pm0, prec_pm=None, prec_f=None, rwT=rwT0,
              ub=None, pb=None, pbm=None)

    for t in range(T):
        st = _step(nc, t, st, cp, sp, wp, psA, psM, psS,
                   ident, identh, ones, offd, nege0_pm, wkT2, rkT2, neg_er,
                   wvT, fgB, c1nB, c2B, modesB, out_sb, pexp2, rsqrt_sb)

    DMA.dma_start(out_d[:].rearrange("t r w -> r t w"), out_sb[:])


def _step(nc, t, st, cp, sp, wp, psA, psM, psS,
          ident, identh, ones, offd, nege0_pm, wkT2, rkT2, neg_er, wvT,
          fgB, c1nB, c2B, modesB, out_sb, pexp2, rsqrt_sb):
    V, S, P, G = nc.vector, nc.scalar, nc.tensor, nc.gpsimd
    memT, mem_nrm, mnorm = st["memT"], st["mem_nrm"], st["mnorm"]
    L, LT, u_pm, rwT = st["L"], st["LT"], st["u_pm"], st["rwT"]
    prec_pm, prec_f = st["prec_pm"], st["prec_f"]
    ub, pb, pbm = st["ub"], st["pb"], st["pbm"]
    last = (t == T - 1)

    # ---- write content dots (PE; memT from prev step) ----
    wdf_p = psM.tile([1, N], F32, tag="m")
    P.matmul(wdf_p[:], wkT2[:, t:t + 1], memT[:])
    wdf = wp.tile([1, N], F32, tag="wdf")
    S.copy(wdf[:], wdf_p[:])
    wdots_p = psA.tile([128, NT], F32, tag="p")
    for c in range(NT):
        P.transpose(wdots_p[:, c:c + 1], wdf[0:1, 128 * c:128 * (c + 1)],
                    ident[0:1, 0:1])

    # ---- DVE head: psi chain + rw mode scales (deps: prev-step rwT) ----
    if 0 < t:
        yyT = wp.tile([128, NT, R], F32, tag="yyT")
        V.scalar_tensor_tensor(
            yyT[:], fgB[:, None, :, t].broadcast_to([128, NT, R]), -1.0,
            rwT[:].rearrange("p (c r) -> p c r", r=R), op0=OP.mult, op1=OP.mult)
        om = wp.tile([128, NT, R], F32, tag="om")
        V.tensor_scalar(om[:], yyT[:], 1.0, None, op0=OP.add)
        p1 = wp.tile([128, NT], F32, tag="p1")
        V.tensor_tensor(p1[:], om[:, :, 0], om[:, :, 1], op=OP.mult)
        p2 = wp.tile([128, NT], F32, tag="p2")
        V.tensor_tensor(p2[:], om[:, :, 2], om[:, :, 3], op=OP.mult)
        psi_pm = wp.tile([128, NT], F32, tag="psi_pm")
        V.tensor_tensor(psi_pm[:], p1[:], p2[:], op=OP.mult)
        rwm0 = wp.tile([128, NT * R], F16, tag="rwm0")
        V.tensor_tensor(rwm0[:].rearrange("p (c r) -> p c r", r=R),
                        rwT[:].rearrange("p (c r) -> p c r", r=R),
                        modesB[:, 0, None, :, t].broadcast_to([128, NT, R]),
                        op=OP.mult)
        rwm2 = wp.tile([128, NT * R], F16, tag="rwm2")
        V.tensor_tensor(rwm2[:].rearrange("p (c r) -> p c r", r=R),
                        rwT[:].rearrange("p (c r) -> p c r", r=R),
                        modesB[:, 2, None, :, t].broadcast_to([128, NT, R]),
                        op=OP.mult)

    # ---- allocation pipeline (PM layout; ub broadcast from prev step) ----
    wes_n = 2 * NT if t > 0 else NT
    wes = wp.tile([128, 2 * NT], F32, tag="wes")
    wlog = wes[:, 0:NT]
    V.tensor_tensor(wlog, wdots_p[:], mnorm[:], op=OP.mult)
    if t > 0:
        lu = wp.tile([128, NT], F32, tag="lu")
        S.activation(lu[:], u_pm[:], AF.Ln)
        lu2 = wp.tile([128, NT], F16, tag="lu2")
        V.tensor_scalar(lu2[:], lu[:], LOG2E, None, op0=OP.mult)
        Gm = wp.tile([128, NT, N], F16, tag="Gm", bufs=1)
        for c in range(NT):
            V.tensor_scalar(Gm[:, c, :], ub[:], u_pm[:, c:c + 1], None,
                            op0=OP.is_gt)
        s_p = psM.tile([1, N], F32, tag="m")
        for c in range(NT):
            P.matmul(s_p[:], lu2[:, c:c + 1], Gm[:, c, :],
                     start=(c == 0), stop=(c == NT - 1))
        s_f = wp.tile([1, N], F32, tag="s_f")
        S.copy(s_f[:], s_p[:])
        s_tp = psA.tile([128, NT], F32, tag="p")
        for c in range(NT):
            P.transpose(s_tp[:, c:c + 1], s_f[0:1, 128 * c:128 * (c + 1)],
                        ident[0:1, 0:1])
        V.tensor_copy(wes[:, NT:2 * NT], s_tp[:])

    # ---- fused exp: wexp (cols 0:NT) + es (cols NT:2NT) in one chain ----
    wese = wp.tile([128, 2 * NT], F32, tag="wese")
    pexp2(wp, wes[:, 0:wes_n], [128, wes_n], "wes",
          out=wese[:, 0:wes_n], clamp=-115.0)
    wexp = wese[:, 0:NT]
    wpart = wp.tile([128, 1], F32, tag="wpart")
    V.tensor_reduce(wpart[:], wexp, axis=mybir.AxisListType.X, op=OP.add)
    wsumB = wp.tile([128, 1], F32, tag="wsumB")
    G.partition_all_reduce(wsumB[:], wpart[:], channels=128, reduce_op=RED.add)
    wrs = wp.tile([128, 1], F32, tag="wrs")
    V.reciprocal(wrs[:], wsumB[:])
    if t > 0:
        negalloc = wp.tile([128, NT], F32, tag="negalloc")
        V.scalar_tensor_tensor(negalloc[:], u_pm[:], 1.0, wese[:, NT:2 * NT],
                               op0=OP.subtract, op1=OP.mult)

    # ---- write weights ww (PM primary) ----
    t_wc = wp.tile([128, NT], F32, tag="t_wc")
    V.tensor_scalar(t_wc[:], wexp, wrs[:], c2B[:, t:t + 1],
                    op0=OP.mult, op1=OP.mult)
    ww_pm = wp.tile([128, NT], F32, tag="ww_pm")
    swp = wp.tile([128, 1], F32, tag="swp")
    na_ap = nege0_pm[:] if t == 0 else negalloc[:]
    V.scalar_tensor_tensor(ww_pm[:], na_ap, c1nB[:, t:t + 1], t_wc[:],
                           op0=OP.mult, op1=OP.add, accum_out=swp[:])
    wwf_p = psA.tile([1, N], F32, tag="p")
    for c in range(NT):
        P.transpose(wwf_p[0:1, 128 * c:128 * (c + 1)], ww_pm[:, c:c + 1],
                    ident[:])
    ww = wp.tile([1, N], F32, tag="ww")
    S.copy(ww[:], wwf_p[:])
    wb = wp.tile([128, N], F32, tag="wb")
    G.partition_broadcast(wb[:], ww[:])
    swB = wp.tile([128, 1], F32, tag="swB")
    G.partition_all_reduce(swB[:], swp[:], channels=128, reduce_op=RED.add)

    # ---- prec update in PM (uses prec BEFORE update) ----
    if t == 0:
        prec_pm_n = ww_pm
        prec_f_n = ww
    elif last:
        prec_pm_n = None
        prec_f_n = None
    else:
        omsw = wp.tile([128, 1], F32, tag="omsw")
        V.tensor_scalar(omsw[:], swB[:], -1.0, 1.0, op0=OP.mult, op1=OP.add)
        prec_pm_n = sp.tile([128, NT], F32, tag="prec_pm")
        V.scalar_tensor_tensor(prec_pm_n[:], prec_pm[:], omsw[:], ww_pm[:],
                               op0=OP.mult, op1=OP.add)
        pf_p = psA.tile([1, N], F32, tag="p")
        for c in range(NT):
            P.transpose(pf_p[0:1, 128 * c:128 * (c + 1)], prec_pm_n[:, c:c + 1],
                        ident[:])
        prec_f_n = sp.tile([1, N], F32, tag="prec_f")
        S.copy(prec_f_n[:], pf_p[:])

    # ---- usage update (PM layout) ----
    if t == 0:
        u_pm_n = ww_pm
        u_f_n = ww
    elif last:
        u_pm_n = None
        u_f_n = None
    else:
        omu_pm = wp.tile([128, NT], F32, tag="omu_pm")
        V.tensor_scalar(omu_pm[:], u_pm[:], -1.0, 1.0, op0=OP.mult, op1=OP.add)
        tpm = wp.tile([128, NT], F32, tag="tpm")
        V.scalar_tensor_tensor(tpm[:], ww_pm[:], 1.0, omu_pm[:],
                               op0=OP.subtract, op1=OP.mult)
        u_pm_n = sp.tile([128, NT], F32, tag="u_pm")
        V.scalar_tensor_tensor(u_pm_n[:], tpm[:], 1.0, psi_pm[:],
                               op0=OP.add, op1=OP.mult)
        uf_p = psA.tile([1, N], F32, tag="p")
        for c in range(NT):
            P.transpose(uf_p[0:1, 128 * c:128 * (c + 1)], u_pm_n[:, c:c + 1],
                        ident[:])
        u_f_n = sp.tile([1, N], F32, tag="u_f")
        S.copy(u_f_n[:], uf_p[:])

    # ---- memory update ----
    keep = wp.tile([W, N], F32, tag="keep", bufs=1)
    V.tensor_scalar(keep[:], wb[0:W, :], neg_er[:, t:t + 1], 1.0,
                    op0=OP.mult, op1=OP.add)
    m1 = wp.tile([W, N], F32, tag="m1", bufs=1)
    G.tensor_tensor(m1[:], memT[:], keep[:], op=OP.mult)
    memT_n = sp.tile([W, N], F32, tag="memT")
    V.scalar_tensor_tensor(memT_n[:], wb[0:W, :], wvT[:, t:t + 1], m1[:],
                           op0=OP.mult, op1=OP.add)
    mem_nrm_p = psA.tile([128, NT, W], F32, tag="p")
    for c in range(NT):
        P.transpose(mem_nrm_p[:, c, :], memT_n[:, 128 * c:128 * (c + 1)],
                    ident[0:W, 0:W])
    mem_nrm_n = sp.tile([128, NT, W], F32, tag="mem_nrm")
    S.copy(mem_nrm_n[:], mem_nrm_p[:])
    sqn = wp.tile([128, NT, W], F32, tag="sqn", bufs=1)
    S.square(sqn[:], mem_nrm_p[:])
    msum = wp.tile([128, NT], F32, tag="msum")
    V.tensor_reduce(msum[:], sqn[:], axis=mybir.AxisListType.X, op=OP.add)
    mnorm_n = sp.tile([128, NT], F32, tag="mnorm")
    rsqrt_sb(wp, msum[:], [128, NT], "w1", out=mnorm_n[:], iters=3)

    # ---- read content dots (PE; hoisted before link transposes) ----
    rdf_p = psM.tile([R, N], F32, tag="m")
    P.matmul(rdf_p[:], rkT2[:, :, t], memT_n[:])
    rdf = wp.tile([R, N], F32, tag="rdf")
    S.copy(rdf[:], rdf_p[:])
    rdots_p = psA.tile([128, NT, R], F32, tag="p")
    for c in range(NT):
        P.transpose(rdots_p[:, c, :], rdf[:, 128 * c:128 * (c + 1)],
                    ident[0:R, 0:R])

    # ---- link update: chunks 0-1 DVE 2-STT, chunks 2-3 DVE-TS + Pool-TT ----
    if t == 0:
        L_n, LT_n = L, LT  # stays zero
    else:
        omw_pm = wp.tile([128, NT], F32, tag="omw_pm")
        V.tensor_scalar(omw_pm[:], ww_pm[:], -1.0, 1.0, op0=OP.mult, op1=OP.add)
        L_n = sp.tile([128, NT, N], F16, tag="L")
        for c in (2, 3):
            Dm = wp.tile([128, N], F16, tag=f"Dm_{c % 2}", name=f"Dm_{c}")
            V.tensor_scalar(Dm[:], wb[:], omw_pm[:, c:c + 1], -1.0,
                            op0=OP.subtract, op1=OP.mult)
            q_ = wp.tile([128, N], F16, tag=f"q_{c % 2}", name=f"q_{c}")
            V.tensor_scalar(q_[:], pbm[:, c, :], ww_pm[:, c:c + 1], None,
                            op0=OP.mult)
            t1 = wp.tile([128, N], F16, tag=f"t1h_{c % 2}", name=f"t1h_{c}")
            G.tensor_tensor(t1[:], Dm[:], L[:, c, :], op=OP.mult)
            G.tensor_tensor(L_n[:, c, :], q_[:], t1[:], op=OP.add)
        for c in (0, 1):
            t1 = wp.tile([128, N], F32, tag=f"t1_{c % 2}", name=f"t1_{c}")
            V.scalar_tensor_tensor(t1[:], wb[:], omw_pm[:, c:c + 1], L[:, c, :],
                                   op0=OP.subtract, op1=OP.mult)
            V.scalar_tensor_tensor(L_n[:, c, :], pbm[:, c, :], ww_pm[:, c:c + 1],
                                   t1[:], op0=OP.mult, op1=OP.subtract)
        LT_n = sp.tile([128, NT, N], F16, tag="LT")
        for j in range(NT):
            lt_p = psA.tile([128, N], F16, tag="plt")
            for i in range(NT):
                P.transpose(lt_p[:, 128 * i:128 * (i + 1)],
                            L_n[:, i, 128 * j:128 * (j + 1)], identh[:])
            if j < 2:
                S.copy(LT_n[:, j, :], lt_p[:])
            else:
                V.tensor_copy(LT_n[:, j, :], lt_p[:])

    # ---- read softmax ----
    rlog = wp.tile([128, NT, R], F32, tag="rlog")
    V.tensor_tensor(rlog[:], rdots_p[:],
                    mnorm_n[:, :, None].broadcast_to([128, NT, R]), op=OP.mult)
    rexp = wp.tile([128, NT, R], F32, tag="rexp")
    pexp2(wp, rlog[:].rearrange("p c r -> p (c r)"), [128, NT * R], "rex",
          out=rexp[:].rearrange("p c r -> p (c r)"))
    rpart = wp.tile([128, R], F32, tag="rpart")
    V.tensor_reduce(rpart[:], rexp[:].rearrange("p c r -> p r c"),
                    axis=mybir.AxisListType.X, op=OP.add)
    rsumB = wp.tile([128, R], F32, tag="rsumB")
    G.partition_all_reduce(rsumB[:], rpart[:], channels=128, reduce_op=RED.add)
    rsr = wp.tile([128, R], F32, tag="rsr")
    V.reciprocal(rsr[:], rsumB[:])
    m1rs = wp.tile([128, R], F32, tag="m1rs")
    V.tensor_tensor(m1rs[:], rsr[:], modesB[:, 1, :, t], op=OP.mult)
    rexp_s = wp.tile([128, NT, R], F32, tag="rexp_s")
    V.tensor_tensor(rexp_s[:], rexp[:],
                    m1rs[:, None, :].broadcast_to([128, NT, R]), op=OP.mult)

    # ---- read weights: accumulate bwd + content + fwd in one PSUM bank ----
    rw_p = psS.tile([R, N], F32, tag="rw")
    if t > 0:
        for c in range(NT):
            P.matmul(rw_p[:], rwm0[:, R * c:R * (c + 1)], L_n[:, c, :],
                     start=(c == 0), stop=False)
        for c in range(NT):
            P.matmul(rw_p[:], rwm2[:, R * c:R * (c + 1)], LT_n[:, c, :],
                     start=False, stop=False)
        for c in range(NT):
            P.matmul(rw_p[:, 128 * c:128 * (c + 1)], rexp_s[:, c, :],
                     ident[:], is_transpose=True, start=False,
                     stop=(c == NT - 1))
    else:
        for c in range(NT):
            P.matmul(rw_p[:, 128 * c:128 * (c + 1)], rexp_s[:, c, :],
                     ident[:], is_transpose=True, start=(c == 0),
                     stop=(c == NT - 1))
    rw = wp.tile([R, N], F32, tag="rwf")
    S.copy(rw[:], rw_p[:])
    rwT_p = psA.tile([128, NT * R], F32, tag="p")
    for c in range(NT):
        P.transpose(rwT_p[:, R * c:R * (c + 1)], rw[:, 128 * c:128 * (c + 1)],
                    ident[0:R, 0:R])
    rwT_n = sp.tile([128, NT * R], F32, tag="rwT")
    V.tensor_copy(rwT_n[:], rwT_p[:])

    # ---- read words ----
    rwd_p = psS.tile([R, W], F32, tag="s")
    for c in range(NT):
        P.matmul(rwd_p[:], rwT_n[:, R * c:R * (c + 1)], mem_nrm_n[:, c, :],
                 start=(c == 0), stop=(c == NT - 1))
    S.copy(out_sb[:, t, :], rwd_p[:])

    # ---- tail: broadcasts for the NEXT step (hoisted into this step) ----
    ub_n = pb_n = pbm_n = None
    if not last:
        ub_n = wp.tile([128, N], F32, tag="ub")
        G.partition_broadcast(ub_n[:], u_f_n[:])
        pb_n = wp.tile([128, N], F32, tag="pb")
        G.partition_broadcast(pb_n[:], prec_f_n[:])
        pbm_n = wp.tile([128, NT, N], F32, tag="pbm")
        for c in range(NT):
            G.tensor_tensor(pbm_n[:, c, :], pb_n[:], offd[:, c, :], op=OP.mult)

    return dict(memT=memT_n, mem_nrm=mem_nrm_n, mnorm=mnorm_n, L=L_n, LT=LT_n,
                u_pm=u_pm_n, prec_pm=prec_pm_n, prec_f=prec_f_n, rwT=rwT_n,
                ub=ub_n, pb=pb_n, pbm=pbm_n)


# ---------------------------------------------------------------------------
_NC_CACHE = {}


def _get_nc():
    if "nc" not in _NC_CACHE:
        _NC_CACHE["nc"] = build_nc()
    return _NC_CACHE["nc"]


def _consts():
    ident = np.eye(128, dtype=np.float32)
    identh = np.eye(128, dtype=np.float16)
    ones = np.ones((128, 128), dtype=np.float32)
    offd = (1.0 - np.eye(N)).astype(np.float16)
    return ident, identh, ones, offd


def make_in_maps(controller_output, W_if, b_if, memory0):
    ident, identh, ones, offd = _consts()
    maps = []
    for b in range(B):
        maps.append({
            "co": np.ascontiguousarray(controller_output[b]),
            "wif": np.ascontiguousarray(W_if),
            "bif": np.ascontiguousarray(b_if.reshape(1, IF)),
            "mem0": np.ascontiguousarray(memory0[b]),
            "ident": ident, "identh": identh, "ones": ones, "offdiag": offd,
        })
    return maps


def kernel(controller_output, W_if, b_if, memory0):
    from concourse.bass_utils import run_bass_kernel_spmd
    controller_output = np.asarray(controller_output, dtype=np.float32)
    W_if = np.asarray(W_if, dtype=np.float32)
    b_if = np.asarray(b_if, dtype=np.float32)
    memory0 = np.asarray(memory0, dtype=np.float32)
    nc = _get_nc()
    maps = make_in_maps(controller_output, W_if, b_if, memory0)
    res = run_bass_kernel_spmd(nc, maps, core_ids=list(range(B)))
    return np.stack([res.results[b]["out"] for b in range(B)], axis=0)


if __name__ == "__main__":
    mode = sys.argv[1] if len(sys.argv) > 1 else "sim"
    sys.path.insert(0, "/root/problem")
    import jax
    with jax.default_device(jax.devices("cpu")[0]):
        import reference
        inputs = {k: np.asarray(v) for k, v in reference.setup_inputs().items()}
        expected = np.asarray(reference.reference(**inputs))

    if mode == "sim":
        from concourse.bass_interp import CoreSim
        nc = build_nc()
        maps = make_in_maps(inputs["controller_output"], inputs["W_if"],
                            inputs["b_if"], inputs["memory0"])
        sim = CoreSim(nc)
        for k, v in maps[0].items():
            sim.tensor(k)[:] = v
        sim.simulate()
        got = sim.tensor("out").copy()
        exp = expected[0]
        err = np.abs(got - exp)
        rel = np.linalg.norm(got - exp) / (np.linalg.norm(exp) + 1e-12)
        print("sim modeled time (ns):", sim.time)
        print("max abs err:", err.max(), " rel err:", rel)
    else:
        got = kernel(**inputs)
        rel = np.linalg.norm(got - expected) / (np.linalg.norm(expected) + 1e-12)
        print("max abs err:", np.abs(got - expected).max(), " rel err:", rel)


# revision 11
# speedup vs baseline: 2.2212x; 1.1100x over previous
"""DNC MemoryAccess kernel for Trainium2 (Bass/Tile), data-parallel over batch.

Shapes (hardcoded): B=8, T=16, C=1024, IFACE=471, N=512, WORD=64, R=4, NW=1.
Each of the 8 cores processes one batch element; all recurrent state
(memory [64,512]T + [128,4,64], link/linkT [128,4,512] f16, usage/prec,
read_weights) stays SBUF-resident across the T=16 sequential steps.

Engine balance (vs the DVE-bound ancestor): broadcasts and the link-decay
scalar_tensor_tensor chain run on GpSimd (partition_broadcast / STT at
0.83 ns/el), PSUM->SBUF copies run on ACT (one act-table set covers
Copy/Ln/Square/Exp so no per-step table reloads), usage/prec update in
partition-major [128,4] layout, softmax sums via gpsimd partition_all_reduce,
and the read-weight mode mix is accumulated in one PSUM bank by the PE
(bwd/fwd matmuls in f16 + transposed content term).

Precision notes: ACT-table exp has ~1e-5 max rel err, enough to flip the
DNC allocation sort on near-tied usage values. So: strengths/key-norms are
precomputed in the prologue with a full-precision polynomial exp, per-step
norms use Newton-refined rsqrt, and the three per-step exps use the
magic-bits 2^x polynomial (pexp2). Link/linkT state and the allocation
log-sum matmul run in f16 (validated against the 2e-2 harness gate).
"""
import sys

sys.path.insert(0, "/opt/trn_rl_repo")

import numpy as np

import concourse.bacc as bacc
import concourse.bass as bass
import concourse.bass_isa as bass_isa
import concourse.mybir as mybir
import concourse.tile as tile

F32 = mybir.dt.float32
F16 = mybir.dt.float16
I32 = mybir.dt.int32
AF = mybir.ActivationFunctionType
OP = mybir.AluOpType
RED = bass_isa.ReduceOp

B, T, C, IF = 8, 16, 1024, 471
N, W, R = 512, 64, 4
NT = N // 128  # 4 N-tiles
LOG2E = 1.4426950408889634
MAGIC2 = 12582912.0 + 127.0  # round-to-int magic + exponent bias for 2^k bits
_LN2 = 0.6931471805599453
# 2^f = 1 + sum_{i>=1} EXPC[i-1] f^i  (Taylor of exp(f ln2); deg-6 err ~2e-9)
EXPC = [_LN2, _LN2**2 / 2, _LN2**3 / 6, _LN2**4 / 24, _LN2**5 / 120,
        _LN2**6 / 720]

# iface field offsets
O_RK, O_RS, O_WK, O_WS = 0, 256, 260, 324
O_ER, O_WV, O_FG, O_AG, O_WG, O_MD = 325, 389, 453, 457, 458, 459


def build_nc():
    nc = bacc.Bacc("TRN2", target_bir_lowering=False, debug=False, num_devices=8)

    co_d = nc.declare_dram_parameter("co", [T, C], F32, isOutput=False)
    w_d = nc.declare_dram_parameter("wif", [C, IF], F32, isOutput=False)
    b_d = nc.declare_dram_parameter("bif", [1, IF], F32, isOutput=False)
    m0_d = nc.declare_dram_parameter("mem0", [N, W], F32, isOutput=False)
    ident_d = nc.declare_dram_parameter("ident", [128, 128], F32, isOutput=False)
    identh_d = nc.declare_dram_parameter("identh", [128, 128], F16, isOutput=False)
    ones_d = nc.declare_dram_parameter("ones", [128, 128], F32, isOutput=False)
    offd_d = nc.declare_dram_parameter("offdiag", [N, N], F16, isOutput=False)
    out_d = nc.declare_dram_parameter("out", [T, R, W], F32, isOutput=True)

    with tile.TileContext(nc) as tc:
        with (
            tc.tile_pool(name="const", bufs=1) as cp,
            tc.tile_pool(name="state", bufs=2) as sp,
            tc.tile_pool(name="work", bufs=2) as wp,
            tc.tile_pool(name="psP", bufs=2, space="PSUM") as psA,
            tc.tile_pool(name="psM", bufs=2, space="PSUM") as psM,
            tc.tile_pool(name="psS", bufs=1, space="PSUM") as psS,
        ):
            _build_body(nc, tc, cp, sp, wp, psA, psM, psS,
                        co_d, w_d, b_d, m0_d, ident_d, identh_d, ones_d,
                        offd_d, out_d)
    nc.compile()
    return nc


def _helpers(nc):
    V, S = nc.vector, nc.scalar

    def pexp(pool, x_ap, shape, tg, nb=1):
        """exp(x) to ~1e-7 via 2^(x*log2e): magic rounding + deg-6 poly +
        exponent-bit assembly. ~13 DVE ops; prologue/small-tensor use."""
        t_ = pool.tile(shape, F32, tag=tg + "_t", name=tg + "_t")
        V.tensor_scalar(t_[:], x_ap, LOG2E, None, op0=OP.mult)
        a_ = pool.tile(shape, F32, tag=tg + "_a", name=tg + "_a")
        V.tensor_scalar(a_[:], t_[:], MAGIC2, None, op0=OP.add)
        k_ = pool.tile(shape, F32, tag=tg + "_k", name=tg + "_k")
        V.tensor_scalar(k_[:], a_[:], MAGIC2, None, op0=OP.subtract)
        f_ = pool.tile(shape, F32, tag=tg + "_f", name=tg + "_f")
        V.tensor_tensor(f_[:], t_[:], k_[:], op=OP.subtract)
        p2 = pool.tile(shape, I32, tag=tg + "_p2", name=tg + "_p2")
        V.tensor_scalar(p2[:], a_[:].bitcast(I32), 23, None,
                        op0=OP.arith_shift_left)
        ac = [pool.tile(shape, F32, tag=tg + "_ac0", name=tg + "_ac0"),
              pool.tile(shape, F32, tag=tg + "_ac1", name=tg + "_ac1")]
        V.tensor_scalar(ac[0][:], f_[:], EXPC[5], None, op0=OP.mult)
        cur = 0
        for c_ in (EXPC[4], EXPC[3], EXPC[2], EXPC[1], EXPC[0]):
            V.scalar_tensor_tensor(ac[1 - cur][:], ac[cur][:], c_, f_[:],
                                   op0=OP.add, op1=OP.mult)
            cur = 1 - cur
        y_ = pool.tile(shape, F32, tag=tg + "_y", name=tg + "_y")
        V.scalar_tensor_tensor(y_[:], ac[cur][:], 1.0, p2[:].bitcast(F32),
                               op0=OP.add, op1=OP.mult)
        return y_

    def pexp2(pool, x_ap, shape, tg, accum_out=None, out=None, clamp=None):
        """2^x for prescaled x (log2 units); magic-round + deg-6 poly +
        exponent bits. All DVE; ~1e-7. x_ap should be SBUF."""
        if clamp is not None:
            t_ = pool.tile(shape, F32, tag=tg + "_t", name=tg + "_t", bufs=1)
            V.tensor_scalar(t_[:], x_ap, clamp, None, op0=OP.max)
            x_ap = t_[:]
        a_ = pool.tile(shape, F32, tag=tg + "_a", name=tg + "_a", bufs=1)
        V.tensor_scalar(a_[:], x_ap, MAGIC2, None, op0=OP.add)
        fn = pool.tile(shape, F32, tag=tg + "_fn", name=tg + "_fn", bufs=1)
        V.scalar_tensor_tensor(fn[:], a_[:], MAGIC2, x_ap,
                               op0=OP.subtract, op1=OP.subtract)  # -f
        p2 = pool.tile(shape, I32, tag=tg + "_p2", name=tg + "_p2", bufs=1)
        V.tensor_scalar(p2[:], a_[:].bitcast(I32), 23, None,
                        op0=OP.arith_shift_left)
        ac = [pool.tile(shape, F32, tag=tg + "_a0", name=tg + "_a0", bufs=1),
              pool.tile(shape, F32, tag=tg + "_a1", name=tg + "_a1", bufs=1)]
        V.tensor_scalar(ac[0][:], fn[:], EXPC[5], None, op0=OP.mult)
        cur = 0
        for i, c_ in ((5, EXPC[4]), (4, EXPC[3]), (3, EXPC[2]), (2, EXPC[1]),
                      (1, EXPC[0])):
            b_ = c_ if (i % 2 == 0) else -c_
            V.scalar_tensor_tensor(ac[1 - cur][:], ac[cur][:], b_, fn[:],
                                   op0=OP.add, op1=OP.mult)
            cur = 1 - cur
        if out is None:
            out_t = pool.tile(shape, F32, tag=tg + "_y", name=tg + "_y", bufs=1)
            out = out_t[:]
        V.scalar_tensor_tensor(out, ac[cur][:], 1.0, p2[:].bitcast(F32),
                               op0=OP.add, op1=OP.mult, accum_out=accum_out)
        return out

    _LNC = [0.9999751958009936, -0.49938365136996526, 0.3277847093008827,
            -0.22061263120600254, 0.1361341477101341, -0.06570959215415353,
            0.019091997353919793, -0.002558717382663216]

    def lnu_dve(pool, x_ap, shape, tg, out=None):
        """ln(x) for x in (0, 1]-ish via exponent bits + deg-7 poly on
        mantissa-1; ~13 DVE ops, max abs err ~5e-7. Subnormal x gives
        ln ~ -88 (harmless under the -87 clamp downstream)."""
        eb = pool.tile(shape, I32, tag=tg + "_eb", name=tg + "_eb", bufs=1)
        V.tensor_scalar(eb[:], x_ap.bitcast(I32), 23, None,
                        op0=OP.logical_shift_right)
        ef = pool.tile(shape, F32, tag=tg + "_ef", name=tg + "_ef", bufs=1)
        V.tensor_scalar(ef[:], eb[:], -127, None, op0=OP.add)
        mb = pool.tile(shape, I32, tag=tg + "_mb", name=tg + "_mb", bufs=1)
        V.tensor_scalar(mb[:], x_ap.bitcast(I32), 0x7FFFFF, None,
                        op0=OP.bitwise_and)
        m_ = pool.tile(shape, F32, tag=tg + "_m", name=tg + "_m", bufs=1)
        V.tensor_scalar(m_[:].bitcast(I32), mb[:], 0x3F800000, None,
                        op0=OP.bitwise_or)
        s_ = pool.tile(shape, F32, tag=tg + "_s", name=tg + "_s", bufs=1)
        V.tensor_scalar(s_[:], m_[:], -1.0, None, op0=OP.add)
        ac = [pool.tile(shape, F32, tag=tg + "_a0", name=tg + "_a0", bufs=1),
              pool.tile(shape, F32, tag=tg + "_a1", name=tg + "_a1", bufs=1)]
        V.tensor_scalar(ac[0][:], s_[:], _LNC[7], None, op0=OP.mult)
        cur = 0
        for c_ in (_LNC[6], _LNC[5], _LNC[4], _LNC[3], _LNC[2], _LNC[1],
                   _LNC[0]):
            V.scalar_tensor_tensor(ac[1 - cur][:], ac[cur][:], c_, s_[:],
                                   op0=OP.add, op1=OP.mult)
            cur = 1 - cur
        ln1p = pool.tile(shape, F32, tag=tg + "_l", name=tg + "_l", bufs=1)
        V.tensor_tensor(ln1p[:], ac[cur][:], s_[:], op=OP.mult)
        if out is None:
            o_ = pool.tile(shape, F32, tag=tg + "_o", name=tg + "_o", bufs=1)
            out = o_[:]
        V.scalar_tensor_tensor(out, ef[:], _LN2, ln1p[:],
                               op0=OP.mult, op1=OP.add)
        return out

    def rsqrt_sb(pool, x_sb, shape, tg, out=None, iters=3):
        """rsqrt via quake seed + Newton; all DVE/gpsimd, no ACT.
        x_sb must be an SBUF ap."""
        sh = pool.tile(shape, I32, tag=tg + "_sh", name=tg + "_sh", bufs=1)
        V.tensor_scalar(sh[:], x_sb.bitcast(I32), 1, None,
                        op0=OP.arith_shift_right)
        nb = pool.tile(shape, I32, tag=tg + "_nb", name=tg + "_nb", bufs=1)
        V.tensor_scalar(nb[:], sh[:], -1, None, op0=OP.bitwise_xor)
        y_ = pool.tile(shape, F32, tag=tg + "_y", name=tg + "_y", bufs=1)
        V.tensor_scalar(y_[:].bitcast(I32), nb[:], 0x5f3759e0, None, op0=OP.add)
        for i in range(iters):
            s_ = pool.tile(shape, F32, tag=tg + f"_s{i}", name=tg + f"_s{i}", bufs=1)
            V.tensor_tensor(s_[:], y_[:], y_[:], op=OP.mult)
            t_ = pool.tile(shape, F32, tag=tg + f"_t{i}", name=tg + f"_t{i}", bufs=1)
            V.tensor_tensor(t_[:], x_sb, s_[:], op=OP.mult)
            h_ = pool.tile(shape, F32, tag=tg + f"_h{i}", name=tg + f"_h{i}", bufs=1)
            V.tensor_scalar(h_[:], t_[:], -0.5, 1.5, op0=OP.mult, op1=OP.add)
            if i == iters - 1 and out is not None:
                V.tensor_tensor(out, y_[:], h_[:], op=OP.mult)
                return out
            y2 = pool.tile(shape, F32, tag=tg + f"_y{i}", name=tg + f"_y{i}", bufs=1)
            V.tensor_tensor(y2[:], y_[:], h_[:], op=OP.mult)
            y_ = y2
        return y_[:]

    def softplus_precise(pool, x_ap, shape, tg):
        """ln(1+e^x) with table-ln seed + one Newton step (via pexp)."""
        e_ = pexp(pool, x_ap, shape, tg + "e")
        w_ = pool.tile(shape, F32, tag=tg + "_w", name=tg + "_w")
        V.tensor_scalar(w_[:], e_[:], 1.0, None, op0=OP.add)
        z_ = pool.tile(shape, F32, tag=tg + "_z", name=tg + "_z")
        S.activation(z_[:], w_[:], AF.Ln)
        nz = pool.tile(shape, F32, tag=tg + "_nz", name=tg + "_nz")
        S.mul(nz[:], z_[:], -1.0)
        e2 = pexp(pool, nz[:], shape, tg + "e2")
        m_ = pool.tile(shape, F32, tag=tg + "_m", name=tg + "_m")
        V.tensor_tensor(m_[:], w_[:], e2[:], op=OP.mult)
        o_ = pool.tile(shape, F32, tag=tg + "_o", name=tg + "_o")
        V.scalar_tensor_tensor(o_[:], m_[:], -1.0, z_[:], op0=OP.add, op1=OP.add)
        return o_

    def rsqrt_refined(pool, x_ap, shape, tg, iters=1, nb=1):
        """rsqrt(x): ACT-table seed exp(-0.5 ln x) + Newton (no division)."""
        l_ = pool.tile(shape, F32, tag=tg + "_l", name=tg + "_l", bufs=nb)
        S.activation(l_[:], x_ap, AF.Ln)
        y_ = pool.tile(shape, F32, tag=tg + "_y", name=tg + "_y", bufs=nb)
        S.activation(y_[:], l_[:], AF.Exp, scale=-0.5)
        for i in range(iters):
            s_ = pool.tile(shape, F32, tag=tg + f"_s{i}", name=tg + f"_s{i}", bufs=nb)
            nc.gpsimd.tensor_tensor(s_[:], y_[:], y_[:], op=OP.mult)
            t_ = pool.tile(shape, F32, tag=tg + f"_t{i}", name=tg + f"_t{i}", bufs=nb)
            V.tensor_tensor(t_[:], x_ap, s_[:], op=OP.mult)
            h_ = pool.tile(shape, F32, tag=tg + f"_h{i}", name=tg + f"_h{i}", bufs=nb)
            V.tensor_scalar(h_[:], t_[:], -0.5, 1.5, op0=OP.mult, op1=OP.add)
            y2 = pool.tile(shape, F32, tag=tg + f"_y{i}", name=tg + f"_y{i}", bufs=nb)
            V.tensor_tensor(y2[:], y_[:], h_[:], op=OP.mult)
            y_ = y2
        return y_

    return (pexp, softplus_precise, rsqrt_refined, pexp2, rsqrt_sb,
            lnu_dve)


def _build_body(nc, tc, cp, sp, wp, psA, psM, psS,
                co_d, w_d, b_d, m0_d, ident_d, identh_d, ones_d, offd_d, out_d):
    V, S, P, DMA, G = nc.vector, nc.scalar, nc.tensor, nc.sync, nc.gpsimd
    (pexp, softplus_precise, rsqrt_refined, pexp2, rsqrt_sb,
     lnu_dve) = _helpers(nc)

    # ---------------- constants ----------------
    ident = cp.tile([128, 128], F32)
    DMA.dma_start(ident[:], ident_d[:])
    identh = cp.tile([128, 128], F16)
    DMA.dma_start(identh[:], identh_d[:])
    ones = cp.tile([128, 128], F32)
    DMA.dma_start(ones[:], ones_d[:])
    offd = cp.tile([128, NT, N], F16)
    for c in range(NT):
        DMA.dma_start(offd[:, c, :], offd_d[128 * c:128 * (c + 1), :])
    nege0_pm = cp.tile([128, NT], F32)
    G.memset(nege0_pm[:], 0.0)
    G.memset(nege0_pm[0:1, 0:1], -1.0)

    # persistent per-t parse tables
    rkT = cp.tile([W, R, T], F32)
    wkT = cp.tile([W, T], F32)
    neg_er = cp.tile([W, T], F32)
    wvT = cp.tile([W, T], F32)
    bkw = cp.tile([1, T], F32)       # softplus(ws)/||wk||
    wkT2 = cp.tile([W, T], F32)      # wk * bkw * log2e
    rkT2 = cp.tile([W, R, T], F32)   # rk * bkr * log2e
    fgB = cp.tile([128, R, T], F32)  # sigmoid(fg) bcast to all partitions
    c1nB = cp.tile([128, T], F32)    # -ag*wg bcast
    c2B = cp.tile([128, T], F32)     # wg - ag*wg bcast
    modesB = cp.tile([128, 3, R, T], F32)  # softmax(modes) bcast

    # ---------------- prologue: iface (transient pool) ----------------
    with tc.tile_pool(name="prolog", bufs=1) as pp:
        co_sb = pp.tile([T, C], F32)
        DMA.dma_start(co_sb[:], co_d[:])
        bif_sb = pp.tile([1, IF], F32)
        DMA.dma_start(bif_sb[:], b_d[:])

        coT_p = psA.tile([128, 8, T], F32, tag="p")
        for k in range(8):
            P.transpose(coT_p[:, k, :], co_sb[:, 128 * k:128 * (k + 1)],
                        ident[0:T, 0:T])
        coT = pp.tile([128, 8, T], F32)
        S.copy(coT[:], coT_p[:])

        if_p = psS.tile([T, IF], F32, tag="s")
        for h in range(2):
            w_sb = pp.tile([128, 4, IF], F32, tag="w_sb", name=f"w_sb{h}")
            for k in range(4):
                DMA.dma_start(w_sb[:, k, :],
                              w_d[128 * (4 * h + k):128 * (4 * h + k + 1), :])
            for k in range(4):
                P.matmul(if_p[:], coT[:, 4 * h + k, :], w_sb[:, k, :],
                         start=(h == 0 and k == 0), stop=False)
        P.matmul(if_p[:], ones[0:1, 0:T], bif_sb[:], start=False, stop=True)
        iface = pp.tile([T, IF], F32)
        S.copy(iface[:], if_p[:])

        # field transposes -> per-t column layouts
        def tp_field(lo, hi):
            n = hi - lo
            pt = psA.tile([128, T], F32, tag="p")
            P.transpose(pt[0:n, :], iface[:, lo:hi], ident[0:T, 0:T])
            return pt

        for r in range(R):
            pt = tp_field(O_RK + W * r, O_RK + W * (r + 1))
            S.copy(rkT[:, r, :], pt[0:W, :])
        pt = tp_field(O_WK, O_WK + W)
        S.copy(wkT[:], pt[0:W, :])
        pt = tp_field(O_ER, O_ER + W)
        er_in = pp.tile([W, T], F32)
        V.tensor_scalar(er_in[:], pt[0:W, :], -1.0, None, op0=OP.mult)
        er_e = pexp(pp, er_in[:], [W, T], "sge")     # e^{-x}
        er_w = pp.tile([W, T], F32)
        V.tensor_scalar(er_w[:], er_e[:], 1.0, None, op0=OP.add)
        er_r = pp.tile([W, T], F32)
        V.reciprocal(er_r[:], er_w[:])               # sigmoid(x)
        V.tensor_scalar(neg_er[:], er_r[:], -1.0, None, op0=OP.mult)
        pt = tp_field(O_WV, O_WV + W)
        S.copy(wvT[:], pt[0:W, :])

        # strengths / ||k|| folded: bkw, bkr  (free-layout pipeline)
        rsF_p = psA.tile([1, R, T], F32, tag="p")
        for r in range(R):
            P.transpose(rsF_p[0:1, r, :], iface[:, O_RS + r:O_RS + r + 1],
                        ident[0:T, 0:T])
        rsF = pp.tile([1, R, T], F32)
        S.copy(rsF[:], rsF_p[:])
        wsF_p = psA.tile([1, T], F32, tag="p")
        P.transpose(wsF_p[:], iface[:, O_WS:O_WS + 1], ident[0:T, 0:T])
        wsF = pp.tile([1, T], F32)
        S.copy(wsF[:], wsF_p[:])
        rs_pre = softplus_precise(pp, rsF[:].rearrange("o r t -> o (r t)"),
                                  [1, R * T], "rsp")
        ws_pre = softplus_precise(pp, wsF[:], [1, T], "wsp")

        sqw = pp.tile([W, T], F32)
        S.square(sqw[:], wkT[:])
        wk2_p = psM.tile([1, T], F32, tag="m")
        P.matmul(wk2_p[:], ones[0:W, 0:1], sqw[:])
        wkr = rsqrt_refined(pp, wk2_p[:], [1, T], "wkr", iters=2)
        V.tensor_tensor(bkw[:], ws_pre[:], wkr[:], op=OP.mult)

        sqr = pp.tile([W, R, T], F32)
        S.square(sqr[:], rkT[:])
        rk2_p = psM.tile([1, R * T], F32, tag="m")
        P.matmul(rk2_p[:], ones[0:W, 0:1], sqr[:].rearrange("w r t -> w (r t)"))
        rkr = rsqrt_refined(pp, rk2_p[:], [1, R * T], "rkr", iters=2)
        bkrF = cp.tile([1, R, T], F32, name="bkrF")
        V.tensor_tensor(bkrF[:].rearrange("o r t -> o (r t)"), rs_pre[:],
                        rkr[:], op=OP.mult)
        # scaled keys (natural-log units): wkT2 = wkT*bkw ; rkT2 = rkT*bkr
        bkwb_p = psA.tile([W, T], F32, tag="p")
        P.matmul(bkwb_p[:], ones[0:1, 0:W], bkw[:])
        V.tensor_tensor(wkT2[:], wkT[:], bkwb_p[:], op=OP.mult)
        bkrb_p = psA.tile([W, R * T], F32, tag="p")
        P.matmul(bkrb_p[:], ones[0:1, 0:W], bkrF[:].rearrange("o r t -> o (r t)"))
        V.tensor_tensor(rkT2[:].rearrange("w r t -> w (r t)"),
                        rkT[:].rearrange("w r t -> w (r t)"),
                        bkrb_p[:], op=OP.mult)

        # gates: fg, ag, wg sigmoids via precise V pipeline, packed [1,6,T]
        gats_p = psA.tile([1, 6, T], F32, tag="p")
        for r in range(R):
            P.transpose(gats_p[0:1, r, :], iface[:, O_FG + r:O_FG + r + 1],
                        ident[0:T, 0:T])
        P.transpose(gats_p[0:1, 4, :], iface[:, O_AG:O_AG + 1], ident[0:T, 0:T])
        P.transpose(gats_p[0:1, 5, :], iface[:, O_WG:O_WG + 1], ident[0:T, 0:T])
        g_in = pp.tile([1, 6 * T], F32)
        V.tensor_scalar(g_in[:], gats_p[:].rearrange("o g t -> o (g t)"), -1.0,
                        None, op0=OP.mult)
        g_e = pexp(pp, g_in[:], [1, 6 * T], "sgg")
        g_w = pp.tile([1, 6 * T], F32)
        V.tensor_scalar(g_w[:], g_e[:], 1.0, None, op0=OP.add)
        g_r = pp.tile([1, 6, T], F32)
        V.reciprocal(g_r[:].rearrange("o g t -> o (g t)"), g_w[:])
        G.partition_broadcast(fgB[:].rearrange("p r t -> p (r t)"),
                              g_r[0:1, 0:R, :].rearrange("o r t -> o (r t)"))
        ag_t = g_r[0:1, 4, :]
        wg_t = g_r[0:1, 5, :]
        c1t = pp.tile([1, T], F32)
        V.tensor_tensor(c1t[:], ag_t, wg_t, op=OP.mult)
        c1n = pp.tile([1, T], F32)
        V.tensor_scalar(c1n[:], c1t[:], -1.0, None, op0=OP.mult)
        c2 = pp.tile([1, T], F32)
        V.tensor_tensor(c2[:], wg_t, c1t[:], op=OP.subtract)
        G.partition_broadcast(c1nB[:], c1n[:])
        G.partition_broadcast(c2B[:], c2[:])

        # modes softmax (precise exp; normalize in [T,12]; flatten; bcast)
        me = pexp(pp, iface[:, O_MD:O_MD + 12], [T, 12], "me")
        me3 = me[:].rearrange("t (r m) -> t r m", m=3)
        msum = pp.tile([T, R], F32)
        V.tensor_tensor(msum[:], me3[:, :, 0], me3[:, :, 1], op=OP.add)
        V.tensor_tensor(msum[:], msum[:], me3[:, :, 2], op=OP.add)
        mrs = pp.tile([T, R], F32)
        V.reciprocal(mrs[:], msum[:])
        mn = pp.tile([T, 12], F32)
        mn3 = mn[:].rearrange("t (r m) -> t r m", m=3)
        for m in range(3):
            V.tensor_tensor(mn3[:, :, m], me3[:, :, m], mrs[:], op=OP.mult)
        mo = pp.tile([T, 12], F32)
        mo3 = mo[:].rearrange("t (m r) -> t m r", r=R)
        S.copy(mo3[:], mn3[:].rearrange("t r m -> t m r"))
        # flatten [T,12] -> [1, (m r t)] via 12 single-column transposes
        mF_p = psA.tile([1, 12, T], F32, tag="p")
        for j in range(12):
            P.transpose(mF_p[0:1, j, :], mo[:, j:j + 1], ident[0:T, 0:T])
        mF = pp.tile([1, 12, T], F32)
        S.copy(mF[:], mF_p[:])
        G.partition_broadcast(modesB[:].rearrange("p m r t -> p (m r t)"),
                              mF[:].rearrange("o j t -> o (j t)"))

    # ---------------- initial state ----------------
    mem_nrm = sp.tile([128, NT, W], F32, tag="mem_nrm")
    for c in range(NT):
        DMA.dma_start(mem_nrm[:, c, :], m0_d[128 * c:128 * (c + 1), :])
    memT_p = psA.tile([W, N], F32, tag="p")
    for c in range(NT):
        P.transpose(memT_p[:, 128 * c:128 * (c + 1)], mem_nrm[:, c, :], ident[:])
    memT = sp.tile([W, N], F32, tag="memT")
    S.copy(memT[:], memT_p[:])

    sq0 = wp.tile([128, NT, W], F32, tag="sq0", bufs=1)
    S.square(sq0[:], mem_nrm[:])
    msum0 = wp.tile([128, NT], F32, tag="msum")
    V.tensor_reduce(msum0[:], sq0[:], axis=mybir.AxisListType.X, op=OP.add)
    mnorm = sp.tile([128, NT], F32, tag="mnorm")
    rsqrt_sb(wp, msum0[:], [128, NT], "w1", out=mnorm[:], iters=3)

    L = sp.tile([128, NT, N], F16, tag="L")
    G.memset(L[:], 0.0)
    LT0 = sp.tile([128, NT, N], F16, tag="LT")
    G.memset(LT0[:], 0.0)
    u_pm0 = sp.tile([128, NT], F32, tag="u_pm")
    G.memset(u_pm0[:], 0.0)
    rwT0 = sp.tile([128, NT * R], F32, tag="rwT")
    G.memset(rwT0[:], 0.0)

    out_sb = cp.tile([R, T, W], F32)

    st = dict(memT=memT, mem_nrm=mem_nrm, mnorm=mnorm, L=L, LT=LT0,
              u_pm=u_pm0, prec_pm=None, prec_f=None, rwT=rwT0,
              ub=None, pb=None, pbm=None)

    for t in range(T):
        st = _step(nc, t, st, cp, sp, wp, psA, psM, psS,
                   ident, identh, ones, offd, nege0_pm, wkT2, rkT2, neg_er,
                   wvT, fgB, c1nB, c2B, modesB, out_sb, pexp2, rsqrt_sb,
                   lnu_dve)

    DMA.dma_start(out_d[:].rearrange("t r w -> r t w"), out_sb[:])


def _step(nc, t, st, cp, sp, wp, psA, psM, psS,
          ident, identh, ones, offd, nege0_pm, wkT2, rkT2, neg_er, wvT,
          fgB, c1nB, c2B, modesB, out_sb, pexp2, rsqrt_sb, lnu_dve):
    V, S, P, G = nc.vector, nc.scalar, nc.tensor, nc.gpsimd
    memT, mem_nrm, mnorm = st["memT"], st["mem_nrm"], st["mnorm"]
    L, LT, u_pm, rwT = st["L"], st["LT"], st["u_pm"], st["rwT"]
    prec_pm, prec_f = st["prec_pm"], st["prec_f"]
    ub, pb, pbm = st["ub"], st["pb"], st["pbm"]
    last = (t == T - 1)

    # ---- write content dots (PE; memT from prev step) ----
    wdf_p = psM.tile([1, N], F32, tag="m")
    P.matmul(wdf_p[:], wkT2[:, t:t + 1], memT[:])
    wdf = wp.tile([1, N], F32, tag="wdf")
    S.copy(wdf[:], wdf_p[:])
    wdots_p = psA.tile([128, NT], F32, tag="p")
    for c in range(NT):
        P.transpose(wdots_p[:, c:c + 1], wdf[0:1, 128 * c:128 * (c + 1)],
                    ident[0:1, 0:1])

    # ---- DVE head: psi chain + rw mode scales (deps: prev-step rwT) ----
    if 0 < t:
        yyT = wp.tile([128, NT, R], F32, tag="yyT")
        V.scalar_tensor_tensor(
            yyT[:], fgB[:, None, :, t].broadcast_to([128, NT, R]), -1.0,
            rwT[:].rearrange("p (c r) -> p c r", r=R), op0=OP.mult, op1=OP.mult)
        om = wp.tile([128, NT, R], F32, tag="om")
        V.tensor_scalar(om[:], yyT[:], 1.0, None, op0=OP.add)
        p1 = wp.tile([128, NT], F32, tag="p1")
        V.tensor_tensor(p1[:], om[:, :, 0], om[:, :, 1], op=OP.mult)
        p2 = wp.tile([128, NT], F32, tag="p2")
        V.tensor_tensor(p2[:], om[:, :, 2], om[:, :, 3], op=OP.mult)
        psi_pm = wp.tile([128, NT], F32, tag="psi_pm")
        V.tensor_tensor(psi_pm[:], p1[:], p2[:], op=OP.mult)
        rwm0 = wp.tile([128, NT * R], F16, tag="rwm0")
        V.tensor_tensor(rwm0[:].rearrange("p (c r) -> p c r", r=R),
                        rwT[:].rearrange("p (c r) -> p c r", r=R),
                        modesB[:, 0, None, :, t].broadcast_to([128, NT, R]),
                        op=OP.mult)
        rwm2 = wp.tile([128, NT * R], F16, tag="rwm2")
        V.tensor_tensor(rwm2[:].rearrange("p (c r) -> p c r", r=R),
                        rwT[:].rearrange("p (c r) -> p c r", r=R),
                        modesB[:, 2, None, :, t].broadcast_to([128, NT, R]),
                        op=OP.mult)

    # ---- allocation pipeline (PM layout; ub broadcast from prev step) ----
    wlog = wp.tile([128, NT], F32, tag="wlog")
    V.tensor_tensor(wlog[:], wdots_p[:], mnorm[:], op=OP.mult)
    wexp = wp.tile([128, NT], F32, tag="wexp")
    S.activation(wexp[:], wlog[:], AF.Exp)
    if t > 0:
        lu2 = wp.tile([128, NT], F16, tag="lu2")
        lnu_dve(wp, u_pm[:], [128, NT], "lnu", out=lu2[:])
        Gm = wp.tile([128, NT, N], F16, tag="Gm", bufs=1)
        for c in range(NT):
            V.tensor_scalar(Gm[:, c, :], ub[:], u_pm[:, c:c + 1], None,
                            op0=OP.is_gt)
        s_p = psM.tile([1, N], F32, tag="m")
        for c in range(NT):
            P.matmul(s_p[:], lu2[:, c:c + 1], Gm[:, c, :],
                     start=(c == 0), stop=(c == NT - 1))
        s_f = wp.tile([1, N], F32, tag="s_f")
        S.copy(s_f[:], s_p[:])
        s_tp = psA.tile([128, NT], F32, tag="p")
        for c in range(NT):
            P.transpose(s_tp[:, c:c + 1], s_f[0:1, 128 * c:128 * (c + 1)],
                        ident[0:1, 0:1])
        s_cl = wp.tile([128, NT], F32, tag="s_cl")
        V.tensor_scalar(s_cl[:], s_tp[:], -87.0, None, op0=OP.max)
        es = wp.tile([128, NT], F32, tag="es")
        S.activation(es[:], s_cl[:], AF.Exp)

    wpart = wp.tile([128, 1], F32, tag="wpart")
    V.tensor_reduce(wpart[:], wexp[:], axis=mybir.AxisListType.X, op=OP.add)
    wsumB = wp.tile([128, 1], F32, tag="wsumB")
    G.partition_all_reduce(wsumB[:], wpart[:], channels=128, reduce_op=RED.add)
    wrs = wp.tile([128, 1], F32, tag="wrs")
    V.reciprocal(wrs[:], wsumB[:])
    if t > 0:
        negalloc = wp.tile([128, NT], F32, tag="negalloc")
        V.scalar_tensor_tensor(negalloc[:], u_pm[:], 1.0, es[:],
                               op0=OP.subtract, op1=OP.mult)

    # ---- write weights ww (PM primary) ----
    t_wc = wp.tile([128, NT], F32, tag="t_wc")
    V.tensor_scalar(t_wc[:], wexp[:], wrs[:], c2B[:, t:t + 1],
                    op0=OP.mult, op1=OP.mult)
    ww_pm = wp.tile([128, NT], F32, tag="ww_pm")
    swp = wp.tile([128, 1], F32, tag="swp")
    na_ap = nege0_pm[:] if t == 0 else negalloc[:]
    V.scalar_tensor_tensor(ww_pm[:], na_ap, c1nB[:, t:t + 1], t_wc[:],
                           op0=OP.mult, op1=OP.add, accum_out=swp[:])
    wwf_p = psA.tile([1, N], F32, tag="p")
    for c in range(NT):
        P.transpose(wwf_p[0:1, 128 * c:128 * (c + 1)], ww_pm[:, c:c + 1],
                    ident[:])
    ww = wp.tile([1, N], F32, tag="ww")
    S.copy(ww[:], wwf_p[:])
    wb = wp.tile([128, N], F32, tag="wb")
    G.partition_broadcast(wb[:], ww[:])
    swB = wp.tile([128, 1], F32, tag="swB")
    G.partition_all_reduce(swB[:], swp[:], channels=128, reduce_op=RED.add)

    # ---- prec update in PM (uses prec BEFORE update) ----
    if t == 0:
        prec_pm_n = ww_pm
        prec_f_n = ww
    elif last:
        prec_pm_n = None
        prec_f_n = None
    else:
        omsw = wp.tile([128, 1], F32, tag="omsw")
        V.tensor_scalar(omsw[:], swB[:], -1.0, 1.0, op0=OP.mult, op1=OP.add)
        prec_pm_n = sp.tile([128, NT], F32, tag="prec_pm")
        V.scalar_tensor_tensor(prec_pm_n[:], prec_pm[:], omsw[:], ww_pm[:],
                               op0=OP.mult, op1=OP.add)
        pf_p = psA.tile([1, N], F32, tag="p")
        for c in range(NT):
            P.transpose(pf_p[0:1, 128 * c:128 * (c + 1)], prec_pm_n[:, c:c + 1],
                        ident[:])
        prec_f_n = sp.tile([1, N], F32, tag="prec_f")
        S.copy(prec_f_n[:], pf_p[:])

    # ---- usage update (PM layout) ----
    if t == 0:
        u_pm_n = ww_pm
        u_f_n = ww
    elif last:
        u_pm_n = None
        u_f_n = None
    else:
        omu_pm = wp.tile([128, NT], F32, tag="omu_pm")
        V.tensor_scalar(omu_pm[:], u_pm[:], -1.0, 1.0, op0=OP.mult, op1=OP.add)
        tpm = wp.tile([128, NT], F32, tag="tpm")
        V.scalar_tensor_tensor(tpm[:], ww_pm[:], 1.0, omu_pm[:],
                               op0=OP.subtract, op1=OP.mult)
        u_pm_n = sp.tile([128, NT], F32, tag="u_pm")
        V.scalar_tensor_tensor(u_pm_n[:], tpm[:], 1.0, psi_pm[:],
                               op0=OP.add, op1=OP.mult)
        uf_p = psA.tile([1, N], F32, tag="p")
        for c in range(NT):
            P.transpose(uf_p[0:1, 128 * c:128 * (c + 1)], u_pm_n[:, c:c + 1],
                        ident[:])
        u_f_n = sp.tile([1, N], F32, tag="u_f")
        S.copy(u_f_n[:], uf_p[:])

    # ---- memory update ----
    keep = wp.tile([W, N], F32, tag="keep", bufs=1)
    V.tensor_scalar(keep[:], wb[0:W, :], neg_er[:, t:t + 1], 1.0,
                    op0=OP.mult, op1=OP.add)
    m1 = wp.tile([W, N], F32, tag="m1", bufs=1)
    G.tensor_tensor(m1[:], memT[:], keep[:], op=OP.mult)
    memT_n = sp.tile([W, N], F32, tag="memT")
    V.scalar_tensor_tensor(memT_n[:], wb[0:W, :], wvT[:, t:t + 1], m1[:],
                           op0=OP.mult, op1=OP.add)
    mem_nrm_p = psA.tile([128, NT, W], F32, tag="p")
    for c in range(NT):
        P.transpose(mem_nrm_p[:, c, :], memT_n[:, 128 * c:128 * (c + 1)],
                    ident[0:W, 0:W])
    mem_nrm_n = sp.tile([128, NT, W], F32, tag="mem_nrm")
    S.copy(mem_nrm_n[:], mem_nrm_p[:])
    sqn = wp.tile([128, NT, W], F32, tag="sqn", bufs=1)
    S.square(sqn[:], mem_nrm_p[:])
    msum = wp.tile([128, NT], F32, tag="msum")
    V.tensor_reduce(msum[:], sqn[:], axis=mybir.AxisListType.X, op=OP.add)
    mnorm_n = sp.tile([128, NT], F32, tag="mnorm")
    rsqrt_sb(wp, msum[:], [128, NT], "w1", out=mnorm_n[:], iters=2)

    # ---- read content dots (PE; hoisted before link transposes) ----
    rdf_p = psM.tile([R, N], F32, tag="m")
    P.matmul(rdf_p[:], rkT2[:, :, t], memT_n[:])
    rdf = wp.tile([R, N], F32, tag="rdf")
    S.copy(rdf[:], rdf_p[:])
    rdots_p = psA.tile([128, NT, R], F32, tag="p")
    for c in range(NT):
        P.transpose(rdots_p[:, c, :], rdf[:, 128 * c:128 * (c + 1)],
                    ident[0:R, 0:R])

    # ---- link update: chunks 0-1 DVE 2-STT, chunks 2-3 DVE-TS + Pool-TT ----
    if t == 0:
        L_n, LT_n = L, LT  # stays zero
    else:
        omw_pm = wp.tile([128, NT], F32, tag="omw_pm")
        V.tensor_scalar(omw_pm[:], ww_pm[:], -1.0, 1.0, op0=OP.mult, op1=OP.add)
        L_n = sp.tile([128, NT, N], F16, tag="L")
        for c in range(NT):
            Dm = wp.tile([128, N], F16, tag=f"Dm_{c % 2}", name=f"Dm_{c}")
            V.tensor_scalar(Dm[:], wb[:], omw_pm[:, c:c + 1], -1.0,
                            op0=OP.subtract, op1=OP.mult)
            q_ = wp.tile([128, N], F16, tag=f"q_{c % 2}", name=f"q_{c}")
            V.tensor_scalar(q_[:], pbm[:, c, :], ww_pm[:, c:c + 1], None,
                            op0=OP.mult)
            t1 = wp.tile([128, N], F16, tag=f"t1h_{c % 2}", name=f"t1h_{c}")
            G.tensor_tensor(t1[:], Dm[:], L[:, c, :], op=OP.mult)
            G.tensor_tensor(L_n[:, c, :], q_[:], t1[:], op=OP.add)
        LT_n = sp.tile([128, NT, N], F16, tag="LT")
        for j in range(NT):
            lt_p = psA.tile([128, N], F16, tag="plt")
            for i in range(NT):
                P.transpose(lt_p[:, 128 * i:128 * (i + 1)],
                            L_n[:, i, 128 * j:128 * (j + 1)], identh[:])
            if j < 2:
                S.copy(LT_n[:, j, :], lt_p[:])
            else:
                V.tensor_copy(LT_n[:, j, :], lt_p[:])

    # ---- read softmax ----
    rlog = wp.tile([128, NT, R], F32, tag="rlog")
    V.tensor_tensor(rlog[:], rdots_p[:],
                    mnorm_n[:, :, None].broadcast_to([128, NT, R]), op=OP.mult)
    rexp = wp.tile([128, NT, R], F32, tag="rexp")
    S.activation(rexp[:], rlog[:], AF.Exp)
    rpart = wp.tile([128, R], F32, tag="rpart")
    V.tensor_reduce(rpart[:], rexp[:].rearrange("p c r -> p r c"),
                    axis=mybir.AxisListType.X, op=OP.add)
    rsumB = wp.tile([128, R], F32, tag="rsumB")
    G.partition_all_reduce(rsumB[:], rpart[:], channels=128, reduce_op=RED.add)
    rsr = wp.tile([128, R], F32, tag="rsr")
    V.reciprocal(rsr[:], rsumB[:])
    m1rs = wp.tile([128, R], F32, tag="m1rs")
    V.tensor_tensor(m1rs[:], rsr[:], modesB[:, 1, :, t], op=OP.mult)
    rexp_s = wp.tile([128, NT, R], F32, tag="rexp_s")
    V.tensor_tensor(rexp_s[:], rexp[:],
                    m1rs[:, None, :].broadcast_to([128, NT, R]), op=OP.mult)

    # ---- read weights: accumulate bwd + content + fwd in one PSUM bank ----
    rw_p = psS.tile([R, N], F32, tag="rw")
    if t > 0:
        for c in range(NT):
            P.matmul(rw_p[:], rwm0[:, R * c:R * (c + 1)], L_n[:, c, :],
                     start=(c == 0), stop=False)
        for c in range(NT):
            P.matmul(rw_p[:], rwm2[:, R * c:R * (c + 1)], LT_n[:, c, :],
                     start=False, stop=False)
        for c in range(NT):
            P.matmul(rw_p[:, 128 * c:128 * (c + 1)], rexp_s[:, c, :],
                     ident[:], is_transpose=True, start=False,
                     stop=(c == NT - 1))
    else:
        for c in range(NT):
            P.matmul(rw_p[:, 128 * c:128 * (c + 1)], rexp_s[:, c, :],
                     ident[:], is_transpose=True, start=(c == 0),
                     stop=(c == NT - 1))
    rw = wp.tile([R, N], F32, tag="rwf")
    S.copy(rw[:], rw_p[:])
    rwT_p = psA.tile([128, NT * R], F32, tag="p")
    for c in range(NT):
        P.transpose(rwT_p[:, R * c:R * (c + 1)], rw[:, 128 * c:128 * (c + 1)],
                    ident[0:R, 0:R])
    rwT_n = sp.tile([128, NT * R], F32, tag="rwT")
    V.tensor_copy(rwT_n[:], rwT_p[:])

    # ---- read words ----
    rwd_p = psS.tile([R, W], F32, tag="s")
    for c in range(NT):
        P.matmul(rwd_p[:], rwT_n[:, R * c:R * (c + 1)], mem_nrm_n[:, c, :],
                 start=(c == 0), stop=(c == NT - 1))
    S.copy(out_sb[:, t, :], rwd_p[:])

    # ---- tail: broadcasts for the NEXT step (hoisted into this step) ----
    ub_n = pb_n = pbm_n = None
    if not last:
        ub_n = wp.tile([128, N], F32, tag="ub")
        G.partition_broadcast(ub_n[:], u_f_n[:])
        pb_n = wp.tile([128, N], F32, tag="pb")
        G.partition_broadcast(pb_n[:], prec_f_n[:])
        pbm_n = wp.tile([128, NT, N], F16, tag="pbm")
        for c in range(NT):
            G.tensor_tensor(pbm_n[:, c, :], pb_n[:], offd[:, c, :], op=OP.mult)

    return dict(memT=memT_n, mem_nrm=mem_nrm_n, mnorm=mnorm_n, L=L_n, LT=LT_n,
                u_pm=u_pm_n, prec_pm=prec_pm_n, prec_f=prec_f_n, rwT=rwT_n,
                ub=ub_n, pb=pb_n, pbm=pbm_n)


# ---------------------------------------------------------------------------
_NC_CACHE = {}


def _get_nc():
    if "nc" not in _NC_CACHE:
        _NC_CACHE["nc"] = build_nc()
    return _NC_CACHE["nc"]


def _consts():
    ident = np.eye(128, dtype=np.float32)
    identh = np.eye(128, dtype=np.float16)
    ones = np.ones((128, 128), dtype=np.float32)
    offd = (1.0 - np.eye(N)).astype(np.float16)
    return ident, identh, ones, offd


def make_in_maps(controller_output, W_if, b_if, memory0):
    ident, identh, ones, offd = _consts()
    maps = []
    for b in range(B):
        maps.append({
            "co": np.ascontiguousarray(controller_output[b]),
            "wif": np.ascontiguousarray(W_if),
            "bif": np.ascontiguousarray(b_if.reshape(1, IF)),
            "mem0": np.ascontiguousarray(memory0[b]),
            "ident": ident, "identh": identh, "ones": ones, "offdiag": offd,
        })
    return maps


def kernel(controller_output, W_if, b_if, memory0):
    from concourse.bass_utils import run_bass_kernel_spmd
    controller_output = np.asarray(controller_output, dtype=np.float32)
    W_if = np.asarray(W_if, dtype=np.float32)
    b_if = np.asarray(b_if, dtype=np.float32)
    memory0 = np.asarray(memory0, dtype=np.float32)
    nc = _get_nc()
    maps = make_in_maps(controller_output, W_if, b_if, memory0)
    res = run_bass_kernel_spmd(nc, maps, core_ids=list(range(B)))
    return np.stack([res.results[b]["out"] for b in range(B)], axis=0)


if __name__ == "__main__":
    mode = sys.argv[1] if len(sys.argv) > 1 else "sim"
    sys.path.insert(0, "/root/problem")
    import jax
    with jax.default_device(jax.devices("cpu")[0]):
        import reference
        inputs = {k: np.asarray(v) for k, v in reference.setup_inputs().items()}
        expected = np.asarray(reference.reference(**inputs))

    if mode == "sim":
        from concourse.bass_interp import CoreSim
        nc = build_nc()
        maps = make_in_maps(inputs["controller_output"], inputs["W_if"],
                            inputs["b_if"], inputs["memory0"])
        sim = CoreSim(nc)
        for k, v in maps[0].items():
            sim.tensor(k)[:] = v
        sim.simulate()
        got = sim.tensor("out").copy()
        exp = expected[0]
        err = np.abs(got - exp)
        rel = np.linalg.norm(got - exp) / (np.linalg.norm(exp) + 1e-12)
        print("sim modeled time (ns):", sim.time)
        print("max abs err:", err.max(), " rel err:", rel)
    else:
        got = kernel(**inputs)
        rel = np.linalg.norm(got - expected) / (np.linalg.norm(expected) + 1e-12)
        print("max abs err:", np.abs(got - expected).max(), " rel err:", rel)


# revision 22
# speedup vs baseline: 2.2670x; 1.0206x over previous
"""DNC MemoryAccess kernel for Trainium2 (Bass/Tile), data-parallel over batch.

Shapes (hardcoded): B=8, T=16, C=1024, IFACE=471, N=512, WORD=64, R=4, NW=1.
Each of the 8 cores processes one batch element; all recurrent state
(memory [64,512]T + [128,4,64], link/linkT [128,4,512] f16, usage/prec,
read_weights) stays SBUF-resident across the T=16 sequential steps.

Engine balance (vs the DVE-bound ancestor): broadcasts and the link-decay
scalar_tensor_tensor chain run on GpSimd (partition_broadcast / STT at
0.83 ns/el), PSUM->SBUF copies run on ACT (one act-table set covers
Copy/Ln/Square/Exp so no per-step table reloads), usage/prec update in
partition-major [128,4] layout, softmax sums via gpsimd partition_all_reduce,
and the read-weight mode mix is accumulated in one PSUM bank by the PE
(bwd/fwd matmuls in f16 + transposed content term).

Precision notes: ACT-table exp has ~1e-5 max rel err, enough to flip the
DNC allocation sort on near-tied usage values. So: strengths/key-norms are
precomputed in the prologue with a full-precision polynomial exp, per-step
norms use Newton-refined rsqrt, and the three per-step exps use the
magic-bits 2^x polynomial (pexp2). Link/linkT state and the allocation
log-sum matmul run in f16 (validated against the 2e-2 harness gate).
"""
import sys

sys.path.insert(0, "/opt/trn_rl_repo")

import numpy as np

import concourse.bacc as bacc
import concourse.bass as bass
import concourse.bass_isa as bass_isa
import concourse.mybir as mybir
import concourse.tile as tile

F32 = mybir.dt.float32
F16 = mybir.dt.float16
F32R = mybir.dt.float32r
I32 = mybir.dt.int32
AF = mybir.ActivationFunctionType
OP = mybir.AluOpType
RED = bass_isa.ReduceOp

B, T, C, IF = 8, 16, 1024, 471
N, W, R = 512, 64, 4
NT = N // 128  # 4 N-tiles
LOG2E = 1.4426950408889634
MAGIC2 = 12582912.0 + 127.0  # round-to-int magic + exponent bias for 2^k bits
_LN2 = 0.6931471805599453
# 2^f = 1 + sum_{i>=1} EXPC[i-1] f^i  (Taylor of exp(f ln2); deg-6 err ~2e-9)
EXPC = [_LN2, _LN2**2 / 2, _LN2**3 / 6, _LN2**4 / 24, _LN2**5 / 120,
        _LN2**6 / 720]

# iface field offsets
O_RK, O_RS, O_WK, O_WS = 0, 256, 260, 324
O_ER, O_WV, O_FG, O_AG, O_WG, O_MD = 325, 389, 453, 457, 458, 459


def build_nc():
    nc = bacc.Bacc("TRN2", target_bir_lowering=False, debug=False, num_devices=8)

    co_d = nc.declare_dram_parameter("co", [T, C], F32, isOutput=False)
    w_d = nc.declare_dram_parameter("wif", [C, IF], F32, isOutput=False)
    b_d = nc.declare_dram_parameter("bif", [1, IF], F32, isOutput=False)
    m0_d = nc.declare_dram_parameter("mem0", [N, W], F32, isOutput=False)
    ident_d = nc.declare_dram_parameter("ident", [128, 128], F32, isOutput=False)
    identh_d = nc.declare_dram_parameter("identh", [128, 128], F16, isOutput=False)
    ones_d = nc.declare_dram_parameter("ones", [128, 128], F32, isOutput=False)
    offd_d = nc.declare_dram_parameter("offdiag", [N, N], F16, isOutput=False)
    out_d = nc.declare_dram_parameter("out", [T, R, W], F32, isOutput=True)

    with tile.TileContext(nc) as tc:
        with (
            tc.tile_pool(name="const", bufs=1) as cp,
            tc.tile_pool(name="state", bufs=2) as sp,
            tc.tile_pool(name="work", bufs=2) as wp,
            tc.tile_pool(name="psP", bufs=2, space="PSUM") as psA,
            tc.tile_pool(name="psM", bufs=2, space="PSUM") as psM,
            tc.tile_pool(name="psS", bufs=1, space="PSUM") as psS,
        ):
            _build_body(nc, tc, cp, sp, wp, psA, psM, psS,
                        co_d, w_d, b_d, m0_d, ident_d, identh_d, ones_d,
                        offd_d, out_d)
    nc.compile()
    return nc


def _helpers(nc):
    V, S = nc.vector, nc.scalar

    def pexp(pool, x_ap, shape, tg, nb=1):
        """exp(x) to ~1e-7 via 2^(x*log2e): magic rounding + deg-6 poly +
        exponent-bit assembly. ~13 DVE ops; prologue/small-tensor use."""
        t_ = pool.tile(shape, F32, tag=tg + "_t", name=tg + "_t")
        V.tensor_scalar(t_[:], x_ap, LOG2E, None, op0=OP.mult)
        a_ = pool.tile(shape, F32, tag=tg + "_a", name=tg + "_a")
        V.tensor_scalar(a_[:], t_[:], MAGIC2, None, op0=OP.add)
        k_ = pool.tile(shape, F32, tag=tg + "_k", name=tg + "_k")
        V.tensor_scalar(k_[:], a_[:], MAGIC2, None, op0=OP.subtract)
        f_ = pool.tile(shape, F32, tag=tg + "_f", name=tg + "_f")
        V.tensor_tensor(f_[:], t_[:], k_[:], op=OP.subtract)
        p2 = pool.tile(shape, I32, tag=tg + "_p2", name=tg + "_p2")
        V.tensor_scalar(p2[:], a_[:].bitcast(I32), 23, None,
                        op0=OP.arith_shift_left)
        ac = [pool.tile(shape, F32, tag=tg + "_ac0", name=tg + "_ac0"),
              pool.tile(shape, F32, tag=tg + "_ac1", name=tg + "_ac1")]
        V.tensor_scalar(ac[0][:], f_[:], EXPC[5], None, op0=OP.mult)
        cur = 0
        for c_ in (EXPC[4], EXPC[3], EXPC[2], EXPC[1], EXPC[0]):
            V.scalar_tensor_tensor(ac[1 - cur][:], ac[cur][:], c_, f_[:],
                                   op0=OP.add, op1=OP.mult)
            cur = 1 - cur
        y_ = pool.tile(shape, F32, tag=tg + "_y", name=tg + "_y")
        V.scalar_tensor_tensor(y_[:], ac[cur][:], 1.0, p2[:].bitcast(F32),
                               op0=OP.add, op1=OP.mult)
        return y_

    def pexp2(pool, x_ap, shape, tg, accum_out=None, out=None, clamp=None):
        """2^x for prescaled x (log2 units); magic-round + deg-6 poly +
        exponent bits. All DVE; ~1e-7. x_ap should be SBUF."""
        if clamp is not None:
            t_ = pool.tile(shape, F32, tag=tg + "_t", name=tg + "_t", bufs=1)
            V.tensor_scalar(t_[:], x_ap, clamp, None, op0=OP.max)
            x_ap = t_[:]
        a_ = pool.tile(shape, F32, tag=tg + "_a", name=tg + "_a", bufs=1)
        V.tensor_scalar(a_[:], x_ap, MAGIC2, None, op0=OP.add)
        fn = pool.tile(shape, F32, tag=tg + "_fn", name=tg + "_fn", bufs=1)
        V.scalar_tensor_tensor(fn[:], a_[:], MAGIC2, x_ap,
                               op0=OP.subtract, op1=OP.subtract)  # -f
        p2 = pool.tile(shape, I32, tag=tg + "_p2", name=tg + "_p2", bufs=1)
        V.tensor_scalar(p2[:], a_[:].bitcast(I32), 23, None,
                        op0=OP.arith_shift_left)
        ac = [pool.tile(shape, F32, tag=tg + "_a0", name=tg + "_a0", bufs=1),
              pool.tile(shape, F32, tag=tg + "_a1", name=tg + "_a1", bufs=1)]
        V.tensor_scalar(ac[0][:], fn[:], EXPC[5], None, op0=OP.mult)
        cur = 0
        for i, c_ in ((5, EXPC[4]), (4, EXPC[3]), (3, EXPC[2]), (2, EXPC[1]),
                      (1, EXPC[0])):
            b_ = c_ if (i % 2 == 0) else -c_
            V.scalar_tensor_tensor(ac[1 - cur][:], ac[cur][:], b_, fn[:],
                                   op0=OP.add, op1=OP.mult)
            cur = 1 - cur
        if out is None:
            out_t = pool.tile(shape, F32, tag=tg + "_y", name=tg + "_y", bufs=1)
            out = out_t[:]
        V.scalar_tensor_tensor(out, ac[cur][:], 1.0, p2[:].bitcast(F32),
                               op0=OP.add, op1=OP.mult, accum_out=accum_out)
        return out

    _LNC = [0.9999751958009936, -0.49938365136996526, 0.3277847093008827,
            -0.22061263120600254, 0.1361341477101341, -0.06570959215415353,
            0.019091997353919793, -0.002558717382663216]

    def lnu_dve(pool, x_ap, shape, tg, out=None):
        """ln(x) for x in (0, 1]-ish via exponent bits + deg-7 poly on
        mantissa-1; ~13 DVE ops, max abs err ~5e-7. Subnormal x gives
        ln ~ -88 (harmless under the -87 clamp downstream)."""
        eb = pool.tile(shape, I32, tag=tg + "_eb", name=tg + "_eb", bufs=1)
        V.tensor_scalar(eb[:], x_ap.bitcast(I32), 23, None,
                        op0=OP.logical_shift_right)
        ef = pool.tile(shape, F32, tag=tg + "_ef", name=tg + "_ef", bufs=1)
        V.tensor_scalar(ef[:], eb[:], -127, None, op0=OP.add)
        mb = pool.tile(shape, I32, tag=tg + "_mb", name=tg + "_mb", bufs=1)
        V.tensor_scalar(mb[:], x_ap.bitcast(I32), 0x7FFFFF, None,
                        op0=OP.bitwise_and)
        m_ = pool.tile(shape, F32, tag=tg + "_m", name=tg + "_m", bufs=1)
        V.tensor_scalar(m_[:].bitcast(I32), mb[:], 0x3F800000, None,
                        op0=OP.bitwise_or)
        s_ = pool.tile(shape, F32, tag=tg + "_s", name=tg + "_s", bufs=1)
        V.tensor_scalar(s_[:], m_[:], -1.0, None, op0=OP.add)
        ac = [pool.tile(shape, F32, tag=tg + "_a0", name=tg + "_a0", bufs=1),
              pool.tile(shape, F32, tag=tg + "_a1", name=tg + "_a1", bufs=1)]
        V.tensor_scalar(ac[0][:], s_[:], _LNC[7], None, op0=OP.mult)
        cur = 0
        for c_ in (_LNC[6], _LNC[5], _LNC[4], _LNC[3], _LNC[2], _LNC[1],
                   _LNC[0]):
            V.scalar_tensor_tensor(ac[1 - cur][:], ac[cur][:], c_, s_[:],
                                   op0=OP.add, op1=OP.mult)
            cur = 1 - cur
        ln1p = pool.tile(shape, F32, tag=tg + "_l", name=tg + "_l", bufs=1)
        V.tensor_tensor(ln1p[:], ac[cur][:], s_[:], op=OP.mult)
        if out is None:
            o_ = pool.tile(shape, F32, tag=tg + "_o", name=tg + "_o", bufs=1)
            out = o_[:]
        V.scalar_tensor_tensor(out, ef[:], _LN2, ln1p[:],
                               op0=OP.mult, op1=OP.add)
        return out

    def rsqrt_sb(pool, x_sb, shape, tg, out=None, iters=3):
        """rsqrt via quake seed + Newton; all DVE/gpsimd, no ACT.
        x_sb must be an SBUF ap."""
        sh = pool.tile(shape, I32, tag=tg + "_sh", name=tg + "_sh", bufs=1)
        V.tensor_scalar(sh[:], x_sb.bitcast(I32), 1, None,
                        op0=OP.arith_shift_right)
        nb = pool.tile(shape, I32, tag=tg + "_nb", name=tg + "_nb", bufs=1)
        V.tensor_scalar(nb[:], sh[:], -1, None, op0=OP.bitwise_xor)
        y_ = pool.tile(shape, F32, tag=tg + "_y", name=tg + "_y", bufs=1)
        V.tensor_scalar(y_[:].bitcast(I32), nb[:], 0x5f3759e0, None, op0=OP.add)
        for i in range(iters):
            s_ = pool.tile(shape, F32, tag=tg + f"_s{i}", name=tg + f"_s{i}", bufs=1)
            V.tensor_tensor(s_[:], y_[:], y_[:], op=OP.mult)
            t_ = pool.tile(shape, F32, tag=tg + f"_t{i}", name=tg + f"_t{i}", bufs=1)
            V.tensor_tensor(t_[:], x_sb, s_[:], op=OP.mult)
            h_ = pool.tile(shape, F32, tag=tg + f"_h{i}", name=tg + f"_h{i}", bufs=1)
            V.tensor_scalar(h_[:], t_[:], -0.5, 1.5, op0=OP.mult, op1=OP.add)
            if i == iters - 1 and out is not None:
                V.tensor_tensor(out, y_[:], h_[:], op=OP.mult)
                return out
            y2 = pool.tile(shape, F32, tag=tg + f"_y{i}", name=tg + f"_y{i}", bufs=1)
            V.tensor_tensor(y2[:], y_[:], h_[:], op=OP.mult)
            y_ = y2
        return y_[:]

    def softplus_precise(pool, x_ap, shape, tg):
        """ln(1+e^x) with table-ln seed + one Newton step (via pexp)."""
        e_ = pexp(pool, x_ap, shape, tg + "e")
        w_ = pool.tile(shape, F32, tag=tg + "_w", name=tg + "_w")
        V.tensor_scalar(w_[:], e_[:], 1.0, None, op0=OP.add)
        z_ = pool.tile(shape, F32, tag=tg + "_z", name=tg + "_z")
        S.activation(z_[:], w_[:], AF.Ln)
        nz = pool.tile(shape, F32, tag=tg + "_nz", name=tg + "_nz")
        S.mul(nz[:], z_[:], -1.0)
        e2 = pexp(pool, nz[:], shape, tg + "e2")
        m_ = pool.tile(shape, F32, tag=tg + "_m", name=tg + "_m")
        V.tensor_tensor(m_[:], w_[:], e2[:], op=OP.mult)
        o_ = pool.tile(shape, F32, tag=tg + "_o", name=tg + "_o")
        V.scalar_tensor_tensor(o_[:], m_[:], -1.0, z_[:], op0=OP.add, op1=OP.add)
        return o_

    def rsqrt_refined(pool, x_ap, shape, tg, iters=1, nb=1):
        """rsqrt(x): ACT-table seed exp(-0.5 ln x) + Newton (no division)."""
        l_ = pool.tile(shape, F32, tag=tg + "_l", name=tg + "_l", bufs=nb)
        S.activation(l_[:], x_ap, AF.Ln)
        y_ = pool.tile(shape, F32, tag=tg + "_y", name=tg + "_y", bufs=nb)
        S.activation(y_[:], l_[:], AF.Exp, scale=-0.5)
        for i in range(iters):
            s_ = pool.tile(shape, F32, tag=tg + f"_s{i}", name=tg + f"_s{i}", bufs=nb)
            nc.gpsimd.tensor_tensor(s_[:], y_[:], y_[:], op=OP.mult)
            t_ = pool.tile(shape, F32, tag=tg + f"_t{i}", name=tg + f"_t{i}", bufs=nb)
            V.tensor_tensor(t_[:], x_ap, s_[:], op=OP.mult)
            h_ = pool.tile(shape, F32, tag=tg + f"_h{i}", name=tg + f"_h{i}", bufs=nb)
            V.tensor_scalar(h_[:], t_[:], -0.5, 1.5, op0=OP.mult, op1=OP.add)
            y2 = pool.tile(shape, F32, tag=tg + f"_y{i}", name=tg + f"_y{i}", bufs=nb)
            V.tensor_tensor(y2[:], y_[:], h_[:], op=OP.mult)
            y_ = y2
        return y_

    return (pexp, softplus_precise, rsqrt_refined, pexp2, rsqrt_sb,
            lnu_dve)


def _build_body(nc, tc, cp, sp, wp, psA, psM, psS,
                co_d, w_d, b_d, m0_d, ident_d, identh_d, ones_d, offd_d, out_d):
    V, S, P, DMA, G = nc.vector, nc.scalar, nc.tensor, nc.sync, nc.gpsimd
    (pexp, softplus_precise, rsqrt_refined, pexp2, rsqrt_sb,
     lnu_dve) = _helpers(nc)

    # ---------------- constants ----------------
    ident = cp.tile([128, 128], F32)
    DMA.dma_start(ident[:], ident_d[:])
    identh = cp.tile([128, 128], F16)
    DMA.dma_start(identh[:], identh_d[:])
    ones = cp.tile([128, 128], F32)
    DMA.dma_start(ones[:], ones_d[:])
    offd = cp.tile([128, NT, N], F16)
    for c in range(NT):
        DMA.dma_start(offd[:, c, :], offd_d[128 * c:128 * (c + 1), :])
    nege0_pm = cp.tile([128, NT], F32)
    G.memset(nege0_pm[:], 0.0)
    G.memset(nege0_pm[0:1, 0:1], -1.0)

    # persistent per-t parse tables
    rkT = cp.tile([W, R, T], F32)
    wkT = cp.tile([W, T], F32)
    neg_er = cp.tile([W, T], F32)
    wvT = cp.tile([W, T], F32)
    bkw = cp.tile([1, T], F32)       # softplus(ws)/||wk||
    wkT2 = cp.tile([W, T], F32R)     # wk * bkw (f32r for fast PE dots)
    rkT2 = cp.tile([W, R, T], F32R)  # rk * bkr (f32r)
    fgB = cp.tile([128, R, T], F32)  # sigmoid(fg) bcast to all partitions
    c1nB = cp.tile([128, T], F32)    # -ag*wg bcast
    c2B = cp.tile([128, T], F32)     # wg - ag*wg bcast
    modesB = cp.tile([128, 3, R, T], F32)  # softmax(modes) bcast

    # ---------------- prologue: iface (transient pool) ----------------
    with tc.tile_pool(name="prolog", bufs=1) as pp:
        co_sb = pp.tile([T, C], F32)
        DMA.dma_start(co_sb[:], co_d[:])
        bif_sb = pp.tile([1, IF], F32)
        DMA.dma_start(bif_sb[:], b_d[:])

        coT_p = psA.tile([128, 8, T], F32, tag="p")
        for k in range(8):
            P.transpose(coT_p[:, k, :], co_sb[:, 128 * k:128 * (k + 1)],
                        ident[0:T, 0:T])
        coT = pp.tile([128, 8, T], F32)
        S.copy(coT[:], coT_p[:])

        if_p = psS.tile([T, IF], F32, tag="s")
        for h in range(2):
            w_sb = pp.tile([128, 4, IF], F32, tag="w_sb", name=f"w_sb{h}")
            for k in range(4):
                DMA.dma_start(w_sb[:, k, :],
                              w_d[128 * (4 * h + k):128 * (4 * h + k + 1), :])
            for k in range(4):
                P.matmul(if_p[:], coT[:, 4 * h + k, :], w_sb[:, k, :],
                         start=(h == 0 and k == 0), stop=False)
        P.matmul(if_p[:], ones[0:1, 0:T], bif_sb[:], start=False, stop=True)
        iface = pp.tile([T, IF], F32)
        S.copy(iface[:], if_p[:])

        # field transposes -> per-t column layouts
        def tp_field(lo, hi):
            n = hi - lo
            pt = psA.tile([128, T], F32, tag="p")
            P.transpose(pt[0:n, :], iface[:, lo:hi], ident[0:T, 0:T])
            return pt

        for r in range(R):
            pt = tp_field(O_RK + W * r, O_RK + W * (r + 1))
            S.copy(rkT[:, r, :], pt[0:W, :])
        pt = tp_field(O_WK, O_WK + W)
        S.copy(wkT[:], pt[0:W, :])
        pt = tp_field(O_ER, O_ER + W)
        er_in = pp.tile([W, T], F32)
        V.tensor_scalar(er_in[:], pt[0:W, :], -1.0, None, op0=OP.mult)
        er_e = pexp(pp, er_in[:], [W, T], "sge")     # e^{-x}
        er_w = pp.tile([W, T], F32)
        V.tensor_scalar(er_w[:], er_e[:], 1.0, None, op0=OP.add)
        er_r = pp.tile([W, T], F32)
        V.reciprocal(er_r[:], er_w[:])               # sigmoid(x)
        V.tensor_scalar(neg_er[:], er_r[:], -1.0, None, op0=OP.mult)
        pt = tp_field(O_WV, O_WV + W)
        S.copy(wvT[:], pt[0:W, :])

        # strengths / ||k|| folded: bkw, bkr  (free-layout pipeline)
        rsF_p = psA.tile([1, R, T], F32, tag="p")
        for r in range(R):
            P.transpose(rsF_p[0:1, r, :], iface[:, O_RS + r:O_RS + r + 1],
                        ident[0:T, 0:T])
        rsF = pp.tile([1, R, T], F32)
        S.copy(rsF[:], rsF_p[:])
        wsF_p = psA.tile([1, T], F32, tag="p")
        P.transpose(wsF_p[:], iface[:, O_WS:O_WS + 1], ident[0:T, 0:T])
        wsF = pp.tile([1, T], F32)
        S.copy(wsF[:], wsF_p[:])
        rs_pre = softplus_precise(pp, rsF[:].rearrange("o r t -> o (r t)"),
                                  [1, R * T], "rsp")
        ws_pre = softplus_precise(pp, wsF[:], [1, T], "wsp")

        sqw = pp.tile([W, T], F32)
        S.square(sqw[:], wkT[:])
        wk2_p = psM.tile([1, T], F32, tag="m")
        P.matmul(wk2_p[:], ones[0:W, 0:1], sqw[:])
        wkr = rsqrt_refined(pp, wk2_p[:], [1, T], "wkr", iters=2)
        V.tensor_tensor(bkw[:], ws_pre[:], wkr[:], op=OP.mult)

        sqr = pp.tile([W, R, T], F32)
        S.square(sqr[:], rkT[:])
        rk2_p = psM.tile([1, R * T], F32, tag="m")
        P.matmul(rk2_p[:], ones[0:W, 0:1], sqr[:].rearrange("w r t -> w (r t)"))
        rkr = rsqrt_refined(pp, rk2_p[:], [1, R * T], "rkr", iters=2)
        bkrF = cp.tile([1, R, T], F32, name="bkrF")
        V.tensor_tensor(bkrF[:].rearrange("o r t -> o (r t)"), rs_pre[:],
                        rkr[:], op=OP.mult)
        # scaled keys (natural-log units): wkT2 = wkT*bkw ; rkT2 = rkT*bkr
        bkwb_p = psA.tile([W, T], F32, tag="p")
        P.matmul(bkwb_p[:], ones[0:1, 0:W], bkw[:])
        V.tensor_tensor(wkT2[:], wkT[:], bkwb_p[:], op=OP.mult)
        bkrb_p = psA.tile([W, R * T], F32, tag="p")
        P.matmul(bkrb_p[:], ones[0:1, 0:W], bkrF[:].rearrange("o r t -> o (r t)"))
        V.tensor_tensor(rkT2[:].rearrange("w r t -> w (r t)"),
                        rkT[:].rearrange("w r t -> w (r t)"),
                        bkrb_p[:], op=OP.mult)

        # gates: fg, ag, wg sigmoids via precise V pipeline, packed [1,6,T]
        gats_p = psA.tile([1, 6, T], F32, tag="p")
        for r in range(R):
            P.transpose(gats_p[0:1, r, :], iface[:, O_FG + r:O_FG + r + 1],
                        ident[0:T, 0:T])
        P.transpose(gats_p[0:1, 4, :], iface[:, O_AG:O_AG + 1], ident[0:T, 0:T])
        P.transpose(gats_p[0:1, 5, :], iface[:, O_WG:O_WG + 1], ident[0:T, 0:T])
        g_in = pp.tile([1, 6 * T], F32)
        V.tensor_scalar(g_in[:], gats_p[:].rearrange("o g t -> o (g t)"), -1.0,
                        None, op0=OP.mult)
        g_e = pexp(pp, g_in[:], [1, 6 * T], "sgg")
        g_w = pp.tile([1, 6 * T], F32)
        V.tensor_scalar(g_w[:], g_e[:], 1.0, None, op0=OP.add)
        g_r = pp.tile([1, 6, T], F32)
        V.reciprocal(g_r[:].rearrange("o g t -> o (g t)"), g_w[:])
        G.partition_broadcast(fgB[:].rearrange("p r t -> p (r t)"),
                              g_r[0:1, 0:R, :].rearrange("o r t -> o (r t)"))
        ag_t = g_r[0:1, 4, :]
        wg_t = g_r[0:1, 5, :]
        c1t = pp.tile([1, T], F32)
        V.tensor_tensor(c1t[:], ag_t, wg_t, op=OP.mult)
        c1n = pp.tile([1, T], F32)
        V.tensor_scalar(c1n[:], c1t[:], -1.0, None, op0=OP.mult)
        c2 = pp.tile([1, T], F32)
        V.tensor_tensor(c2[:], wg_t, c1t[:], op=OP.subtract)
        G.partition_broadcast(c1nB[:], c1n[:])
        G.partition_broadcast(c2B[:], c2[:])

        # modes softmax (precise exp; normalize in [T,12]; flatten; bcast)
        me = pexp(pp, iface[:, O_MD:O_MD + 12], [T, 12], "me")
        me3 = me[:].rearrange("t (r m) -> t r m", m=3)
        msum = pp.tile([T, R], F32)
        V.tensor_tensor(msum[:], me3[:, :, 0], me3[:, :, 1], op=OP.add)
        V.tensor_tensor(msum[:], msum[:], me3[:, :, 2], op=OP.add)
        mrs = pp.tile([T, R], F32)
        V.reciprocal(mrs[:], msum[:])
        mn = pp.tile([T, 12], F32)
        mn3 = mn[:].rearrange("t (r m) -> t r m", m=3)
        for m in range(3):
            V.tensor_tensor(mn3[:, :, m], me3[:, :, m], mrs[:], op=OP.mult)
        mo = pp.tile([T, 12], F32)
        mo3 = mo[:].rearrange("t (m r) -> t m r", r=R)
        S.copy(mo3[:], mn3[:].rearrange("t r m -> t m r"))
        # flatten [T,12] -> [1, (m r t)] via 12 single-column transposes
        mF_p = psA.tile([1, 12, T], F32, tag="p")
        for j in range(12):
            P.transpose(mF_p[0:1, j, :], mo[:, j:j + 1], ident[0:T, 0:T])
        mF = pp.tile([1, 12, T], F32)
        S.copy(mF[:], mF_p[:])
        G.partition_broadcast(modesB[:].rearrange("p m r t -> p (m r t)"),
                              mF[:].rearrange("o j t -> o (j t)"))

    # ---------------- initial state ----------------
    mem_nrm = sp.tile([128, NT, W], F32, tag="mem_nrm")
    for c in range(NT):
        DMA.dma_start(mem_nrm[:, c, :], m0_d[128 * c:128 * (c + 1), :])
    memT_p = psA.tile([W, N], F32, tag="p")
    for c in range(NT):
        P.transpose(memT_p[:, 128 * c:128 * (c + 1)], mem_nrm[:, c, :], ident[:])
    memT = sp.tile([W, N], F32R, tag="memT")
    S.copy(memT[:], memT_p[:])

    sq0 = wp.tile([128, NT, W], F32, tag="sq0", bufs=1)
    S.square(sq0[:], mem_nrm[:])
    msum0 = wp.tile([128, NT], F32, tag="msum")
    V.tensor_reduce(msum0[:], sq0[:], axis=mybir.AxisListType.X, op=OP.add)
    mnorm = sp.tile([128, NT], F32, tag="mnorm")
    rsqrt_sb(wp, msum0[:], [128, NT], "w1", out=mnorm[:], iters=3)

    L = sp.tile([128, NT, N], F16, tag="L")
    G.memset(L[:], 0.0)
    LT0 = sp.tile([128, NT, N], F16, tag="LT")
    G.memset(LT0[:], 0.0)
    u_pm0 = sp.tile([128, NT], F32, tag="u_pm")
    G.memset(u_pm0[:], 0.0)
    rwT0 = sp.tile([128, NT * R], F32, tag="rwT")
    G.memset(rwT0[:], 0.0)

    out_sb = cp.tile([R, T, W], F32)

    st = dict(memT=memT, mem_nrm=mem_nrm, mnorm=mnorm, L=L, LT=LT0,
              u_pm=u_pm0, prec_pm=None, prec_f=None, rwT=rwT0,
              ub=None, pb=None, pbm=None)

    for t in range(T):
        st = _step(nc, tc, t, st, cp, sp, wp, psA, psM, psS,
                   ident, identh, ones, offd, nege0_pm, wkT2, rkT2, neg_er,
                   wvT, fgB, c1nB, c2B, modesB, out_sb, pexp2, rsqrt_sb,
                   lnu_dve)

    DMA.dma_start(out_d[:].rearrange("t r w -> r t w"), out_sb[:])


def _step(nc, tc, t, st, cp, sp, wp, psA, psM, psS,
          ident, identh, ones, offd, nege0_pm, wkT2, rkT2, neg_er, wvT,
          fgB, c1nB, c2B, modesB, out_sb, pexp2, rsqrt_sb, lnu_dve):
    LOW = 50000  # deprioritize slack work for the Tile scheduler
    V, S, P, G = nc.vector, nc.scalar, nc.tensor, nc.gpsimd
    memT, mem_nrm, mnorm = st["memT"], st["mem_nrm"], st["mnorm"]
    L, LT, u_pm, rwT = st["L"], st["LT"], st["u_pm"], st["rwT"]
    prec_pm, prec_f = st["prec_pm"], st["prec_f"]
    ub, pb, pbm = st["ub"], st["pb"], st["pbm"]
    last = (t == T - 1)

    # ---- write content dots (PE; memT from prev step; slack path) ----
    hp = tc.high_priority
    wdf_p = psM.tile([1, N], F32, tag="m")
    P.matmul(wdf_p[:], wkT2[:, t:t + 1], memT[:])
    wdf = wp.tile([1, N], F32, tag="wdf")
    S.copy(wdf[:], wdf_p[:])
    wdots_p = psA.tile([128, NT], F32, tag="p")
    for c in range(NT):
        P.transpose(wdots_p[:, c:c + 1], wdf[0:1, 128 * c:128 * (c + 1)],
                    ident[0:1, 0:1])

    # ---- DVE head: psi chain + rw mode scales (deps: prev-step rwT) ----
    if 0 < t:
        yyT = wp.tile([128, NT, R], F32, tag="yyT")
        V.scalar_tensor_tensor(
            yyT[:], fgB[:, None, :, t].broadcast_to([128, NT, R]), -1.0,
            rwT[:].rearrange("p (c r) -> p c r", r=R), op0=OP.mult, op1=OP.mult)
        om = wp.tile([128, NT, R], F32, tag="om")
        V.tensor_scalar(om[:], yyT[:], 1.0, None, op0=OP.add)
        p1 = wp.tile([128, NT], F32, tag="p1")
        V.tensor_tensor(p1[:], om[:, :, 0], om[:, :, 1], op=OP.mult)
        p2 = wp.tile([128, NT], F32, tag="p2")
        V.tensor_tensor(p2[:], om[:, :, 2], om[:, :, 3], op=OP.mult)
        psi_pm = wp.tile([128, NT], F32, tag="psi_pm")
        V.tensor_tensor(psi_pm[:], p1[:], p2[:], op=OP.mult)
        rwm0 = wp.tile([128, NT * R], F16, tag="rwm0")
        V.tensor_tensor(rwm0[:].rearrange("p (c r) -> p c r", r=R),
                        rwT[:].rearrange("p (c r) -> p c r", r=R),
                        modesB[:, 0, None, :, t].broadcast_to([128, NT, R]),
                        op=OP.mult)
        rwm2 = wp.tile([128, NT * R], F16, tag="rwm2")
        V.tensor_tensor(rwm2[:].rearrange("p (c r) -> p c r", r=R),
                        rwT[:].rearrange("p (c r) -> p c r", r=R),
                        modesB[:, 2, None, :, t].broadcast_to([128, NT, R]),
                        op=OP.mult)

    # ---- allocation pipeline (PM layout; ub broadcast from prev step) ----
    wlog = wp.tile([128, NT], F32, tag="wlog")
    V.tensor_tensor(wlog[:], wdots_p[:], mnorm[:], op=OP.mult)
    wexp = wp.tile([128, NT], F32, tag="wexp")
    S.activation(wexp[:], wlog[:], AF.Exp)
    if t > 0:
        lu2 = wp.tile([128, NT], F16, tag="lu2")
        lnu_dve(wp, u_pm[:], [128, NT], "lnu", out=lu2[:])
        Gm = wp.tile([128, NT, N], F16, tag="Gm", bufs=1)
        for c in range(NT):
            V.tensor_scalar(Gm[:, c, :], ub[:], u_pm[:, c:c + 1], None,
                            op0=OP.is_gt)
        s_p = psM.tile([1, N], F32, tag="m")
        for c in range(NT):
            P.matmul(s_p[:], lu2[:, c:c + 1], Gm[:, c, :],
                     start=(c == 0), stop=(c == NT - 1))
        s_f = wp.tile([1, N], F32, tag="s_f")
        S.copy(s_f[:], s_p[:])
        s_tp = psA.tile([128, NT], F32, tag="p")
        for c in range(NT):
            P.transpose(s_tp[:, c:c + 1], s_f[0:1, 128 * c:128 * (c + 1)],
                        ident[0:1, 0:1])
        s_cl = wp.tile([128, NT], F32, tag="s_cl")
        V.tensor_scalar(s_cl[:], s_tp[:], -87.0, None, op0=OP.max)
        es = wp.tile([128, NT], F32, tag="es")
        S.activation(es[:], s_cl[:], AF.Exp)

    wpart = wp.tile([128, 1], F32, tag="wpart")
    V.tensor_reduce(wpart[:], wexp[:], axis=mybir.AxisListType.X, op=OP.add)
    wsumB = wp.tile([128, 1], F32, tag="wsumB")
    G.partition_all_reduce(wsumB[:], wpart[:], channels=128, reduce_op=RED.add)
    wrs = wp.tile([128, 1], F32, tag="wrs")
    V.reciprocal(wrs[:], wsumB[:])
    if t > 0:
        negalloc = wp.tile([128, NT], F32, tag="negalloc")
        V.scalar_tensor_tensor(negalloc[:], u_pm[:], 1.0, es[:],
                               op0=OP.subtract, op1=OP.mult)

    # ---- write weights ww (PM primary) ----
    t_wc = wp.tile([128, NT], F32, tag="t_wc")
    V.tensor_scalar(t_wc[:], wexp[:], wrs[:], c2B[:, t:t + 1],
                    op0=OP.mult, op1=OP.mult)
    ww_pm = wp.tile([128, NT], F32, tag="ww_pm")
    swp = wp.tile([128, 1], F32, tag="swp")
    na_ap = nege0_pm[:] if t == 0 else negalloc[:]
    V.scalar_tensor_tensor(ww_pm[:], na_ap, c1nB[:, t:t + 1], t_wc[:],
                           op0=OP.mult, op1=OP.add, accum_out=swp[:])
    wwf_p = psA.tile([1, N], F32, tag="p")
    ww = wp.tile([1, N], F32, tag="ww")
    wb = wp.tile([128, N], F32, tag="wb")
    with hp():
        for c in range(NT):
            P.transpose(wwf_p[0:1, 128 * c:128 * (c + 1)], ww_pm[:, c:c + 1],
                        ident[:])
        S.copy(ww[:], wwf_p[:])
        G.partition_broadcast(wb[:], ww[:])
    swB = wp.tile([128, 1], F32, tag="swB")
    G.partition_all_reduce(swB[:], swp[:], channels=128, reduce_op=RED.add)

    # ---- prec update in PM (uses prec BEFORE update) ----
    if t == 0:
        prec_pm_n = ww_pm
        prec_f_n = ww
    elif last:
        prec_pm_n = None
        prec_f_n = None
    else:
        omsw = wp.tile([128, 1], F32, tag="omsw")
        V.tensor_scalar(omsw[:], swB[:], -1.0, 1.0, op0=OP.mult, op1=OP.add)
        prec_pm_n = sp.tile([128, NT], F32, tag="prec_pm")
        V.scalar_tensor_tensor(prec_pm_n[:], prec_pm[:], omsw[:], ww_pm[:],
                               op0=OP.mult, op1=OP.add)
        pf_p = psA.tile([1, N], F32, tag="p")
        for c in range(NT):
            P.transpose(pf_p[0:1, 128 * c:128 * (c + 1)], prec_pm_n[:, c:c + 1],
                        ident[:])
        prec_f_n = sp.tile([1, N], F32, tag="prec_f")
        S.copy(prec_f_n[:], pf_p[:])

    # ---- usage update (PM layout) ----
    if t == 0:
        u_pm_n = ww_pm
        u_f_n = ww
    elif last:
        u_pm_n = None
        u_f_n = None
    else:
        omu_pm = wp.tile([128, NT], F32, tag="omu_pm")
        V.tensor_scalar(omu_pm[:], u_pm[:], -1.0, 1.0, op0=OP.mult, op1=OP.add)
        tpm = wp.tile([128, NT], F32, tag="tpm")
        V.scalar_tensor_tensor(tpm[:], ww_pm[:], 1.0, omu_pm[:],
                               op0=OP.subtract, op1=OP.mult)
        u_pm_n = sp.tile([128, NT], F32, tag="u_pm")
        V.scalar_tensor_tensor(u_pm_n[:], tpm[:], 1.0, psi_pm[:],
                               op0=OP.add, op1=OP.mult)
        uf_p = psA.tile([1, N], F32, tag="p")
        for c in range(NT):
            P.transpose(uf_p[0:1, 128 * c:128 * (c + 1)], u_pm_n[:, c:c + 1],
                        ident[:])
        u_f_n = sp.tile([1, N], F32, tag="u_f")
        S.copy(u_f_n[:], uf_p[:])

    # ---- memory update ----
    keep = wp.tile([W, N], F32, tag="keep", bufs=1)
    m1 = wp.tile([W, N], F32, tag="m1", bufs=1)
    memT_n = sp.tile([W, N], F32R, tag="memT")
    with hp():
        V.tensor_scalar(keep[:], wb[0:W, :], neg_er[:, t:t + 1], 1.0,
                        op0=OP.mult, op1=OP.add)
        G.tensor_tensor(m1[:], memT[:], keep[:], op=OP.mult)
        V.scalar_tensor_tensor(memT_n[:], wb[0:W, :], wvT[:, t:t + 1], m1[:],
                               op0=OP.mult, op1=OP.add)
    mem_nrm_p = psA.tile([128, NT, W], F32, tag="p")
    for c in range(NT):
        P.transpose(mem_nrm_p[:, c, :],
                    memT_n[:, 128 * c:128 * (c + 1)].bitcast(F32),
                    ident[0:W, 0:W])
    mem_nrm_n = sp.tile([128, NT, W], F32, tag="mem_nrm")
    S.copy(mem_nrm_n[:], mem_nrm_p[:])
    sqn = wp.tile([128, NT, W], F32, tag="sqn", bufs=1)
    S.square(sqn[:], mem_nrm_p[:])
    msum = wp.tile([128, NT], F32, tag="msum")
    V.tensor_reduce(msum[:], sqn[:], axis=mybir.AxisListType.X, op=OP.add)
    mnorm_n = sp.tile([128, NT], F32, tag="mnorm")
    rsqrt_sb(wp, msum[:], [128, NT], "w1", out=mnorm_n[:], iters=2)

    # ---- read content dots (PE; hoisted before link transposes) ----
    rdf_p = psM.tile([R, N], F32, tag="m")
    P.matmul(rdf_p[:], rkT2[:, :, t], memT_n[:])
    rdf = wp.tile([R, N], F32, tag="rdf")
    S.copy(rdf[:], rdf_p[:])
    rdots_p = psA.tile([128, NT, R], F32, tag="p")
    for c in range(NT):
        P.transpose(rdots_p[:, c, :], rdf[:, 128 * c:128 * (c + 1)],
                    ident[0:R, 0:R])

    # ---- link update: chunks 0-1 DVE 2-STT, chunks 2-3 DVE-TS + Pool-TT ----
    if t == 0:
        L_n, LT_n = L, LT  # stays zero
    else:
        omw_pm = wp.tile([128, NT], F32, tag="omw_pm")
        V.tensor_scalar(omw_pm[:], ww_pm[:], -1.0, 1.0, op0=OP.mult, op1=OP.add)
        L_n = sp.tile([128, NT, N], F16, tag="L")
        for c in range(NT):
            Dm = wp.tile([128, N], F16, tag=f"Dm_{c % 2}", name=f"Dm_{c}")
            V.tensor_scalar(Dm[:], wb[:], omw_pm[:, c:c + 1], -1.0,
                            op0=OP.subtract, op1=OP.mult)
            q_ = wp.tile([128, N], F16, tag=f"q_{c % 2}", name=f"q_{c}")
            V.tensor_scalar(q_[:], pbm[:, c, :], ww_pm[:, c:c + 1], None,
                            op0=OP.mult)
            t1 = wp.tile([128, N], F16, tag=f"t1h_{c % 2}", name=f"t1h_{c}")
            G.tensor_tensor(t1[:], Dm[:], L[:, c, :], op=OP.mult)
            G.tensor_tensor(L_n[:, c, :], q_[:], t1[:], op=OP.add)
        LT_n = sp.tile([128, NT, N], F16, tag="LT")
        for j in range(NT):
            lt_p = psA.tile([128, N], F16, tag="plt")
            for i in range(NT):
                P.transpose(lt_p[:, 128 * i:128 * (i + 1)],
                            L_n[:, i, 128 * j:128 * (j + 1)], identh[:])
            if j < 2:
                S.copy(LT_n[:, j, :], lt_p[:])
            else:
                V.tensor_copy(LT_n[:, j, :], lt_p[:])

    # ---- read softmax ----
    rlog = wp.tile([128, NT, R], F32, tag="rlog")
    V.tensor_tensor(rlog[:], rdots_p[:],
                    mnorm_n[:, :, None].broadcast_to([128, NT, R]), op=OP.mult)
    rexp = wp.tile([128, NT, R], F32, tag="rexp")
    S.activation(rexp[:], rlog[:], AF.Exp)
    rpart = wp.tile([128, R], F32, tag="rpart")
    V.tensor_reduce(rpart[:], rexp[:].rearrange("p c r -> p r c"),
                    axis=mybir.AxisListType.X, op=OP.add)
    rsumB = wp.tile([128, R], F32, tag="rsumB")
    G.partition_all_reduce(rsumB[:], rpart[:], channels=128, reduce_op=RED.add)
    rsr = wp.tile([128, R], F32, tag="rsr")
    V.reciprocal(rsr[:], rsumB[:])
    m1rs = wp.tile([128, R], F32, tag="m1rs")
    V.tensor_tensor(m1rs[:], rsr[:], modesB[:, 1, :, t], op=OP.mult)
    rexp_s = wp.tile([128, NT, R], F32, tag="rexp_s")
    V.tensor_tensor(rexp_s[:], rexp[:],
                    m1rs[:, None, :].broadcast_to([128, NT, R]), op=OP.mult)

    # ---- read weights: accumulate bwd + content + fwd in one PSUM bank ----
    rw_p = psS.tile([R, N], F32, tag="rw")
    if t > 0:
        for c in range(NT):
            P.matmul(rw_p[:], rwm0[:, R * c:R * (c + 1)], L_n[:, c, :],
                     start=(c == 0), stop=False)
        for c in range(NT):
            P.matmul(rw_p[:], rwm2[:, R * c:R * (c + 1)], LT_n[:, c, :],
                     start=False, stop=False)
        for c in range(NT):
            P.matmul(rw_p[:, 128 * c:128 * (c + 1)], rexp_s[:, c, :],
                     ident[:], is_transpose=True, start=False,
                     stop=(c == NT - 1))
    else:
        for c in range(NT):
            P.matmul(rw_p[:, 128 * c:128 * (c + 1)], rexp_s[:, c, :],
                     ident[:], is_transpose=True, start=(c == 0),
                     stop=(c == NT - 1))
    rw = wp.tile([R, N], F32, tag="rwf")
    rwT_p = psA.tile([128, NT * R], F32, tag="p")
    rwT_n = sp.tile([128, NT * R], F32, tag="rwT")
    with hp():
        S.copy(rw[:], rw_p[:])
        for c in range(NT):
            P.transpose(rwT_p[:, R * c:R * (c + 1)],
                        rw[:, 128 * c:128 * (c + 1)], ident[0:R, 0:R])
        V.tensor_copy(rwT_n[:], rwT_p[:])

    # ---- read words ----
    rwd_p = psS.tile([R, W], F32, tag="s")
    for c in range(NT):
        P.matmul(rwd_p[:], rwT_n[:, R * c:R * (c + 1)], mem_nrm_n[:, c, :],
                 start=(c == 0), stop=(c == NT - 1))
    S.copy(out_sb[:, t, :], rwd_p[:])

    # ---- tail: broadcasts for the NEXT step (hoisted into this step) ----
    ub_n = pb_n = pbm_n = None
    if not last:
        ub_n = wp.tile([128, N], F32, tag="ub")
        pb_n = wp.tile([128, N], F32, tag="pb")
        with hp():
            G.partition_broadcast(ub_n[:], u_f_n[:])
            G.partition_broadcast(pb_n[:], prec_f_n[:])
        pbm_n = wp.tile([128, NT, N], F16, tag="pbm")
        for c in range(NT):
            G.tensor_tensor(pbm_n[:, c, :], pb_n[:], offd[:, c, :], op=OP.mult)

    return dict(memT=memT_n, mem_nrm=mem_nrm_n, mnorm=mnorm_n, L=L_n, LT=LT_n,
                u_pm=u_pm_n, prec_pm=prec_pm_n, prec_f=prec_f_n, rwT=rwT_n,
                ub=ub_n, pb=pb_n, pbm=pbm_n)


# ---------------------------------------------------------------------------
_NC_CACHE = {}


def _get_nc():
    if "nc" not in _NC_CACHE:
        _NC_CACHE["nc"] = build_nc()
    return _NC_CACHE["nc"]


def _consts():
    ident = np.eye(128, dtype=np.float32)
    identh = np.eye(128, dtype=np.float16)
    ones = np.ones((128, 128), dtype=np.float32)
    offd = (1.0 - np.eye(N)).astype(np.float16)
    return ident, identh, ones, offd


def make_in_maps(controller_output, W_if, b_if, memory0):
    ident, identh, ones, offd = _consts()
    maps = []
    for b in range(B):
        maps.append({
            "co": np.ascontiguousarray(controller_output[b]),
            "wif": np.ascontiguousarray(W_if),
            "bif": np.ascontiguousarray(b_if.reshape(1, IF)),
            "mem0": np.ascontiguousarray(memory0[b]),
            "ident": ident, "identh": identh, "ones": ones, "offdiag": offd,
        })
    return maps


def kernel(controller_output, W_if, b_if, memory0):
    from concourse.bass_utils import run_bass_kernel_spmd
    controller_output = np.asarray(controller_output, dtype=np.float32)
    W_if = np.asarray(W_if, dtype=np.float32)
    b_if = np.asarray(b_if, dtype=np.float32)
    memory0 = np.asarray(memory0, dtype=np.float32)
    nc = _get_nc()
    maps = make_in_maps(controller_output, W_if, b_if, memory0)
    res = run_bass_kernel_spmd(nc, maps, core_ids=list(range(B)))
    return np.stack([res.results[b]["out"] for b in range(B)], axis=0)


if __name__ == "__main__":
    mode = sys.argv[1] if len(sys.argv) > 1 else "sim"
    sys.path.insert(0, "/root/problem")
    import jax
    with jax.default_device(jax.devices("cpu")[0]):
        import reference
        inputs = {k: np.asarray(v) for k, v in reference.setup_inputs().items()}
        expected = np.asarray(reference.reference(**inputs))

    if mode == "sim":
        from concourse.bass_interp import CoreSim
        nc = build_nc()
        maps = make_in_maps(inputs["controller_output"], inputs["W_if"],
                            inputs["b_if"], inputs["memory0"])
        sim = CoreSim(nc)
        for k, v in maps[0].items():
            sim.tensor(k)[:] = v
        sim.simulate()
        got = sim.tensor("out").copy()
        exp = expected[0]
        err = np.abs(got - exp)
        rel = np.linalg.norm(got - exp) / (np.linalg.norm(exp) + 1e-12)
        print("sim modeled time (ns):", sim.time)
        print("max abs err:", err.max(), " rel err:", rel)
    else:
        got = kernel(**inputs)
        rel = np.linalg.norm(got - expected) / (np.linalg.norm(expected) + 1e-12)
        print("max abs err:", np.abs(got - expected).max(), " rel err:", rel)


# revision 31
# speedup vs baseline: 2.3355x; 1.0302x over previous
"""DNC MemoryAccess kernel for Trainium2 (Bass/Tile), data-parallel over batch.

Shapes (hardcoded): B=8, T=16, C=1024, IFACE=471, N=512, WORD=64, R=4, NW=1.
Each of the 8 cores processes one batch element; all recurrent state
(memory [64,512]T + [128,4,64], link/linkT [128,4,512] f16, usage/prec,
read_weights) stays SBUF-resident across the T=16 sequential steps.

Engine balance (vs the DVE-bound ancestor): broadcasts and the link-decay
scalar_tensor_tensor chain run on GpSimd (partition_broadcast / STT at
0.83 ns/el), PSUM->SBUF copies run on ACT (one act-table set covers
Copy/Ln/Square/Exp so no per-step table reloads), usage/prec update in
partition-major [128,4] layout, softmax sums via gpsimd partition_all_reduce,
and the read-weight mode mix is accumulated in one PSUM bank by the PE
(bwd/fwd matmuls in f16 + transposed content term).

Precision notes: ACT-table exp has ~1e-5 max rel err, enough to flip the
DNC allocation sort on near-tied usage values. So: strengths/key-norms are
precomputed in the prologue with a full-precision polynomial exp, per-step
norms use Newton-refined rsqrt, and the three per-step exps use the
magic-bits 2^x polynomial (pexp2). Link/linkT state and the allocation
log-sum matmul run in f16 (validated against the 2e-2 harness gate).
"""
import sys

sys.path.insert(0, "/opt/trn_rl_repo")

import numpy as np

import concourse.bacc as bacc
import concourse.bass as bass
import concourse.bass_isa as bass_isa
import concourse.mybir as mybir
import concourse.tile as tile

F32 = mybir.dt.float32
F16 = mybir.dt.float16
F32R = mybir.dt.float32r
I32 = mybir.dt.int32
AF = mybir.ActivationFunctionType
OP = mybir.AluOpType
RED = bass_isa.ReduceOp

B, T, C, IF = 8, 16, 1024, 471
N, W, R = 512, 64, 4
NT = N // 128  # 4 N-tiles
LOG2E = 1.4426950408889634
MAGIC2 = 12582912.0 + 127.0  # round-to-int magic + exponent bias for 2^k bits
_LN2 = 0.6931471805599453
# 2^f = 1 + sum_{i>=1} EXPC[i-1] f^i  (Taylor of exp(f ln2); deg-6 err ~2e-9)
EXPC = [_LN2, _LN2**2 / 2, _LN2**3 / 6, _LN2**4 / 24, _LN2**5 / 120,
        _LN2**6 / 720]

# iface field offsets
O_RK, O_RS, O_WK, O_WS = 0, 256, 260, 324
O_ER, O_WV, O_FG, O_AG, O_WG, O_MD = 325, 389, 453, 457, 458, 459


def build_nc():
    nc = bacc.Bacc("TRN2", target_bir_lowering=False, debug=False, num_devices=8)

    co_d = nc.declare_dram_parameter("co", [T, C], F32, isOutput=False)
    w_d = nc.declare_dram_parameter("wif", [C, IF], F32, isOutput=False)
    b_d = nc.declare_dram_parameter("bif", [1, IF], F32, isOutput=False)
    m0_d = nc.declare_dram_parameter("mem0", [N, W], F32, isOutput=False)
    ident_d = nc.declare_dram_parameter("ident", [128, 128], F32, isOutput=False)
    identh_d = nc.declare_dram_parameter("identh", [128, 128], F16, isOutput=False)
    ones_d = nc.declare_dram_parameter("ones", [128, 128], F32, isOutput=False)
    offd_d = nc.declare_dram_parameter("offdiag", [N, N], F16, isOutput=False)
    out_d = nc.declare_dram_parameter("out", [T, R, W], F32, isOutput=True)

    with tile.TileContext(nc) as tc:
        with (
            tc.tile_pool(name="const", bufs=1) as cp,
            tc.tile_pool(name="state", bufs=2) as sp,
            tc.tile_pool(name="work", bufs=2) as wp,
            tc.tile_pool(name="psP", bufs=2, space="PSUM") as psA,
            tc.tile_pool(name="psM", bufs=2, space="PSUM") as psM,
            tc.tile_pool(name="psS", bufs=1, space="PSUM") as psS,
        ):
            _build_body(nc, tc, cp, sp, wp, psA, psM, psS,
                        co_d, w_d, b_d, m0_d, ident_d, identh_d, ones_d,
                        offd_d, out_d)
    nc.compile()
    return nc


def _helpers(nc):
    V, S = nc.vector, nc.scalar

    def pexp(pool, x_ap, shape, tg, nb=1):
        """exp(x) to ~1e-7 via 2^(x*log2e): magic rounding + deg-6 poly +
        exponent-bit assembly. ~13 DVE ops; prologue/small-tensor use."""
        t_ = pool.tile(shape, F32, tag=tg + "_t", name=tg + "_t")
        V.tensor_scalar(t_[:], x_ap, LOG2E, None, op0=OP.mult)
        a_ = pool.tile(shape, F32, tag=tg + "_a", name=tg + "_a")
        V.tensor_scalar(a_[:], t_[:], MAGIC2, None, op0=OP.add)
        k_ = pool.tile(shape, F32, tag=tg + "_k", name=tg + "_k")
        V.tensor_scalar(k_[:], a_[:], MAGIC2, None, op0=OP.subtract)
        f_ = pool.tile(shape, F32, tag=tg + "_f", name=tg + "_f")
        V.tensor_tensor(f_[:], t_[:], k_[:], op=OP.subtract)
        p2 = pool.tile(shape, I32, tag=tg + "_p2", name=tg + "_p2")
        V.tensor_scalar(p2[:], a_[:].bitcast(I32), 23, None,
                        op0=OP.arith_shift_left)
        ac = [pool.tile(shape, F32, tag=tg + "_ac0", name=tg + "_ac0"),
              pool.tile(shape, F32, tag=tg + "_ac1", name=tg + "_ac1")]
        V.tensor_scalar(ac[0][:], f_[:], EXPC[5], None, op0=OP.mult)
        cur = 0
        for c_ in (EXPC[4], EXPC[3], EXPC[2], EXPC[1], EXPC[0]):
            V.scalar_tensor_tensor(ac[1 - cur][:], ac[cur][:], c_, f_[:],
                                   op0=OP.add, op1=OP.mult)
            cur = 1 - cur
        y_ = pool.tile(shape, F32, tag=tg + "_y", name=tg + "_y")
        V.scalar_tensor_tensor(y_[:], ac[cur][:], 1.0, p2[:].bitcast(F32),
                               op0=OP.add, op1=OP.mult)
        return y_

    def pexp2(pool, x_ap, shape, tg, accum_out=None, out=None, clamp=None):
        """2^x for prescaled x (log2 units); magic-round + deg-6 poly +
        exponent bits. All DVE; ~1e-7. x_ap should be SBUF."""
        if clamp is not None:
            t_ = pool.tile(shape, F32, tag=tg + "_t", name=tg + "_t", bufs=1)
            V.tensor_scalar(t_[:], x_ap, clamp, None, op0=OP.max)
            x_ap = t_[:]
        a_ = pool.tile(shape, F32, tag=tg + "_a", name=tg + "_a", bufs=1)
        V.tensor_scalar(a_[:], x_ap, MAGIC2, None, op0=OP.add)
        fn = pool.tile(shape, F32, tag=tg + "_fn", name=tg + "_fn", bufs=1)
        V.scalar_tensor_tensor(fn[:], a_[:], MAGIC2, x_ap,
                               op0=OP.subtract, op1=OP.subtract)  # -f
        p2 = pool.tile(shape, I32, tag=tg + "_p2", name=tg + "_p2", bufs=1)
        V.tensor_scalar(p2[:], a_[:].bitcast(I32), 23, None,
                        op0=OP.arith_shift_left)
        ac = [pool.tile(shape, F32, tag=tg + "_a0", name=tg + "_a0", bufs=1),
              pool.tile(shape, F32, tag=tg + "_a1", name=tg + "_a1", bufs=1)]
        V.tensor_scalar(ac[0][:], fn[:], EXPC[5], None, op0=OP.mult)
        cur = 0
        for i, c_ in ((5, EXPC[4]), (4, EXPC[3]), (3, EXPC[2]), (2, EXPC[1]),
                      (1, EXPC[0])):
            b_ = c_ if (i % 2 == 0) else -c_
            V.scalar_tensor_tensor(ac[1 - cur][:], ac[cur][:], b_, fn[:],
                                   op0=OP.add, op1=OP.mult)
            cur = 1 - cur
        if out is None:
            out_t = pool.tile(shape, F32, tag=tg + "_y", name=tg + "_y", bufs=1)
            out = out_t[:]
        V.scalar_tensor_tensor(out, ac[cur][:], 1.0, p2[:].bitcast(F32),
                               op0=OP.add, op1=OP.mult, accum_out=accum_out)
        return out

    _LNC = [0.9999751958009936, -0.49938365136996526, 0.3277847093008827,
            -0.22061263120600254, 0.1361341477101341, -0.06570959215415353,
            0.019091997353919793, -0.002558717382663216]

    def lnu_dve(pool, x_ap, shape, tg, out=None):
        """ln(x) for x in (0, 1]-ish via exponent bits + deg-7 poly on
        mantissa-1; ~13 DVE ops, max abs err ~5e-7. Subnormal x gives
        ln ~ -88 (harmless under the -87 clamp downstream)."""
        eb = pool.tile(shape, I32, tag=tg + "_eb", name=tg + "_eb", bufs=1)
        V.tensor_scalar(eb[:], x_ap.bitcast(I32), 23, None,
                        op0=OP.logical_shift_right)
        ef = pool.tile(shape, F32, tag=tg + "_ef", name=tg + "_ef", bufs=1)
        V.tensor_scalar(ef[:], eb[:], -127, None, op0=OP.add)
        mb = pool.tile(shape, I32, tag=tg + "_mb", name=tg + "_mb", bufs=1)
        V.tensor_scalar(mb[:], x_ap.bitcast(I32), 0x7FFFFF, None,
                        op0=OP.bitwise_and)
        m_ = pool.tile(shape, F32, tag=tg + "_m", name=tg + "_m", bufs=1)
        V.tensor_scalar(m_[:].bitcast(I32), mb[:], 0x3F800000, None,
                        op0=OP.bitwise_or)
        s_ = pool.tile(shape, F32, tag=tg + "_s", name=tg + "_s", bufs=1)
        V.tensor_scalar(s_[:], m_[:], -1.0, None, op0=OP.add)
        ac = [pool.tile(shape, F32, tag=tg + "_a0", name=tg + "_a0", bufs=1),
              pool.tile(shape, F32, tag=tg + "_a1", name=tg + "_a1", bufs=1)]
        V.tensor_scalar(ac[0][:], s_[:], _LNC[7], None, op0=OP.mult)
        cur = 0
        for c_ in (_LNC[6], _LNC[5], _LNC[4], _LNC[3], _LNC[2], _LNC[1],
                   _LNC[0]):
            V.scalar_tensor_tensor(ac[1 - cur][:], ac[cur][:], c_, s_[:],
                                   op0=OP.add, op1=OP.mult)
            cur = 1 - cur
        ln1p = pool.tile(shape, F32, tag=tg + "_l", name=tg + "_l", bufs=1)
        V.tensor_tensor(ln1p[:], ac[cur][:], s_[:], op=OP.mult)
        if out is None:
            o_ = pool.tile(shape, F32, tag=tg + "_o", name=tg + "_o", bufs=1)
            out = o_[:]
        V.scalar_tensor_tensor(out, ef[:], _LN2, ln1p[:],
                               op0=OP.mult, op1=OP.add)
        return out

    def rsqrt_sb(pool, x_sb, shape, tg, out=None, iters=3):
        """rsqrt via quake seed + Newton; all DVE/gpsimd, no ACT.
        x_sb must be an SBUF ap."""
        sh = pool.tile(shape, I32, tag=tg + "_sh", name=tg + "_sh", bufs=1)
        V.tensor_scalar(sh[:], x_sb.bitcast(I32), 1, None,
                        op0=OP.arith_shift_right)
        nb = pool.tile(shape, I32, tag=tg + "_nb", name=tg + "_nb", bufs=1)
        V.tensor_scalar(nb[:], sh[:], -1, None, op0=OP.bitwise_xor)
        y_ = pool.tile(shape, F32, tag=tg + "_y", name=tg + "_y", bufs=1)
        V.tensor_scalar(y_[:].bitcast(I32), nb[:], 0x5f3759e0, None, op0=OP.add)
        for i in range(iters):
            s_ = pool.tile(shape, F32, tag=tg + f"_s{i}", name=tg + f"_s{i}", bufs=1)
            V.tensor_tensor(s_[:], y_[:], y_[:], op=OP.mult)
            t_ = pool.tile(shape, F32, tag=tg + f"_t{i}", name=tg + f"_t{i}", bufs=1)
            V.tensor_tensor(t_[:], x_sb, s_[:], op=OP.mult)
            h_ = pool.tile(shape, F32, tag=tg + f"_h{i}", name=tg + f"_h{i}", bufs=1)
            V.tensor_scalar(h_[:], t_[:], -0.5, 1.5, op0=OP.mult, op1=OP.add)
            if i == iters - 1 and out is not None:
                V.tensor_tensor(out, y_[:], h_[:], op=OP.mult)
                return out
            y2 = pool.tile(shape, F32, tag=tg + f"_y{i}", name=tg + f"_y{i}", bufs=1)
            V.tensor_tensor(y2[:], y_[:], h_[:], op=OP.mult)
            y_ = y2
        return y_[:]

    def softplus_precise(pool, x_ap, shape, tg):
        """ln(1+e^x) with table-ln seed + one Newton step (via pexp)."""
        e_ = pexp(pool, x_ap, shape, tg + "e")
        w_ = pool.tile(shape, F32, tag=tg + "_w", name=tg + "_w")
        V.tensor_scalar(w_[:], e_[:], 1.0, None, op0=OP.add)
        z_ = pool.tile(shape, F32, tag=tg + "_z", name=tg + "_z")
        S.activation(z_[:], w_[:], AF.Ln)
        nz = pool.tile(shape, F32, tag=tg + "_nz", name=tg + "_nz")
        S.mul(nz[:], z_[:], -1.0)
        e2 = pexp(pool, nz[:], shape, tg + "e2")
        m_ = pool.tile(shape, F32, tag=tg + "_m", name=tg + "_m")
        V.tensor_tensor(m_[:], w_[:], e2[:], op=OP.mult)
        o_ = pool.tile(shape, F32, tag=tg + "_o", name=tg + "_o")
        V.scalar_tensor_tensor(o_[:], m_[:], -1.0, z_[:], op0=OP.add, op1=OP.add)
        return o_

    def rsqrt_refined(pool, x_ap, shape, tg, iters=1, nb=1):
        """rsqrt(x): ACT-table seed exp(-0.5 ln x) + Newton (no division)."""
        l_ = pool.tile(shape, F32, tag=tg + "_l", name=tg + "_l", bufs=nb)
        S.activation(l_[:], x_ap, AF.Ln)
        y_ = pool.tile(shape, F32, tag=tg + "_y", name=tg + "_y", bufs=nb)
        S.activation(y_[:], l_[:], AF.Exp, scale=-0.5)
        for i in range(iters):
            s_ = pool.tile(shape, F32, tag=tg + f"_s{i}", name=tg + f"_s{i}", bufs=nb)
            nc.gpsimd.tensor_tensor(s_[:], y_[:], y_[:], op=OP.mult)
            t_ = pool.tile(shape, F32, tag=tg + f"_t{i}", name=tg + f"_t{i}", bufs=nb)
            V.tensor_tensor(t_[:], x_ap, s_[:], op=OP.mult)
            h_ = pool.tile(shape, F32, tag=tg + f"_h{i}", name=tg + f"_h{i}", bufs=nb)
            V.tensor_scalar(h_[:], t_[:], -0.5, 1.5, op0=OP.mult, op1=OP.add)
            y2 = pool.tile(shape, F32, tag=tg + f"_y{i}", name=tg + f"_y{i}", bufs=nb)
            V.tensor_tensor(y2[:], y_[:], h_[:], op=OP.mult)
            y_ = y2
        return y_

    return (pexp, softplus_precise, rsqrt_refined, pexp2, rsqrt_sb,
            lnu_dve)


def _build_body(nc, tc, cp, sp, wp, psA, psM, psS,
                co_d, w_d, b_d, m0_d, ident_d, identh_d, ones_d, offd_d, out_d):
    V, S, P, DMA, G = nc.vector, nc.scalar, nc.tensor, nc.sync, nc.gpsimd
    (pexp, softplus_precise, rsqrt_refined, pexp2, rsqrt_sb,
     lnu_dve) = _helpers(nc)

    # ---------------- constants ----------------
    ident = cp.tile([128, 128], F32)
    DMA.dma_start(ident[:], ident_d[:])
    identh = cp.tile([128, 128], F16)
    DMA.dma_start(identh[:], identh_d[:])
    ones = cp.tile([128, 128], F32)
    DMA.dma_start(ones[:], ones_d[:])
    offd = cp.tile([128, NT, N], F16)
    for c in range(NT):
        DMA.dma_start(offd[:, c, :], offd_d[128 * c:128 * (c + 1), :])
    nege0_pm = cp.tile([128, NT], F32)
    G.memset(nege0_pm[:], 0.0)
    G.memset(nege0_pm[0:1, 0:1], -1.0)

    # persistent per-t parse tables
    rkT = cp.tile([W, R, T], F32)
    wkT = cp.tile([W, T], F32)
    neg_er = cp.tile([W, T], F32)
    wvT = cp.tile([W, T], F32)
    bkw = cp.tile([1, T], F32)       # softplus(ws)/||wk||
    wkT2 = cp.tile([W, T], F32R)     # wk * bkw (f32r for fast PE dots)
    rkT2 = cp.tile([W, R, T], F32R)  # rk * bkr (f32r)
    fgB = cp.tile([128, R, T], F32)  # sigmoid(fg) bcast to all partitions
    c1nB = cp.tile([128, T], F32)    # -ag*wg bcast
    c2B = cp.tile([128, T], F32)     # wg - ag*wg bcast
    modesB = cp.tile([128, 3, R, T], F32)  # softmax(modes) bcast

    # ---------------- prologue: iface (transient pool) ----------------
    with tc.tile_pool(name="prolog", bufs=1) as pp:
        co_sb = pp.tile([T, C], F32)
        DMA.dma_start(co_sb[:], co_d[:])
        bif_sb = pp.tile([1, IF], F32)
        DMA.dma_start(bif_sb[:], b_d[:])

        coT_p = psA.tile([128, 8, T], F32, tag="p")
        for k in range(8):
            P.transpose(coT_p[:, k, :], co_sb[:, 128 * k:128 * (k + 1)],
                        ident[0:T, 0:T])
        coT = pp.tile([128, 8, T], F32)
        S.copy(coT[:], coT_p[:])

        if_p = psS.tile([T, IF], F32, tag="s")
        for h in range(2):
            w_sb = pp.tile([128, 4, IF], F32, tag="w_sb", name=f"w_sb{h}")
            for k in range(4):
                DMA.dma_start(w_sb[:, k, :],
                              w_d[128 * (4 * h + k):128 * (4 * h + k + 1), :])
            for k in range(4):
                P.matmul(if_p[:], coT[:, 4 * h + k, :], w_sb[:, k, :],
                         start=(h == 0 and k == 0), stop=False)
        P.matmul(if_p[:], ones[0:1, 0:T], bif_sb[:], start=False, stop=True)
        iface = pp.tile([T, IF], F32)
        S.copy(iface[:], if_p[:])

        # field transposes -> per-t column layouts
        def tp_field(lo, hi):
            n = hi - lo
            pt = psA.tile([128, T], F32, tag="p")
            P.transpose(pt[0:n, :], iface[:, lo:hi], ident[0:T, 0:T])
            return pt

        for r in range(R):
            pt = tp_field(O_RK + W * r, O_RK + W * (r + 1))
            S.copy(rkT[:, r, :], pt[0:W, :])
        pt = tp_field(O_WK, O_WK + W)
        S.copy(wkT[:], pt[0:W, :])
        pt = tp_field(O_ER, O_ER + W)
        er_in = pp.tile([W, T], F32)
        V.tensor_scalar(er_in[:], pt[0:W, :], -1.0, None, op0=OP.mult)
        er_e = pexp(pp, er_in[:], [W, T], "sge")     # e^{-x}
        er_w = pp.tile([W, T], F32)
        V.tensor_scalar(er_w[:], er_e[:], 1.0, None, op0=OP.add)
        er_r = pp.tile([W, T], F32)
        V.reciprocal(er_r[:], er_w[:])               # sigmoid(x)
        V.tensor_scalar(neg_er[:], er_r[:], -1.0, None, op0=OP.mult)
        pt = tp_field(O_WV, O_WV + W)
        S.copy(wvT[:], pt[0:W, :])

        # strengths / ||k|| folded: bkw, bkr  (free-layout pipeline)
        rsF_p = psA.tile([1, R, T], F32, tag="p")
        for r in range(R):
            P.transpose(rsF_p[0:1, r, :], iface[:, O_RS + r:O_RS + r + 1],
                        ident[0:T, 0:T])
        rsF = pp.tile([1, R, T], F32)
        S.copy(rsF[:], rsF_p[:])
        wsF_p = psA.tile([1, T], F32, tag="p")
        P.transpose(wsF_p[:], iface[:, O_WS:O_WS + 1], ident[0:T, 0:T])
        wsF = pp.tile([1, T], F32)
        S.copy(wsF[:], wsF_p[:])
        rs_pre = softplus_precise(pp, rsF[:].rearrange("o r t -> o (r t)"),
                                  [1, R * T], "rsp")
        ws_pre = softplus_precise(pp, wsF[:], [1, T], "wsp")

        sqw = pp.tile([W, T], F32)
        S.square(sqw[:], wkT[:])
        wk2_p = psM.tile([1, T], F32, tag="m")
        P.matmul(wk2_p[:], ones[0:W, 0:1], sqw[:])
        wkr = rsqrt_refined(pp, wk2_p[:], [1, T], "wkr", iters=2)
        V.tensor_tensor(bkw[:], ws_pre[:], wkr[:], op=OP.mult)

        sqr = pp.tile([W, R, T], F32)
        S.square(sqr[:], rkT[:])
        rk2_p = psM.tile([1, R * T], F32, tag="m")
        P.matmul(rk2_p[:], ones[0:W, 0:1], sqr[:].rearrange("w r t -> w (r t)"))
        rkr = rsqrt_refined(pp, rk2_p[:], [1, R * T], "rkr", iters=2)
        bkrF = cp.tile([1, R, T], F32, name="bkrF")
        V.tensor_tensor(bkrF[:].rearrange("o r t -> o (r t)"), rs_pre[:],
                        rkr[:], op=OP.mult)
        # scaled keys (natural-log units): wkT2 = wkT*bkw ; rkT2 = rkT*bkr
        bkwb_p = psA.tile([W, T], F32, tag="p")
        P.matmul(bkwb_p[:], ones[0:1, 0:W], bkw[:])
        V.tensor_tensor(wkT2[:], wkT[:], bkwb_p[:], op=OP.mult)
        bkrb_p = psA.tile([W, R * T], F32, tag="p")
        P.matmul(bkrb_p[:], ones[0:1, 0:W], bkrF[:].rearrange("o r t -> o (r t)"))
        V.tensor_tensor(rkT2[:].rearrange("w r t -> w (r t)"),
                        rkT[:].rearrange("w r t -> w (r t)"),
                        bkrb_p[:], op=OP.mult)

        # gates: fg, ag, wg sigmoids via precise V pipeline, packed [1,6,T]
        gats_p = psA.tile([1, 6, T], F32, tag="p")
        for r in range(R):
            P.transpose(gats_p[0:1, r, :], iface[:, O_FG + r:O_FG + r + 1],
                        ident[0:T, 0:T])
        P.transpose(gats_p[0:1, 4, :], iface[:, O_AG:O_AG + 1], ident[0:T, 0:T])
        P.transpose(gats_p[0:1, 5, :], iface[:, O_WG:O_WG + 1], ident[0:T, 0:T])
        g_in = pp.tile([1, 6 * T], F32)
        V.tensor_scalar(g_in[:], gats_p[:].rearrange("o g t -> o (g t)"), -1.0,
                        None, op0=OP.mult)
        g_e = pexp(pp, g_in[:], [1, 6 * T], "sgg")
        g_w = pp.tile([1, 6 * T], F32)
        V.tensor_scalar(g_w[:], g_e[:], 1.0, None, op0=OP.add)
        g_r = pp.tile([1, 6, T], F32)
        V.reciprocal(g_r[:].rearrange("o g t -> o (g t)"), g_w[:])
        G.partition_broadcast(fgB[:].rearrange("p r t -> p (r t)"),
                              g_r[0:1, 0:R, :].rearrange("o r t -> o (r t)"))
        ag_t = g_r[0:1, 4, :]
        wg_t = g_r[0:1, 5, :]
        c1t = pp.tile([1, T], F32)
        V.tensor_tensor(c1t[:], ag_t, wg_t, op=OP.mult)
        c1n = pp.tile([1, T], F32)
        V.tensor_scalar(c1n[:], c1t[:], -1.0, None, op0=OP.mult)
        c2 = pp.tile([1, T], F32)
        V.tensor_tensor(c2[:], wg_t, c1t[:], op=OP.subtract)
        G.partition_broadcast(c1nB[:], c1n[:])
        G.partition_broadcast(c2B[:], c2[:])

        # modes softmax (precise exp; normalize in [T,12]; flatten; bcast)
        me = pexp(pp, iface[:, O_MD:O_MD + 12], [T, 12], "me")
        me3 = me[:].rearrange("t (r m) -> t r m", m=3)
        msum = pp.tile([T, R], F32)
        V.tensor_tensor(msum[:], me3[:, :, 0], me3[:, :, 1], op=OP.add)
        V.tensor_tensor(msum[:], msum[:], me3[:, :, 2], op=OP.add)
        mrs = pp.tile([T, R], F32)
        V.reciprocal(mrs[:], msum[:])
        mn = pp.tile([T, 12], F32)
        mn3 = mn[:].rearrange("t (r m) -> t r m", m=3)
        for m in range(3):
            V.tensor_tensor(mn3[:, :, m], me3[:, :, m], mrs[:], op=OP.mult)
        mo = pp.tile([T, 12], F32)
        mo3 = mo[:].rearrange("t (m r) -> t m r", r=R)
        S.copy(mo3[:], mn3[:].rearrange("t r m -> t m r"))
        # flatten [T,12] -> [1, (m r t)] via 12 single-column transposes
        mF_p = psA.tile([1, 12, T], F32, tag="p")
        for j in range(12):
            P.transpose(mF_p[0:1, j, :], mo[:, j:j + 1], ident[0:T, 0:T])
        mF = pp.tile([1, 12, T], F32)
        S.copy(mF[:], mF_p[:])
        G.partition_broadcast(modesB[:].rearrange("p m r t -> p (m r t)"),
                              mF[:].rearrange("o j t -> o (j t)"))

    # ---------------- initial state ----------------
    mem_nrm = sp.tile([128, NT, W], F32, tag="mem_nrm")
    for c in range(NT):
        DMA.dma_start(mem_nrm[:, c, :], m0_d[128 * c:128 * (c + 1), :])
    memT_p = psA.tile([W, N], F32, tag="p")
    for c in range(NT):
        P.transpose(memT_p[:, 128 * c:128 * (c + 1)], mem_nrm[:, c, :], ident[:])
    memT = sp.tile([W, N], F32R, tag="memT")
    S.copy(memT[:], memT_p[:])

    sq0 = wp.tile([128, NT, W], F32, tag="sq0", bufs=1)
    S.square(sq0[:], mem_nrm[:])
    msum0 = wp.tile([128, NT], F32, tag="msum")
    V.tensor_reduce(msum0[:], sq0[:], axis=mybir.AxisListType.X, op=OP.add)
    mnorm = sp.tile([128, NT], F32, tag="mnorm")
    rsqrt_sb(wp, msum0[:], [128, NT], "w1", out=mnorm[:], iters=3)

    L = sp.tile([128, NT, N], F16, tag="L")
    G.memset(L[:], 0.0)
    LT0 = sp.tile([128, NT, N], F16, tag="LT")
    G.memset(LT0[:], 0.0)
    u_pm0 = sp.tile([128, NT], F32, tag="u_pm")
    G.memset(u_pm0[:], 0.0)
    rwT0 = sp.tile([128, NT * R], F32, tag="rwT")
    G.memset(rwT0[:], 0.0)

    out_sb = cp.tile([R, T, W], F32)

    st = dict(memT=memT, mem_nrm=mem_nrm, mnorm=mnorm, L=L, LT=LT0,
              u_pm=u_pm0, prec_pm=None, prec_f=None, rwT=rwT0,
              ub=None, pb=None, pbm=None)

    for t in range(T):
        st = _step(nc, tc, t, st, cp, sp, wp, psA, psM, psS,
                   ident, identh, ones, offd, nege0_pm, wkT2, rkT2, neg_er,
                   wvT, fgB, c1nB, c2B, modesB, out_sb, pexp2, rsqrt_sb,
                   lnu_dve)

    DMA.dma_start(out_d[:].rearrange("t r w -> r t w"), out_sb[:])


def _step(nc, tc, t, st, cp, sp, wp, psA, psM, psS,
          ident, identh, ones, offd, nege0_pm, wkT2, rkT2, neg_er, wvT,
          fgB, c1nB, c2B, modesB, out_sb, pexp2, rsqrt_sb, lnu_dve):
    LOW = 50000  # deprioritize slack work for the Tile scheduler
    V, S, P, G = nc.vector, nc.scalar, nc.tensor, nc.gpsimd
    memT, mem_nrm, mnorm = st["memT"], st["mem_nrm"], st["mnorm"]
    L, LT, u_pm, rwT = st["L"], st["LT"], st["u_pm"], st["rwT"]
    prec_pm, prec_f = st["prec_pm"], st["prec_f"]
    ub, pb, pbm = st["ub"], st["pb"], st["pbm"]
    last = (t == T - 1)

    # ---- write content dots (PE; memT from prev step; slack path) ----
    hp = tc.high_priority
    wdf_p = psM.tile([1, N], F32, tag="m")
    P.matmul(wdf_p[:], wkT2[:, t:t + 1], memT[:])
    wdf = wp.tile([1, N], F32, tag="wdf")
    with hp(offset=-150):
        S.copy(wdf[:], wdf_p[:])
    wdots_p = psA.tile([128, NT], F32, tag="p")
    for c in range(NT):
        P.transpose(wdots_p[:, c:c + 1], wdf[0:1, 128 * c:128 * (c + 1)],
                    ident[0:1, 0:1])

    # ---- DVE head: psi chain + rw mode scales (deps: prev-step rwT) ----
    if 0 < t:
        yyT = wp.tile([128, NT, R], F32, tag="yyT")
        V.scalar_tensor_tensor(
            yyT[:], fgB[:, None, :, t].broadcast_to([128, NT, R]), -1.0,
            rwT[:].rearrange("p (c r) -> p c r", r=R), op0=OP.mult, op1=OP.mult)
        om = wp.tile([128, NT, R], F32, tag="om")
        V.tensor_scalar(om[:], yyT[:], 1.0, None, op0=OP.add)
        p1 = wp.tile([128, NT], F32, tag="p1")
        V.tensor_tensor(p1[:], om[:, :, 0], om[:, :, 1], op=OP.mult)
        p2 = wp.tile([128, NT], F32, tag="p2")
        V.tensor_tensor(p2[:], om[:, :, 2], om[:, :, 3], op=OP.mult)
        psi_pm = wp.tile([128, NT], F32, tag="psi_pm")
        V.tensor_tensor(psi_pm[:], p1[:], p2[:], op=OP.mult)
        rwm0 = wp.tile([128, NT * R], F16, tag="rwm0")
        V.tensor_tensor(rwm0[:].rearrange("p (c r) -> p c r", r=R),
                        rwT[:].rearrange("p (c r) -> p c r", r=R),
                        modesB[:, 0, None, :, t].broadcast_to([128, NT, R]),
                        op=OP.mult)
        rwm2 = wp.tile([128, NT * R], F16, tag="rwm2")
        V.tensor_tensor(rwm2[:].rearrange("p (c r) -> p c r", r=R),
                        rwT[:].rearrange("p (c r) -> p c r", r=R),
                        modesB[:, 2, None, :, t].broadcast_to([128, NT, R]),
                        op=OP.mult)

    # ---- allocation pipeline (PM layout; ub broadcast from prev step) ----
    wlog = wp.tile([128, NT], F32, tag="wlog")
    V.tensor_tensor(wlog[:], wdots_p[:], mnorm[:], op=OP.mult)
    wexp = wp.tile([128, NT], F32, tag="wexp")
    S.activation(wexp[:], wlog[:], AF.Exp)
    if t > 0:
        lu2 = wp.tile([128, NT], F16, tag="lu2")
        with hp():
            lnu_dve(wp, u_pm[:], [128, NT], "lnu", out=lu2[:])
        Gm = wp.tile([128, NT, N], F16, tag="Gm", bufs=1)
        for c in range(NT):
            V.tensor_scalar(Gm[:, c, :], ub[:], u_pm[:, c:c + 1], None,
                            op0=OP.is_gt)
        # s in PM layout directly: s_pm[p,c] = sum_cin Gm-block^T @ lu2-col
        s_pmP = psM.tile([128, NT], F32, tag="m")
        for c in range(NT):
            for cin in range(NT):
                P.matmul(s_pmP[:, c:c + 1],
                         Gm[:, cin, 128 * c:128 * (c + 1)],
                         lu2[:, cin:cin + 1],
                         start=(cin == 0), stop=(cin == NT - 1))
        s_cl = wp.tile([128, NT], F32, tag="s_cl")
        V.tensor_scalar(s_cl[:], s_pmP[:], -87.0, None, op0=OP.max)
        es = wp.tile([128, NT], F32, tag="es")
        S.activation(es[:], s_cl[:], AF.Exp)

    wpart = wp.tile([128, 1], F32, tag="wpart")
    V.tensor_reduce(wpart[:], wexp[:], axis=mybir.AxisListType.X, op=OP.add)
    wsumB = wp.tile([128, 1], F32, tag="wsumB")
    G.partition_all_reduce(wsumB[:], wpart[:], channels=128, reduce_op=RED.add)
    wrs = wp.tile([128, 1], F32, tag="wrs")
    V.reciprocal(wrs[:], wsumB[:])
    if t > 0:
        negalloc = wp.tile([128, NT], F32, tag="negalloc")
        V.scalar_tensor_tensor(negalloc[:], u_pm[:], 1.0, es[:],
                               op0=OP.subtract, op1=OP.mult)

    # ---- write weights ww (PM primary) ----
    t_wc = wp.tile([128, NT], F32, tag="t_wc")
    V.tensor_scalar(t_wc[:], wexp[:], wrs[:], c2B[:, t:t + 1],
                    op0=OP.mult, op1=OP.mult)
    ww_pm = wp.tile([128, NT], F32, tag="ww_pm")
    swp = wp.tile([128, 1], F32, tag="swp")
    na_ap = nege0_pm[:] if t == 0 else negalloc[:]
    V.scalar_tensor_tensor(ww_pm[:], na_ap, c1nB[:, t:t + 1], t_wc[:],
                           op0=OP.mult, op1=OP.add, accum_out=swp[:])
    wwf_p = psA.tile([1, N], F32, tag="p")
    ww = wp.tile([1, N], F32, tag="ww")
    wb = wp.tile([128, N], F32, tag="wb")
    with hp():
        for c in range(NT):
            P.transpose(wwf_p[0:1, 128 * c:128 * (c + 1)], ww_pm[:, c:c + 1],
                        ident[:])
        S.copy(ww[:], wwf_p[:])
        G.partition_broadcast(wb[:], ww[:])
    swB = wp.tile([128, 1], F32, tag="swB")
    G.partition_all_reduce(swB[:], swp[:], channels=128, reduce_op=RED.add)

    # ---- prec update in PM (uses prec BEFORE update) ----
    if t == 0:
        prec_pm_n = ww_pm
        prec_f_n = ww
    elif last:
        prec_pm_n = None
        prec_f_n = None
    else:
        omsw = wp.tile([128, 1], F32, tag="omsw")
        V.tensor_scalar(omsw[:], swB[:], -1.0, 1.0, op0=OP.mult, op1=OP.add)
        prec_pm_n = sp.tile([128, NT], F32, tag="prec_pm")
        V.scalar_tensor_tensor(prec_pm_n[:], prec_pm[:], omsw[:], ww_pm[:],
                               op0=OP.mult, op1=OP.add)
        pf_p = psA.tile([1, N], F32, tag="p")
        for c in range(NT):
            P.transpose(pf_p[0:1, 128 * c:128 * (c + 1)], prec_pm_n[:, c:c + 1],
                        ident[:])
        prec_f_n = sp.tile([1, N], F32, tag="prec_f")
        S.copy(prec_f_n[:], pf_p[:])

    # ---- usage update (PM layout) ----
    if t == 0:
        u_pm_n = ww_pm
        u_f_n = ww
    elif last:
        u_pm_n = None
        u_f_n = None
    else:
        omu_pm = wp.tile([128, NT], F32, tag="omu_pm")
        V.tensor_scalar(omu_pm[:], u_pm[:], -1.0, 1.0, op0=OP.mult, op1=OP.add)
        tpm = wp.tile([128, NT], F32, tag="tpm")
        V.scalar_tensor_tensor(tpm[:], ww_pm[:], 1.0, omu_pm[:],
                               op0=OP.subtract, op1=OP.mult)
        u_pm_n = sp.tile([128, NT], F32, tag="u_pm")
        V.scalar_tensor_tensor(u_pm_n[:], tpm[:], 1.0, psi_pm[:],
                               op0=OP.add, op1=OP.mult)
        uf_p = psA.tile([1, N], F32, tag="p")
        u_f_n = sp.tile([1, N], F32, tag="u_f")
        with hp():
            for c in range(NT):
                P.transpose(uf_p[0:1, 128 * c:128 * (c + 1)],
                            u_pm_n[:, c:c + 1], ident[:])
            S.copy(u_f_n[:], uf_p[:])

    # ---- memory update ----
    keep = wp.tile([W, N], F32, tag="keep", bufs=1)
    m1 = wp.tile([W, N], F32, tag="m1", bufs=1)
    memT_n = sp.tile([W, N], F32R, tag="memT")
    with hp():
        V.tensor_scalar(keep[:], wb[0:W, :], neg_er[:, t:t + 1], 1.0,
                        op0=OP.mult, op1=OP.add)
        G.tensor_tensor(m1[:], memT[:], keep[:], op=OP.mult)
        V.scalar_tensor_tensor(memT_n[:], wb[0:W, :], wvT[:, t:t + 1], m1[:],
                               op0=OP.mult, op1=OP.add)
    mem_nrm_p = psA.tile([128, NT, W], F32, tag="p")
    for c in range(NT):
        P.transpose(mem_nrm_p[:, c, :],
                    memT_n[:, 128 * c:128 * (c + 1)].bitcast(F32),
                    ident[0:W, 0:W])
    mem_nrm_n = sp.tile([128, NT, W], F32, tag="mem_nrm")
    with hp(offset=-150):
        S.copy(mem_nrm_n[:], mem_nrm_p[:])
    sqn = wp.tile([128, NT, W], F32, tag="sqn", bufs=1)
    with hp():
        S.square(sqn[:], mem_nrm_p[:])
    msum = wp.tile([128, NT], F32, tag="msum")
    V.tensor_reduce(msum[:], sqn[:], axis=mybir.AxisListType.X, op=OP.add)
    mnorm_n = sp.tile([128, NT], F32, tag="mnorm")
    rsqrt_sb(wp, msum[:], [128, NT], "w1", out=mnorm_n[:], iters=2)

    # ---- read content dots (PE; hoisted before link transposes) ----
    rdf_p = psM.tile([R, N], F32, tag="m")
    P.matmul(rdf_p[:], rkT2[:, :, t], memT_n[:])
    rdf = wp.tile([R, N], F32, tag="rdf")
    S.copy(rdf[:], rdf_p[:])
    rdots_p = psA.tile([128, NT, R], F32, tag="p")
    for c in range(NT):
        P.transpose(rdots_p[:, c, :], rdf[:, 128 * c:128 * (c + 1)],
                    ident[0:R, 0:R])

    # ---- link update: chunks 0-1 DVE 2-STT, chunks 2-3 DVE-TS + Pool-TT ----
    if t == 0:
        L_n, LT_n = L, LT  # stays zero
    else:
        omw_pm = wp.tile([128, NT], F32, tag="omw_pm")
        V.tensor_scalar(omw_pm[:], ww_pm[:], -1.0, 1.0, op0=OP.mult, op1=OP.add)
        L_n = sp.tile([128, NT, N], F16, tag="L")
        for c in range(NT):
            Dm = wp.tile([128, N], F16, tag=f"Dm_{c % 2}", name=f"Dm_{c}")
            V.tensor_scalar(Dm[:], wb[:], omw_pm[:, c:c + 1], -1.0,
                            op0=OP.subtract, op1=OP.mult)
            q_ = wp.tile([128, N], F16, tag=f"q_{c % 2}", name=f"q_{c}")
            V.tensor_scalar(q_[:], pbm[:, c, :], ww_pm[:, c:c + 1], None,
                            op0=OP.mult)
            t1 = wp.tile([128, N], F16, tag=f"t1h_{c % 2}", name=f"t1h_{c}")
            G.tensor_tensor(t1[:], Dm[:], L[:, c, :], op=OP.mult)
            G.tensor_tensor(L_n[:, c, :], q_[:], t1[:], op=OP.add)
        LT_n = sp.tile([128, NT, N], F16, tag="LT")
        for j in range(NT):
            lt_p = psA.tile([128, N], F16, tag="plt")
            for i in range(NT):
                P.transpose(lt_p[:, 128 * i:128 * (i + 1)],
                            L_n[:, i, 128 * j:128 * (j + 1)], identh[:])
            if j < 2:
                S.copy(LT_n[:, j, :], lt_p[:])
            else:
                V.tensor_copy(LT_n[:, j, :], lt_p[:])

    # ---- read softmax ----
    rlog = wp.tile([128, NT, R], F32, tag="rlog")
    V.tensor_tensor(rlog[:], rdots_p[:],
                    mnorm_n[:, :, None].broadcast_to([128, NT, R]), op=OP.mult)
    rexp = wp.tile([128, NT, R], F32, tag="rexp")
    S.activation(rexp[:], rlog[:], AF.Exp)
    rpart = wp.tile([128, R], F32, tag="rpart")
    V.tensor_reduce(rpart[:], rexp[:].rearrange("p c r -> p r c"),
                    axis=mybir.AxisListType.X, op=OP.add)
    rsumB = wp.tile([128, R], F32, tag="rsumB")
    G.partition_all_reduce(rsumB[:], rpart[:], channels=128, reduce_op=RED.add)
    rsr = wp.tile([128, R], F32, tag="rsr")
    V.reciprocal(rsr[:], rsumB[:])
    m1rs = wp.tile([128, R], F32, tag="m1rs")
    V.tensor_tensor(m1rs[:], rsr[:], modesB[:, 1, :, t], op=OP.mult)
    rexp_s = wp.tile([128, NT, R], F32, tag="rexp_s")
    V.tensor_tensor(rexp_s[:], rexp[:],
                    m1rs[:, None, :].broadcast_to([128, NT, R]), op=OP.mult)

    # ---- read weights: accumulate bwd + content + fwd in one PSUM bank ----
    rw_p = psS.tile([R, N], F32, tag="rw")
    if t > 0:
        for c in range(NT):
            P.matmul(rw_p[:], rwm0[:, R * c:R * (c + 1)], L_n[:, c, :],
                     start=(c == 0), stop=False)
        for c in range(NT):
            P.matmul(rw_p[:], rwm2[:, R * c:R * (c + 1)], LT_n[:, c, :],
                     start=False, stop=False)
        for c in range(NT):
            P.matmul(rw_p[:, 128 * c:128 * (c + 1)], rexp_s[:, c, :],
                     ident[:], is_transpose=True, start=False,
                     stop=(c == NT - 1))
    else:
        for c in range(NT):
            P.matmul(rw_p[:, 128 * c:128 * (c + 1)], rexp_s[:, c, :],
                     ident[:], is_transpose=True, start=(c == 0),
                     stop=(c == NT - 1))
    rw = wp.tile([R, N], F32, tag="rwf")
    rwT_p = psA.tile([128, NT * R], F32, tag="p")
    rwT_n = sp.tile([128, NT * R], F32, tag="rwT")
    with hp():
        S.copy(rw[:], rw_p[:])
        for c in range(NT):
            P.transpose(rwT_p[:, R * c:R * (c + 1)],
                        rw[:, 128 * c:128 * (c + 1)], ident[0:R, 0:R])
        V.tensor_copy(rwT_n[:], rwT_p[:])

    # ---- read words ----
    rwd_p = psS.tile([R, W], F32, tag="s")
    for c in range(NT):
        P.matmul(rwd_p[:], rwT_n[:, R * c:R * (c + 1)], mem_nrm_n[:, c, :],
                 start=(c == 0), stop=(c == NT - 1))
    with hp(offset=-150):
        S.copy(out_sb[:, t, :], rwd_p[:])

    # ---- tail: broadcasts for the NEXT step (hoisted into this step) ----
    ub_n = pb_n = pbm_n = None
    if not last:
        ub_n = wp.tile([128, N], F32, tag="ub")
        pb_n = wp.tile([128, N], F32, tag="pb")
        with hp():
            G.partition_broadcast(ub_n[:], u_f_n[:])
            G.partition_broadcast(pb_n[:], prec_f_n[:])
        pbm_n = wp.tile([128, NT, N], F16, tag="pbm")
        for c in range(NT):
            G.tensor_tensor(pbm_n[:, c, :], pb_n[:], offd[:, c, :], op=OP.mult)

    return dict(memT=memT_n, mem_nrm=mem_nrm_n, mnorm=mnorm_n, L=L_n, LT=LT_n,
                u_pm=u_pm_n, prec_pm=prec_pm_n, prec_f=prec_f_n, rwT=rwT_n,
                ub=ub_n, pb=pb_n, pbm=pbm_n)


# ---------------------------------------------------------------------------
_NC_CACHE = {}


def _get_nc():
    if "nc" not in _NC_CACHE:
        _NC_CACHE["nc"] = build_nc()
    return _NC_CACHE["nc"]


def _consts():
    ident = np.eye(128, dtype=np.float32)
    identh = np.eye(128, dtype=np.float16)
    ones = np.ones((128, 128), dtype=np.float32)
    offd = (1.0 - np.eye(N)).astype(np.float16)
    return ident, identh, ones, offd


def make_in_maps(controller_output, W_if, b_if, memory0):
    ident, identh, ones, offd = _consts()
    maps = []
    for b in range(B):
        maps.append({
            "co": np.ascontiguousarray(controller_output[b]),
            "wif": np.ascontiguousarray(W_if),
            "bif": np.ascontiguousarray(b_if.reshape(1, IF)),
            "mem0": np.ascontiguousarray(memory0[b]),
            "ident": ident, "identh": identh, "ones": ones, "offdiag": offd,
        })
    return maps


def kernel(controller_output, W_if, b_if, memory0):
    from concourse.bass_utils import run_bass_kernel_spmd
    controller_output = np.asarray(controller_output, dtype=np.float32)
    W_if = np.asarray(W_if, dtype=np.float32)
    b_if = np.asarray(b_if, dtype=np.float32)
    memory0 = np.asarray(memory0, dtype=np.float32)
    nc = _get_nc()
    maps = make_in_maps(controller_output, W_if, b_if, memory0)
    res = run_bass_kernel_spmd(nc, maps, core_ids=list(range(B)))
    return np.stack([res.results[b]["out"] for b in range(B)], axis=0)


if __name__ == "__main__":
    mode = sys.argv[1] if len(sys.argv) > 1 else "sim"
    sys.path.insert(0, "/root/problem")
    import jax
    with jax.default_device(jax.devices("cpu")[0]):
        import reference
        inputs = {k: np.asarray(v) for k, v in reference.setup_inputs().items()}
        expected = np.asarray(reference.reference(**inputs))

    if mode == "sim":
        from concourse.bass_interp import CoreSim
        nc = build_nc()
        maps = make_in_maps(inputs["controller_output"], inputs["W_if"],
                            inputs["b_if"], inputs["memory0"])
        sim = CoreSim(nc)
        for k, v in maps[0].items():
            sim.tensor(k)[:] = v
        sim.simulate()
        got = sim.tensor("out").copy()
        exp = expected[0]
        err = np.abs(got - exp)
        rel = np.linalg.norm(got - exp) / (np.linalg.norm(exp) + 1e-12)
        print("sim modeled time (ns):", sim.time)
        print("max abs err:", err.max(), " rel err:", rel)
    else:
        got = kernel(**inputs)
        rel = np.linalg.norm(got - expected) / (np.linalg.norm(expected) + 1e-12)
        print("max abs err:", np.abs(got - expected).max(), " rel err:", rel)


# revision 35
# speedup vs baseline: 2.4616x; 1.0540x over previous
"""DNC MemoryAccess kernel for Trainium2 (Bass/Tile), data-parallel over batch.

Shapes (hardcoded): B=8, T=16, C=1024, IFACE=471, N=512, WORD=64, R=4, NW=1.
Each of the 8 cores processes one batch element; all recurrent state
(memory [64,512] f32r + [128,4,64], link/linkT [128,4,512] f16, usage/prec
in partition-major [128,4], read_weights) stays SBUF-resident across the
T=16 sequential steps.

Engine balance (vs the DVE-bound ancestor at 529us): broadcasts run on
GpSimd partition_broadcast, softmax sums on gpsimd partition_all_reduce,
PSUM->SBUF copies on ACT (single act-table set = no per-step reloads),
the link decay/write terms split DVE tensor_scalar + Pool tensor_tensor
(scalar-ptr ops are not HW-legal on Pool), linkT via PE transposes with
ACT/DVE copies, content dots as float32r matmuls (1 cyc/row), the
allocation log-sum as 16 block matmuls emitting partition-major [128,4]
directly, and the read-weight content term merged in PM layout after the
bwd/fwd PSUM accumulation so the rw copy never waits on the softmax path.

Precision notes: the reference allocation sort is exactly reproducible
only in full fp32 (near-ties at ~1e-9); this kernel trades that for speed
under the 2e-2 harness gate: f16 link state, f16 allocation-mask matmul,
f32r content dots, ACT-table exp for the three per-step softmaxes, a
deg-7 bitwise-ln on DVE (usage log), and quake-seeded Newton rsqrt for
the cosine norms. Measured: two batches flip one allocation slot each
(~7e-3 rel err there, ~7e-4 elsewhere; 3.7e-3 overall vs the 2e-2 gate).
"""
import sys

sys.path.insert(0, "/opt/trn_rl_repo")

import numpy as np

import concourse.bacc as bacc
import concourse.bass as bass
import concourse.bass_isa as bass_isa
import concourse.mybir as mybir
import concourse.tile as tile

F32 = mybir.dt.float32
F16 = mybir.dt.float16
F32R = mybir.dt.float32r
I32 = mybir.dt.int32
AF = mybir.ActivationFunctionType
OP = mybir.AluOpType
RED = bass_isa.ReduceOp

B, T, C, IF = 8, 16, 1024, 471
N, W, R = 512, 64, 4
NT = N // 128  # 4 N-tiles
LOG2E = 1.4426950408889634
MAGIC2 = 12582912.0 + 127.0  # round-to-int magic + exponent bias for 2^k bits
_LN2 = 0.6931471805599453
# 2^f = 1 + sum_{i>=1} EXPC[i-1] f^i  (Taylor of exp(f ln2); deg-6 err ~2e-9)
EXPC = [_LN2, _LN2**2 / 2, _LN2**3 / 6, _LN2**4 / 24, _LN2**5 / 120,
        _LN2**6 / 720]

# iface field offsets
O_RK, O_RS, O_WK, O_WS = 0, 256, 260, 324
O_ER, O_WV, O_FG, O_AG, O_WG, O_MD = 325, 389, 453, 457, 458, 459


def build_nc():
    nc = bacc.Bacc("TRN2", target_bir_lowering=False, debug=False, num_devices=8)

    co_d = nc.declare_dram_parameter("co", [T, C], F32, isOutput=False)
    w_d = nc.declare_dram_parameter("wif", [C, IF], F32, isOutput=False)
    b_d = nc.declare_dram_parameter("bif", [1, IF], F32, isOutput=False)
    m0_d = nc.declare_dram_parameter("mem0", [N, W], F32, isOutput=False)
    ident_d = nc.declare_dram_parameter("ident", [128, 128], F32, isOutput=False)
    identh_d = nc.declare_dram_parameter("identh", [128, 128], F16, isOutput=False)
    ones_d = nc.declare_dram_parameter("ones", [128, 128], F32, isOutput=False)
    offd_d = nc.declare_dram_parameter("offdiag", [N, N], F16, isOutput=False)
    out_d = nc.declare_dram_parameter("out", [T, R, W], F32, isOutput=True)

    with tile.TileContext(nc) as tc:
        with (
            tc.tile_pool(name="const", bufs=1) as cp,
            tc.tile_pool(name="state", bufs=2) as sp,
            tc.tile_pool(name="work", bufs=2) as wp,
            tc.tile_pool(name="psP", bufs=2, space="PSUM") as psA,
            tc.tile_pool(name="psM", bufs=2, space="PSUM") as psM,
            tc.tile_pool(name="psS", bufs=1, space="PSUM") as psS,
        ):
            _build_body(nc, tc, cp, sp, wp, psA, psM, psS,
                        co_d, w_d, b_d, m0_d, ident_d, identh_d, ones_d,
                        offd_d, out_d)
    nc.compile()
    return nc


def _helpers(nc):
    V, S = nc.vector, nc.scalar

    def pexp(pool, x_ap, shape, tg, nb=1):
        """exp(x) to ~1e-7 via 2^(x*log2e): magic rounding + deg-6 poly +
        exponent-bit assembly. ~13 DVE ops; prologue/small-tensor use."""
        t_ = pool.tile(shape, F32, tag=tg + "_t", name=tg + "_t")
        V.tensor_scalar(t_[:], x_ap, LOG2E, None, op0=OP.mult)
        a_ = pool.tile(shape, F32, tag=tg + "_a", name=tg + "_a")
        V.tensor_scalar(a_[:], t_[:], MAGIC2, None, op0=OP.add)
        k_ = pool.tile(shape, F32, tag=tg + "_k", name=tg + "_k")
        V.tensor_scalar(k_[:], a_[:], MAGIC2, None, op0=OP.subtract)
        f_ = pool.tile(shape, F32, tag=tg + "_f", name=tg + "_f")
        V.tensor_tensor(f_[:], t_[:], k_[:], op=OP.subtract)
        p2 = pool.tile(shape, I32, tag=tg + "_p2", name=tg + "_p2")
        V.tensor_scalar(p2[:], a_[:].bitcast(I32), 23, None,
                        op0=OP.arith_shift_left)
        ac = [pool.tile(shape, F32, tag=tg + "_ac0", name=tg + "_ac0"),
              pool.tile(shape, F32, tag=tg + "_ac1", name=tg + "_ac1")]
        V.tensor_scalar(ac[0][:], f_[:], EXPC[5], None, op0=OP.mult)
        cur = 0
        for c_ in (EXPC[4], EXPC[3], EXPC[2], EXPC[1], EXPC[0]):
            V.scalar_tensor_tensor(ac[1 - cur][:], ac[cur][:], c_, f_[:],
                                   op0=OP.add, op1=OP.mult)
            cur = 1 - cur
        y_ = pool.tile(shape, F32, tag=tg + "_y", name=tg + "_y")
        V.scalar_tensor_tensor(y_[:], ac[cur][:], 1.0, p2[:].bitcast(F32),
                               op0=OP.add, op1=OP.mult)
        return y_

    def pexp2(pool, x_ap, shape, tg, accum_out=None, out=None, clamp=None):
        """2^x for prescaled x (log2 units); magic-round + deg-6 poly +
        exponent bits. All DVE; ~1e-7. x_ap should be SBUF."""
        if clamp is not None:
            t_ = pool.tile(shape, F32, tag=tg + "_t", name=tg + "_t", bufs=1)
            V.tensor_scalar(t_[:], x_ap, clamp, None, op0=OP.max)
            x_ap = t_[:]
        a_ = pool.tile(shape, F32, tag=tg + "_a", name=tg + "_a", bufs=1)
        V.tensor_scalar(a_[:], x_ap, MAGIC2, None, op0=OP.add)
        fn = pool.tile(shape, F32, tag=tg + "_fn", name=tg + "_fn", bufs=1)
        V.scalar_tensor_tensor(fn[:], a_[:], MAGIC2, x_ap,
                               op0=OP.subtract, op1=OP.subtract)  # -f
        p2 = pool.tile(shape, I32, tag=tg + "_p2", name=tg + "_p2", bufs=1)
        V.tensor_scalar(p2[:], a_[:].bitcast(I32), 23, None,
                        op0=OP.arith_shift_left)
        ac = [pool.tile(shape, F32, tag=tg + "_a0", name=tg + "_a0", bufs=1),
              pool.tile(shape, F32, tag=tg + "_a1", name=tg + "_a1", bufs=1)]
        V.tensor_scalar(ac[0][:], fn[:], EXPC[5], None, op0=OP.mult)
        cur = 0
        for i, c_ in ((5, EXPC[4]), (4, EXPC[3]), (3, EXPC[2]), (2, EXPC[1]),
                      (1, EXPC[0])):
            b_ = c_ if (i % 2 == 0) else -c_
            V.scalar_tensor_tensor(ac[1 - cur][:], ac[cur][:], b_, fn[:],
                                   op0=OP.add, op1=OP.mult)
            cur = 1 - cur
        if out is None:
            out_t = pool.tile(shape, F32, tag=tg + "_y", name=tg + "_y", bufs=1)
            out = out_t[:]
        V.scalar_tensor_tensor(out, ac[cur][:], 1.0, p2[:].bitcast(F32),
                               op0=OP.add, op1=OP.mult, accum_out=accum_out)
        return out

    _LNC = [0.9999751958009936, -0.49938365136996526, 0.3277847093008827,
            -0.22061263120600254, 0.1361341477101341, -0.06570959215415353,
            0.019091997353919793, -0.002558717382663216]

    def lnu_dve(pool, x_ap, shape, tg, out=None):
        """ln(x) for x in (0, 1]-ish via exponent bits + deg-7 poly on
        mantissa-1; ~13 DVE ops, max abs err ~5e-7. Subnormal x gives
        ln ~ -88 (harmless under the -87 clamp downstream)."""
        eb = pool.tile(shape, I32, tag=tg + "_eb", name=tg + "_eb", bufs=1)
        V.tensor_scalar(eb[:], x_ap.bitcast(I32), 23, None,
                        op0=OP.logical_shift_right)
        ef = pool.tile(shape, F32, tag=tg + "_ef", name=tg + "_ef", bufs=1)
        V.tensor_scalar(ef[:], eb[:], -127, None, op0=OP.add)
        mb = pool.tile(shape, I32, tag=tg + "_mb", name=tg + "_mb", bufs=1)
        V.tensor_scalar(mb[:], x_ap.bitcast(I32), 0x7FFFFF, None,
                        op0=OP.bitwise_and)
        m_ = pool.tile(shape, F32, tag=tg + "_m", name=tg + "_m", bufs=1)
        V.tensor_scalar(m_[:].bitcast(I32), mb[:], 0x3F800000, None,
                        op0=OP.bitwise_or)
        s_ = pool.tile(shape, F32, tag=tg + "_s", name=tg + "_s", bufs=1)
        V.tensor_scalar(s_[:], m_[:], -1.0, None, op0=OP.add)
        ac = [pool.tile(shape, F32, tag=tg + "_a0", name=tg + "_a0", bufs=1),
              pool.tile(shape, F32, tag=tg + "_a1", name=tg + "_a1", bufs=1)]
        V.tensor_scalar(ac[0][:], s_[:], _LNC[7], None, op0=OP.mult)
        cur = 0
        for c_ in (_LNC[6], _LNC[5], _LNC[4], _LNC[3], _LNC[2], _LNC[1],
                   _LNC[0]):
            V.scalar_tensor_tensor(ac[1 - cur][:], ac[cur][:], c_, s_[:],
                                   op0=OP.add, op1=OP.mult)
            cur = 1 - cur
        ln1p = pool.tile(shape, F32, tag=tg + "_l", name=tg + "_l", bufs=1)
        V.tensor_tensor(ln1p[:], ac[cur][:], s_[:], op=OP.mult)
        if out is None:
            o_ = pool.tile(shape, F32, tag=tg + "_o", name=tg + "_o", bufs=1)
            out = o_[:]
        V.scalar_tensor_tensor(out, ef[:], _LN2, ln1p[:],
                               op0=OP.mult, op1=OP.add)
        return out

    def rsqrt_sb(pool, x_sb, shape, tg, out=None, iters=3):
        """rsqrt via quake seed + Newton; all DVE/gpsimd, no ACT.
        x_sb must be an SBUF ap."""
        sh = pool.tile(shape, I32, tag=tg + "_sh", name=tg + "_sh", bufs=1)
        V.tensor_scalar(sh[:], x_sb.bitcast(I32), 1, None,
                        op0=OP.arith_shift_right)
        nb = pool.tile(shape, I32, tag=tg + "_nb", name=tg + "_nb", bufs=1)
        V.tensor_scalar(nb[:], sh[:], -1, None, op0=OP.bitwise_xor)
        y_ = pool.tile(shape, F32, tag=tg + "_y", name=tg + "_y", bufs=1)
        V.tensor_scalar(y_[:].bitcast(I32), nb[:], 0x5f3759e0, None, op0=OP.add)
        for i in range(iters):
            s_ = pool.tile(shape, F32, tag=tg + f"_s{i}", name=tg + f"_s{i}", bufs=1)
            V.tensor_tensor(s_[:], y_[:], y_[:], op=OP.mult)
            t_ = pool.tile(shape, F32, tag=tg + f"_t{i}", name=tg + f"_t{i}", bufs=1)
            V.tensor_tensor(t_[:], x_sb, s_[:], op=OP.mult)
            h_ = pool.tile(shape, F32, tag=tg + f"_h{i}", name=tg + f"_h{i}", bufs=1)
            V.tensor_scalar(h_[:], t_[:], -0.5, 1.5, op0=OP.mult, op1=OP.add)
            if i == iters - 1 and out is not None:
                V.tensor_tensor(out, y_[:], h_[:], op=OP.mult)
                return out
            y2 = pool.tile(shape, F32, tag=tg + f"_y{i}", name=tg + f"_y{i}", bufs=1)
            V.tensor_tensor(y2[:], y_[:], h_[:], op=OP.mult)
            y_ = y2
        return y_[:]

    def softplus_precise(pool, x_ap, shape, tg):
        """ln(1+e^x) with table-ln seed + one Newton step (via pexp)."""
        e_ = pexp(pool, x_ap, shape, tg + "e")
        w_ = pool.tile(shape, F32, tag=tg + "_w", name=tg + "_w")
        V.tensor_scalar(w_[:], e_[:], 1.0, None, op0=OP.add)
        z_ = pool.tile(shape, F32, tag=tg + "_z", name=tg + "_z")
        S.activation(z_[:], w_[:], AF.Ln)
        nz = pool.tile(shape, F32, tag=tg + "_nz", name=tg + "_nz")
        S.mul(nz[:], z_[:], -1.0)
        e2 = pexp(pool, nz[:], shape, tg + "e2")
        m_ = pool.tile(shape, F32, tag=tg + "_m", name=tg + "_m")
        V.tensor_tensor(m_[:], w_[:], e2[:], op=OP.mult)
        o_ = pool.tile(shape, F32, tag=tg + "_o", name=tg + "_o")
        V.scalar_tensor_tensor(o_[:], m_[:], -1.0, z_[:], op0=OP.add, op1=OP.add)
        return o_

    def rsqrt_refined(pool, x_ap, shape, tg, iters=1, nb=1):
        """rsqrt(x): ACT-table seed exp(-0.5 ln x) + Newton (no division)."""
        l_ = pool.tile(shape, F32, tag=tg + "_l", name=tg + "_l", bufs=nb)
        S.activation(l_[:], x_ap, AF.Ln)
        y_ = pool.tile(shape, F32, tag=tg + "_y", name=tg + "_y", bufs=nb)
        S.activation(y_[:], l_[:], AF.Exp, scale=-0.5)
        for i in range(iters):
            s_ = pool.tile(shape, F32, tag=tg + f"_s{i}", name=tg + f"_s{i}", bufs=nb)
            nc.gpsimd.tensor_tensor(s_[:], y_[:], y_[:], op=OP.mult)
            t_ = pool.tile(shape, F32, tag=tg + f"_t{i}", name=tg + f"_t{i}", bufs=nb)
            V.tensor_tensor(t_[:], x_ap, s_[:], op=OP.mult)
            h_ = pool.tile(shape, F32, tag=tg + f"_h{i}", name=tg + f"_h{i}", bufs=nb)
            V.tensor_scalar(h_[:], t_[:], -0.5, 1.5, op0=OP.mult, op1=OP.add)
            y2 = pool.tile(shape, F32, tag=tg + f"_y{i}", name=tg + f"_y{i}", bufs=nb)
            V.tensor_tensor(y2[:], y_[:], h_[:], op=OP.mult)
            y_ = y2
        return y_

    return (pexp, softplus_precise, rsqrt_refined, pexp2, rsqrt_sb,
            lnu_dve)


def _build_body(nc, tc, cp, sp, wp, psA, psM, psS,
                co_d, w_d, b_d, m0_d, ident_d, identh_d, ones_d, offd_d, out_d):
    V, S, P, DMA, G = nc.vector, nc.scalar, nc.tensor, nc.sync, nc.gpsimd
    (pexp, softplus_precise, rsqrt_refined, pexp2, rsqrt_sb,
     lnu_dve) = _helpers(nc)

    # ---------------- constants ----------------
    ident = cp.tile([128, 128], F32)
    DMA.dma_start(ident[:], ident_d[:])
    identh = cp.tile([128, 128], F16)
    DMA.dma_start(identh[:], identh_d[:])
    ones = cp.tile([128, 128], F32)
    DMA.dma_start(ones[:], ones_d[:])
    offd = cp.tile([128, NT, N], F16)
    for c in range(NT):
        DMA.dma_start(offd[:, c, :], offd_d[128 * c:128 * (c + 1), :])
    nege0_pm = cp.tile([128, NT], F32)
    G.memset(nege0_pm[:], 0.0)
    G.memset(nege0_pm[0:1, 0:1], -1.0)

    # persistent per-t parse tables
    rkT = cp.tile([W, R, T], F32)
    wkT = cp.tile([W, T], F32)
    neg_er = cp.tile([W, T], F32)
    wvT = cp.tile([W, T], F32)
    bkw = cp.tile([1, T], F32)       # softplus(ws)/||wk||
    wkT2 = cp.tile([W, T], F32R)     # wk * bkw (f32r for fast PE dots)
    rkT2 = cp.tile([W, R, T], F32R)  # rk * bkr (f32r)
    fgB = cp.tile([128, R, T], F32)  # sigmoid(fg) bcast to all partitions
    c1nB = cp.tile([128, T], F32)    # -ag*wg bcast
    c2B = cp.tile([128, T], F32)     # wg - ag*wg bcast
    modesB = cp.tile([128, 3, R, T], F32)  # softmax(modes) bcast

    # ---------------- prologue: iface (transient pool) ----------------
    with tc.tile_pool(name="prolog", bufs=1) as pp:
        co_sb = pp.tile([T, C], F32)
        DMA.dma_start(co_sb[:], co_d[:])
        bif_sb = pp.tile([1, IF], F32)
        DMA.dma_start(bif_sb[:], b_d[:])

        coT_p = psA.tile([128, 8, T], F32, tag="p")
        for k in range(8):
            P.transpose(coT_p[:, k, :], co_sb[:, 128 * k:128 * (k + 1)],
                        ident[0:T, 0:T])
        coT = pp.tile([128, 8, T], F32)
        S.copy(coT[:], coT_p[:])

        if_p = psS.tile([T, IF], F32, tag="s")
        for h in range(2):
            w_sb = pp.tile([128, 4, IF], F32, tag="w_sb", name=f"w_sb{h}")
            for k in range(4):
                DMA.dma_start(w_sb[:, k, :],
                              w_d[128 * (4 * h + k):128 * (4 * h + k + 1), :])
            for k in range(4):
                P.matmul(if_p[:], coT[:, 4 * h + k, :], w_sb[:, k, :],
                         start=(h == 0 and k == 0), stop=False)
        P.matmul(if_p[:], ones[0:1, 0:T], bif_sb[:], start=False, stop=True)
        iface = pp.tile([T, IF], F32)
        S.copy(iface[:], if_p[:])

        # field transposes -> per-t column layouts
        def tp_field(lo, hi):
            n = hi - lo
            pt = psA.tile([128, T], F32, tag="p")
            P.transpose(pt[0:n, :], iface[:, lo:hi], ident[0:T, 0:T])
            return pt

        for r in range(R):
            pt = tp_field(O_RK + W * r, O_RK + W * (r + 1))
            S.copy(rkT[:, r, :], pt[0:W, :])
        pt = tp_field(O_WK, O_WK + W)
        S.copy(wkT[:], pt[0:W, :])
        pt = tp_field(O_ER, O_ER + W)
        er_in = pp.tile([W, T], F32)
        V.tensor_scalar(er_in[:], pt[0:W, :], -1.0, None, op0=OP.mult)
        er_e = pexp(pp, er_in[:], [W, T], "sge")     # e^{-x}
        er_w = pp.tile([W, T], F32)
        V.tensor_scalar(er_w[:], er_e[:], 1.0, None, op0=OP.add)
        er_r = pp.tile([W, T], F32)
        V.reciprocal(er_r[:], er_w[:])               # sigmoid(x)
        V.tensor_scalar(neg_er[:], er_r[:], -1.0, None, op0=OP.mult)
        pt = tp_field(O_WV, O_WV + W)
        S.copy(wvT[:], pt[0:W, :])

        # strengths / ||k|| folded: bkw, bkr  (free-layout pipeline)
        rsF_p = psA.tile([1, R, T], F32, tag="p")
        for r in range(R):
            P.transpose(rsF_p[0:1, r, :], iface[:, O_RS + r:O_RS + r + 1],
                        ident[0:T, 0:T])
        rsF = pp.tile([1, R, T], F32)
        S.copy(rsF[:], rsF_p[:])
        wsF_p = psA.tile([1, T], F32, tag="p")
        P.transpose(wsF_p[:], iface[:, O_WS:O_WS + 1], ident[0:T, 0:T])
        wsF = pp.tile([1, T], F32)
        S.copy(wsF[:], wsF_p[:])
        rs_pre = softplus_precise(pp, rsF[:].rearrange("o r t -> o (r t)"),
                                  [1, R * T], "rsp")
        ws_pre = softplus_precise(pp, wsF[:], [1, T], "wsp")

        sqw = pp.tile([W, T], F32)
        S.square(sqw[:], wkT[:])
        wk2_p = psM.tile([1, T], F32, tag="m")
        P.matmul(wk2_p[:], ones[0:W, 0:1], sqw[:])
        wkr = rsqrt_refined(pp, wk2_p[:], [1, T], "wkr", iters=2)
        V.tensor_tensor(bkw[:], ws_pre[:], wkr[:], op=OP.mult)

        sqr = pp.tile([W, R, T], F32)
        S.square(sqr[:], rkT[:])
        rk2_p = psM.tile([1, R * T], F32, tag="m")
        P.matmul(rk2_p[:], ones[0:W, 0:1], sqr[:].rearrange("w r t -> w (r t)"))
        rkr = rsqrt_refined(pp, rk2_p[:], [1, R * T], "rkr", iters=2)
        bkrF = cp.tile([1, R, T], F32, name="bkrF")
        V.tensor_tensor(bkrF[:].rearrange("o r t -> o (r t)"), rs_pre[:],
                        rkr[:], op=OP.mult)
        # scaled keys (natural-log units): wkT2 = wkT*bkw ; rkT2 = rkT*bkr
        bkwb_p = psA.tile([W, T], F32, tag="p")
        P.matmul(bkwb_p[:], ones[0:1, 0:W], bkw[:])
        V.tensor_tensor(wkT2[:], wkT[:], bkwb_p[:], op=OP.mult)
        bkrb_p = psA.tile([W, R * T], F32, tag="p")
        P.matmul(bkrb_p[:], ones[0:1, 0:W], bkrF[:].rearrange("o r t -> o (r t)"))
        V.tensor_tensor(rkT2[:].rearrange("w r t -> w (r t)"),
                        rkT[:].rearrange("w r t -> w (r t)"),
                        bkrb_p[:], op=OP.mult)

        # gates: fg, ag, wg sigmoids via precise V pipeline, packed [1,6,T]
        gats_p = psA.tile([1, 6, T], F32, tag="p")
        for r in range(R):
            P.transpose(gats_p[0:1, r, :], iface[:, O_FG + r:O_FG + r + 1],
                        ident[0:T, 0:T])
        P.transpose(gats_p[0:1, 4, :], iface[:, O_AG:O_AG + 1], ident[0:T, 0:T])
        P.transpose(gats_p[0:1, 5, :], iface[:, O_WG:O_WG + 1], ident[0:T, 0:T])
        g_in = pp.tile([1, 6 * T], F32)
        V.tensor_scalar(g_in[:], gats_p[:].rearrange("o g t -> o (g t)"), -1.0,
                        None, op0=OP.mult)
        g_e = pexp(pp, g_in[:], [1, 6 * T], "sgg")
        g_w = pp.tile([1, 6 * T], F32)
        V.tensor_scalar(g_w[:], g_e[:], 1.0, None, op0=OP.add)
        g_r = pp.tile([1, 6, T], F32)
        V.reciprocal(g_r[:].rearrange("o g t -> o (g t)"), g_w[:])
        G.partition_broadcast(fgB[:].rearrange("p r t -> p (r t)"),
                              g_r[0:1, 0:R, :].rearrange("o r t -> o (r t)"))
        ag_t = g_r[0:1, 4, :]
        wg_t = g_r[0:1, 5, :]
        c1t = pp.tile([1, T], F32)
        V.tensor_tensor(c1t[:], ag_t, wg_t, op=OP.mult)
        c1n = pp.tile([1, T], F32)
        V.tensor_scalar(c1n[:], c1t[:], -1.0, None, op0=OP.mult)
        c2 = pp.tile([1, T], F32)
        V.tensor_tensor(c2[:], wg_t, c1t[:], op=OP.subtract)
        G.partition_broadcast(c1nB[:], c1n[:])
        G.partition_broadcast(c2B[:], c2[:])

        # modes softmax (precise exp; normalize in [T,12]; flatten; bcast)
        me = pexp(pp, iface[:, O_MD:O_MD + 12], [T, 12], "me")
        me3 = me[:].rearrange("t (r m) -> t r m", m=3)
        msum = pp.tile([T, R], F32)
        V.tensor_tensor(msum[:], me3[:, :, 0], me3[:, :, 1], op=OP.add)
        V.tensor_tensor(msum[:], msum[:], me3[:, :, 2], op=OP.add)
        mrs = pp.tile([T, R], F32)
        V.reciprocal(mrs[:], msum[:])
        mn = pp.tile([T, 12], F32)
        mn3 = mn[:].rearrange("t (r m) -> t r m", m=3)
        for m in range(3):
            V.tensor_tensor(mn3[:, :, m], me3[:, :, m], mrs[:], op=OP.mult)
        mo = pp.tile([T, 12], F32)
        mo3 = mo[:].rearrange("t (m r) -> t m r", r=R)
        S.copy(mo3[:], mn3[:].rearrange("t r m -> t m r"))
        # flatten [T,12] -> [1, (m r t)] via 12 single-column transposes
        mF_p = psA.tile([1, 12, T], F32, tag="p")
        for j in range(12):
            P.transpose(mF_p[0:1, j, :], mo[:, j:j + 1], ident[0:T, 0:T])
        mF = pp.tile([1, 12, T], F32)
        S.copy(mF[:], mF_p[:])
        G.partition_broadcast(modesB[:].rearrange("p m r t -> p (m r t)"),
                              mF[:].rearrange("o j t -> o (j t)"))

    # ---------------- initial state ----------------
    mem_nrm = sp.tile([128, NT, W], F32, tag="mem_nrm")
    for c in range(NT):
        DMA.dma_start(mem_nrm[:, c, :], m0_d[128 * c:128 * (c + 1), :])
    memT_p = psA.tile([W, N], F32, tag="p")
    for c in range(NT):
        P.transpose(memT_p[:, 128 * c:128 * (c + 1)], mem_nrm[:, c, :], ident[:])
    memT = sp.tile([W, N], F32R, tag="memT")
    S.copy(memT[:], memT_p[:])

    sq0 = wp.tile([128, NT, W], F32, tag="sq0", bufs=1)
    S.square(sq0[:], mem_nrm[:])
    msum0 = wp.tile([128, NT], F32, tag="msum")
    V.tensor_reduce(msum0[:], sq0[:], axis=mybir.AxisListType.X, op=OP.add)
    mnorm = sp.tile([128, NT], F32, tag="mnorm")
    rsqrt_sb(wp, msum0[:], [128, NT], "w1", out=mnorm[:], iters=3)

    L = sp.tile([128, NT, N], F16, tag="L")
    G.memset(L[:], 0.0)
    LT0 = sp.tile([128, NT, N], F16, tag="LT")
    G.memset(LT0[:], 0.0)
    u_pm0 = sp.tile([128, NT], F32, tag="u_pm")
    G.memset(u_pm0[:], 0.0)
    rwT0 = sp.tile([128, NT * R], F32, tag="rwT")
    G.memset(rwT0[:], 0.0)

    out_sb = cp.tile([R, T, W], F32)

    st = dict(memT=memT, mem_nrm=mem_nrm, mnorm=mnorm, L=L, LT=LT0,
              u_pm=u_pm0, prec_pm=None, prec_f=None, rwT=rwT0,
              ub=None, pb=None, pbm=None)

    for t in range(T):
        st = _step(nc, tc, t, st, cp, sp, wp, psA, psM, psS,
                   ident, identh, ones, offd, nege0_pm, wkT2, rkT2, neg_er,
                   wvT, fgB, c1nB, c2B, modesB, out_sb, pexp2, rsqrt_sb,
                   lnu_dve)

    DMA.dma_start(out_d[:].rearrange("t r w -> r t w"), out_sb[:])


def _step(nc, tc, t, st, cp, sp, wp, psA, psM, psS,
          ident, identh, ones, offd, nege0_pm, wkT2, rkT2, neg_er, wvT,
          fgB, c1nB, c2B, modesB, out_sb, pexp2, rsqrt_sb, lnu_dve):
    LOW = 50000  # deprioritize slack work for the Tile scheduler
    V, S, P, G = nc.vector, nc.scalar, nc.tensor, nc.gpsimd
    memT, mem_nrm, mnorm = st["memT"], st["mem_nrm"], st["mnorm"]
    L, LT, u_pm, rwT = st["L"], st["LT"], st["u_pm"], st["rwT"]
    prec_pm, prec_f = st["prec_pm"], st["prec_f"]
    ub, pb, pbm = st["ub"], st["pb"], st["pbm"]
    last = (t == T - 1)

    # ---- write content dots (PE; memT from prev step; slack path) ----
    hp = tc.high_priority
    wdf_p = psM.tile([1, N], F32, tag="m")
    P.matmul(wdf_p[:], wkT2[:, t:t + 1], memT[:])
    wdf = wp.tile([1, N], F32, tag="wdf")
    with hp(offset=-150):
        S.copy(wdf[:], wdf_p[:])
    wdots_p = psA.tile([128, NT], F32, tag="p")
    for c in range(NT):
        P.transpose(wdots_p[:, c:c + 1], wdf[0:1, 128 * c:128 * (c + 1)],
                    ident[0:1, 0:1])

    # ---- DVE head: psi chain + rw mode scales (deps: prev-step rwT) ----
    if 0 < t:
        yyT = wp.tile([128, NT, R], F32, tag="yyT")
        V.scalar_tensor_tensor(
            yyT[:], fgB[:, None, :, t].broadcast_to([128, NT, R]), -1.0,
            rwT[:].rearrange("p (c r) -> p c r", r=R), op0=OP.mult, op1=OP.mult)
        om = wp.tile([128, NT, R], F32, tag="om")
        V.tensor_scalar(om[:], yyT[:], 1.0, None, op0=OP.add)
        p1 = wp.tile([128, NT], F32, tag="p1")
        V.tensor_tensor(p1[:], om[:, :, 0], om[:, :, 1], op=OP.mult)
        p2 = wp.tile([128, NT], F32, tag="p2")
        V.tensor_tensor(p2[:], om[:, :, 2], om[:, :, 3], op=OP.mult)
        psi_pm = wp.tile([128, NT], F32, tag="psi_pm")
        V.tensor_tensor(psi_pm[:], p1[:], p2[:], op=OP.mult)
        rwm0 = wp.tile([128, NT * R], F16, tag="rwm0")
        V.tensor_tensor(rwm0[:].rearrange("p (c r) -> p c r", r=R),
                        rwT[:].rearrange("p (c r) -> p c r", r=R),
                        modesB[:, 0, None, :, t].broadcast_to([128, NT, R]),
                        op=OP.mult)
        rwm2 = wp.tile([128, NT * R], F16, tag="rwm2")
        V.tensor_tensor(rwm2[:].rearrange("p (c r) -> p c r", r=R),
                        rwT[:].rearrange("p (c r) -> p c r", r=R),
                        modesB[:, 2, None, :, t].broadcast_to([128, NT, R]),
                        op=OP.mult)

    # ---- allocation pipeline (PM layout; ub broadcast from prev step) ----
    wlog = wp.tile([128, NT], F32, tag="wlog")
    V.tensor_tensor(wlog[:], wdots_p[:], mnorm[:], op=OP.mult)
    wexp = wp.tile([128, NT], F32, tag="wexp")
    S.activation(wexp[:], wlog[:], AF.Exp)
    if t > 0:
        lu2 = wp.tile([128, NT], F16, tag="lu2")
        with hp():
            lnu_dve(wp, u_pm[:], [128, NT], "lnu", out=lu2[:])
        Gm = wp.tile([128, NT, N], F16, tag="Gm", bufs=1)
        for c in range(NT):
            V.tensor_scalar(Gm[:, c, :], ub[:], u_pm[:, c:c + 1], None,
                            op0=OP.is_gt)
        # s in PM layout directly: s_pm[p,c] = sum_cin Gm-block^T @ lu2-col
        s_pmP = psM.tile([128, NT], F32, tag="m")
        for c in range(NT):
            for cin in range(NT):
                P.matmul(s_pmP[:, c:c + 1],
                         Gm[:, cin, 128 * c:128 * (c + 1)],
                         lu2[:, cin:cin + 1],
                         start=(cin == 0), stop=(cin == NT - 1))
        s_cl = wp.tile([128, NT], F32, tag="s_cl")
        V.tensor_scalar(s_cl[:], s_pmP[:], -87.0, None, op0=OP.max)
        es = wp.tile([128, NT], F32, tag="es")
        S.activation(es[:], s_cl[:], AF.Exp)

    wpart = wp.tile([128, 1], F32, tag="wpart")
    V.tensor_reduce(wpart[:], wexp[:], axis=mybir.AxisListType.X, op=OP.add)
    wsumB = wp.tile([128, 1], F32, tag="wsumB")
    G.partition_all_reduce(wsumB[:], wpart[:], channels=128, reduce_op=RED.add)
    wrs = wp.tile([128, 1], F32, tag="wrs")
    V.reciprocal(wrs[:], wsumB[:])
    if t > 0:
        negalloc = wp.tile([128, NT], F32, tag="negalloc")
        V.scalar_tensor_tensor(negalloc[:], u_pm[:], 1.0, es[:],
                               op0=OP.subtract, op1=OP.mult)

    # ---- write weights ww (PM primary) ----
    t_wc = wp.tile([128, NT], F32, tag="t_wc")
    V.tensor_scalar(t_wc[:], wexp[:], wrs[:], c2B[:, t:t + 1],
                    op0=OP.mult, op1=OP.mult)
    ww_pm = wp.tile([128, NT], F32, tag="ww_pm")
    swp = wp.tile([128, 1], F32, tag="swp")
    na_ap = nege0_pm[:] if t == 0 else negalloc[:]
    V.scalar_tensor_tensor(ww_pm[:], na_ap, c1nB[:, t:t + 1], t_wc[:],
                           op0=OP.mult, op1=OP.add, accum_out=swp[:])
    wwf_p = psA.tile([1, N], F32, tag="p")
    ww = wp.tile([1, N], F32, tag="ww")
    wb = wp.tile([128, N], F32, tag="wb")
    with hp():
        for c in range(NT):
            P.transpose(wwf_p[0:1, 128 * c:128 * (c + 1)], ww_pm[:, c:c + 1],
                        ident[:])
        S.copy(ww[:], wwf_p[:])
        G.partition_broadcast(wb[:], ww[:])
    swB = wp.tile([128, 1], F32, tag="swB")
    G.partition_all_reduce(swB[:], swp[:], channels=128, reduce_op=RED.add)

    # ---- prec update in PM (uses prec BEFORE update) ----
    if t == 0:
        prec_pm_n = ww_pm
        prec_f_n = ww
    elif last:
        prec_pm_n = None
        prec_f_n = None
    else:
        omsw = wp.tile([128, 1], F32, tag="omsw")
        V.tensor_scalar(omsw[:], swB[:], -1.0, 1.0, op0=OP.mult, op1=OP.add)
        prec_pm_n = sp.tile([128, NT], F32, tag="prec_pm")
        V.scalar_tensor_tensor(prec_pm_n[:], prec_pm[:], omsw[:], ww_pm[:],
                               op0=OP.mult, op1=OP.add)
        pf_p = psA.tile([1, N], F32, tag="p")
        for c in range(NT):
            P.transpose(pf_p[0:1, 128 * c:128 * (c + 1)], prec_pm_n[:, c:c + 1],
                        ident[:])
        prec_f_n = sp.tile([1, N], F32, tag="prec_f")
        S.copy(prec_f_n[:], pf_p[:])

    # ---- usage update (PM layout) ----
    if t == 0:
        u_pm_n = ww_pm
        u_f_n = ww
    elif last:
        u_pm_n = None
        u_f_n = None
    else:
        omu_pm = wp.tile([128, NT], F32, tag="omu_pm")
        V.tensor_scalar(omu_pm[:], u_pm[:], -1.0, 1.0, op0=OP.mult, op1=OP.add)
        tpm = wp.tile([128, NT], F32, tag="tpm")
        V.scalar_tensor_tensor(tpm[:], ww_pm[:], 1.0, omu_pm[:],
                               op0=OP.subtract, op1=OP.mult)
        u_pm_n = sp.tile([128, NT], F32, tag="u_pm")
        V.scalar_tensor_tensor(u_pm_n[:], tpm[:], 1.0, psi_pm[:],
                               op0=OP.add, op1=OP.mult)
        uf_p = psA.tile([1, N], F32, tag="p")
        u_f_n = sp.tile([1, N], F32, tag="u_f")
        with hp():
            for c in range(NT):
                P.transpose(uf_p[0:1, 128 * c:128 * (c + 1)],
                            u_pm_n[:, c:c + 1], ident[:])
            S.copy(u_f_n[:], uf_p[:])

    # ---- memory update ----
    keep = wp.tile([W, N], F32, tag="keep", bufs=1)
    m1 = wp.tile([W, N], F32, tag="m1", bufs=1)
    memT_n = sp.tile([W, N], F32R, tag="memT")
    with hp():
        V.tensor_scalar(keep[:], wb[0:W, :], neg_er[:, t:t + 1], 1.0,
                        op0=OP.mult, op1=OP.add)
        G.tensor_tensor(m1[:], memT[:], keep[:], op=OP.mult)
        V.scalar_tensor_tensor(memT_n[:], wb[0:W, :], wvT[:, t:t + 1], m1[:],
                               op0=OP.mult, op1=OP.add)
    mem_nrm_p = psA.tile([128, NT, W], F32, tag="p")
    for c in range(NT):
        P.transpose(mem_nrm_p[:, c, :],
                    memT_n[:, 128 * c:128 * (c + 1)].bitcast(F32),
                    ident[0:W, 0:W])
    mem_nrm_n = sp.tile([128, NT, W], F32, tag="mem_nrm")
    with hp(offset=-150):
        S.copy(mem_nrm_n[:], mem_nrm_p[:])
    sqn = wp.tile([128, NT, W], F32, tag="sqn", bufs=1)
    with hp():
        S.square(sqn[:], mem_nrm_p[:])
    msum = wp.tile([128, NT], F32, tag="msum")
    V.tensor_reduce(msum[:], sqn[:], axis=mybir.AxisListType.X, op=OP.add)
    mnorm_n = sp.tile([128, NT], F32, tag="mnorm")
    rsqrt_sb(wp, msum[:], [128, NT], "w1", out=mnorm_n[:], iters=2)

    # ---- read content dots (PE; hoisted before link transposes) ----
    rdf_p = psM.tile([R, N], F32, tag="m")
    P.matmul(rdf_p[:], rkT2[:, :, t], memT_n[:])
    rdf = wp.tile([R, N], F32, tag="rdf")
    S.copy(rdf[:], rdf_p[:])
    rdots_p = psA.tile([128, NT, R], F32, tag="p")
    for c in range(NT):
        P.transpose(rdots_p[:, c, :], rdf[:, 128 * c:128 * (c + 1)],
                    ident[0:R, 0:R])

    # ---- link update: chunks 0-1 DVE 2-STT, chunks 2-3 DVE-TS + Pool-TT ----
    if t == 0:
        L_n, LT_n = L, LT  # stays zero
    else:
        omw_pm = wp.tile([128, NT], F32, tag="omw_pm")
        V.tensor_scalar(omw_pm[:], ww_pm[:], -1.0, 1.0, op0=OP.mult, op1=OP.add)
        L_n = sp.tile([128, NT, N], F16, tag="L")
        for c in range(NT):
            Dm = wp.tile([128, N], F16, tag=f"Dm_{c % 2}", name=f"Dm_{c}")
            V.tensor_scalar(Dm[:], wb[:], omw_pm[:, c:c + 1], -1.0,
                            op0=OP.subtract, op1=OP.mult)
            q_ = wp.tile([128, N], F16, tag=f"q_{c % 2}", name=f"q_{c}")
            V.tensor_scalar(q_[:], pbm[:, c, :], ww_pm[:, c:c + 1], None,
                            op0=OP.mult)
            t1 = wp.tile([128, N], F16, tag=f"t1h_{c % 2}", name=f"t1h_{c}")
            G.tensor_tensor(t1[:], Dm[:], L[:, c, :], op=OP.mult)
            G.tensor_tensor(L_n[:, c, :], q_[:], t1[:], op=OP.add)
        LT_n = sp.tile([128, NT, N], F16, tag="LT")
        for j in range(NT):
            lt_p = psA.tile([128, N], F16, tag="plt")
            for i in range(NT):
                P.transpose(lt_p[:, 128 * i:128 * (i + 1)],
                            L_n[:, i, 128 * j:128 * (j + 1)], identh[:])
            if j < 2:
                S.copy(LT_n[:, j, :], lt_p[:])
            else:
                V.tensor_copy(LT_n[:, j, :], lt_p[:])

    # ---- read softmax ----
    rlog = wp.tile([128, NT, R], F32, tag="rlog")
    V.tensor_tensor(rlog[:], rdots_p[:],
                    mnorm_n[:, :, None].broadcast_to([128, NT, R]), op=OP.mult)
    rexp = wp.tile([128, NT, R], F32, tag="rexp")
    S.activation(rexp[:], rlog[:], AF.Exp)
    rpart = wp.tile([128, R], F32, tag="rpart")
    V.tensor_reduce(rpart[:], rexp[:].rearrange("p c r -> p r c"),
                    axis=mybir.AxisListType.X, op=OP.add)
    rsumB = wp.tile([128, R], F32, tag="rsumB")
    G.partition_all_reduce(rsumB[:], rpart[:], channels=128, reduce_op=RED.add)
    rsr = wp.tile([128, R], F32, tag="rsr")
    V.reciprocal(rsr[:], rsumB[:])
    m1rs = wp.tile([128, R], F32, tag="m1rs")
    V.tensor_tensor(m1rs[:], rsr[:], modesB[:, 1, :, t], op=OP.mult)
    rexp_s = wp.tile([128, NT, R], F32, tag="rexp_s")
    V.tensor_tensor(rexp_s[:], rexp[:],
                    m1rs[:, None, :].broadcast_to([128, NT, R]), op=OP.mult)

    # ---- read weights: bwd+fwd in PSUM; content merged in PM after ----
    rwT_n = sp.tile([128, NT * R], F32, tag="rwT")
    rexp_v = rexp_s[:].rearrange("p c r -> p (c r)")
    if t > 0:
        rw_p = psS.tile([R, N], F32, tag="rw")
        for c in range(NT):
            P.matmul(rw_p[:], rwm0[:, R * c:R * (c + 1)], L_n[:, c, :],
                     start=(c == 0), stop=False)
        for c in range(NT):
            P.matmul(rw_p[:], rwm2[:, R * c:R * (c + 1)], LT_n[:, c, :],
                     start=False, stop=(c == NT - 1))
        rw = wp.tile([R, N], F32, tag="rwf")
        rwT_p = psA.tile([128, NT * R], F32, tag="p")
        with hp():
            S.copy(rw[:], rw_p[:])
            for c in range(NT):
                P.transpose(rwT_p[:, R * c:R * (c + 1)],
                            rw[:, 128 * c:128 * (c + 1)], ident[0:R, 0:R])
            V.tensor_tensor(rwT_n[:], rwT_p[:], rexp_v, op=OP.add)
    else:
        V.tensor_copy(rwT_n[:], rexp_v)

    # ---- read words ----
    rwd_p = psS.tile([R, W], F32, tag="s")
    for c in range(NT):
        P.matmul(rwd_p[:], rwT_n[:, R * c:R * (c + 1)], mem_nrm_n[:, c, :],
                 start=(c == 0), stop=(c == NT - 1))
    with hp(offset=-150):
        S.copy(out_sb[:, t, :], rwd_p[:])

    # ---- tail: broadcasts for the NEXT step (hoisted into this step) ----
    ub_n = pb_n = pbm_n = None
    if not last:
        ub_n = wp.tile([128, N], F32, tag="ub")
        pb_n = wp.tile([128, N], F32, tag="pb")
        with hp():
            G.partition_broadcast(ub_n[:], u_f_n[:])
            G.partition_broadcast(pb_n[:], prec_f_n[:])
        pbm_n = wp.tile([128, NT, N], F16, tag="pbm")
        for c in range(NT):
            G.tensor_tensor(pbm_n[:, c, :], pb_n[:], offd[:, c, :], op=OP.mult)

    return dict(memT=memT_n, mem_nrm=mem_nrm_n, mnorm=mnorm_n, L=L_n, LT=LT_n,
                u_pm=u_pm_n, prec_pm=prec_pm_n, prec_f=prec_f_n, rwT=rwT_n,
                ub=ub_n, pb=pb_n, pbm=pbm_n)


# ---------------------------------------------------------------------------
_NC_CACHE = {}


def _get_nc():
    if "nc" not in _NC_CACHE:
        _NC_CACHE["nc"] = build_nc()
    return _NC_CACHE["nc"]


def _consts():
    ident = np.eye(128, dtype=np.float32)
    identh = np.eye(128, dtype=np.float16)
    ones = np.ones((128, 128), dtype=np.float32)
    offd = (1.0 - np.eye(N)).astype(np.float16)
    return ident, identh, ones, offd


def make_in_maps(controller_output, W_if, b_if, memory0):
    ident, identh, ones, offd = _consts()
    maps = []
    for b in range(B):
        maps.append({
            "co": np.ascontiguousarray(controller_output[b]),
            "wif": np.ascontiguousarray(W_if),
            "bif": np.ascontiguousarray(b_if.reshape(1, IF)),
            "mem0": np.ascontiguousarray(memory0[b]),
            "ident": ident, "identh": identh, "ones": ones, "offdiag": offd,
        })
    return maps


def kernel(controller_output, W_if, b_if, memory0):
    from concourse.bass_utils import run_bass_kernel_spmd
    controller_output = np.asarray(controller_output, dtype=np.float32)
    W_if = np.asarray(W_if, dtype=np.float32)
    b_if = np.asarray(b_if, dtype=np.float32)
    memory0 = np.asarray(memory0, dtype=np.float32)
    nc = _get_nc()
    maps = make_in_maps(controller_output, W_if, b_if, memory0)
    res = run_bass_kernel_spmd(nc, maps, core_ids=list(range(B)))
    return np.stack([res.results[b]["out"] for b in range(B)], axis=0)


if __name__ == "__main__":
    mode = sys.argv[1] if len(sys.argv) > 1 else "sim"
    sys.path.insert(0, "/root/problem")
    import jax
    with jax.default_device(jax.devices("cpu")[0]):
        import reference
        inputs = {k: np.asarray(v) for k, v in reference.setup_inputs().items()}
        expected = np.asarray(reference.reference(**inputs))

    if mode == "sim":
        from concourse.bass_interp import CoreSim
        nc = build_nc()
        maps = make_in_maps(inputs["controller_output"], inputs["W_if"],
                            inputs["b_if"], inputs["memory0"])
        sim = CoreSim(nc)
        for k, v in maps[0].items():
            sim.tensor(k)[:] = v
        sim.simulate()
        got = sim.tensor("out").copy()
        exp = expected[0]
        err = np.abs(got - exp)
        rel = np.linalg.norm(got - exp) / (np.linalg.norm(exp) + 1e-12)
        print("sim modeled time (ns):", sim.time)
        print("max abs err:", err.max(), " rel err:", rel)
    else:
        got = kernel(**inputs)
        rel = np.linalg.norm(got - expected) / (np.linalg.norm(expected) + 1e-12)
        print("max abs err:", np.abs(got - expected).max(), " rel err:", rel)
